# revision 1
# baseline (speedup 1.0000x reference)
"""ConvLRUBlock Trainium2 kernel.

Reference computation (per batch b):
    h   = rms_norm(x, norm_w)                  # over channel dim
    uv  = conv3d_3x3x3(h, w_in) + b_in         # pad: replicate T/H, circular W
    u   = silu(a) * g          (a, g = uv split on channels)
    y_t = Re(h_t) c_re + Im(h_t) c_im,  h_t = lam h_{t-1} + gamma u_t  (diag LRU)
    out = x + conv3d_3x3x3(y, w_out) + b_out

Sharding: 8 cores = (batch 2) x (H quarters 4). Each core receives its H
slice plus 2 halo rows each side (edge-replicated) and the W dim circularly
padded to W+2, so no inter-core communication is needed. All conv padding is
resolved by host-side halo materialization + in-kernel index clamping (T) +
in-SBUF wrap-column fixes (W for the second conv).

In-kernel layout: channels (96) on SBUF partitions; spatial (rows x (W+2))
flattened on the free dim. 3x3x3 convs = 27 accumulating matmuls per output
tile; kh/kw become column shifts of the rhs AP, kt picks one of 3 t-slabs.
The LRU scan is 16 sequential complex steps on the vector engine.
"""

import os
from contextlib import ExitStack

import ml_dtypes
import numpy as np

import concourse.bacc as bacc
import concourse.bass as bass  # noqa: F401
import concourse.tile as tile
from concourse import mybir

F32 = mybir.dt.float32
BF16 = mybir.dt.bfloat16
ALU = mybir.AluOpType
AF = mybir.ActivationFunctionType

EPS = 1e-6

# Full-problem constants
B_FULL, C_FULL, T_FULL, H_FULL, W_FULL = 2, 96, 16, 64, 128
QH = 4  # H quarters
N_CORES = 8


def build_program(C=96, T=16, HR=16, W=128, CT=512, use_silu=True,
                  pack=False, pack2=False):
    """Build the single-core SPMD Bass program.

    C: channels; T: time steps; HR: output H rows per core; W: width.
    CT: matmul/psum column tile (<=512). use_silu: Silu on ACT vs
    Sigmoid+mults (the simulator does not implement Silu).
    """
    Wp = W + 2           # circular-padded width
    RIN = HR + 4         # input rows (2 halo each side, for two convs)
    RU = HR + 2          # u/y rows (1 halo each side, for conv_out)
    NIN = RIN * Wp       # flattened input cols per t
    NU = RU * Wp         # flattened u/y cols per t
    NO = HR * Wp         # flattened output cols per t

    nc = bacc.Bacc()
    xh = nc.declare_dram_parameter("xh", [C, T, RIN, Wp], F32, isOutput=False)
    w_in = nc.declare_dram_parameter("w_in", [C, 27, 2 * C], BF16, isOutput=False)
    w_out = nc.declare_dram_parameter("w_out", [C, 27, C], BF16, isOutput=False)
    onesw = nc.declare_dram_parameter("onesw", [C, 128], BF16, isOutput=False)
    consts = nc.declare_dram_parameter("consts", [C, 13], F32, isOutput=False)
    consts2 = nc.declare_dram_parameter("consts2", [128, 2], F32, isOutput=False)
    out = nc.declare_dram_parameter("out", [C, T, HR, W], F32, isOutput=True)

    def col_tiles(total):
        return [(i, min(CT, total - i)) for i in range(0, total, CT)]

    with tile.TileContext(nc) as tc, ExitStack() as ctx:
        singles = ctx.enter_context(tc.tile_pool(name="singles", bufs=1))
        xpool = ctx.enter_context(tc.tile_pool(name="xpool", bufs=2))
        sqpool = ctx.enter_context(tc.tile_pool(name="sqpool", bufs=2))
        statpool = ctx.enter_context(tc.tile_pool(name="statpool", bufs=2))
        hnpool = ctx.enter_context(tc.tile_pool(name="hnpool", bufs=4))
        sapool = ctx.enter_context(tc.tile_pool(name="sapool", bufs=3))
        bpool = ctx.enter_context(tc.tile_pool(name="bpool", bufs=2))
        hrpool = ctx.enter_context(tc.tile_pool(name="hrpool", bufs=2))
        hipool = ctx.enter_context(tc.tile_pool(name="hipool", bufs=2))
        tmppool = ctx.enter_context(tc.tile_pool(name="tmppool", bufs=2))
        ypool = ctx.enter_context(tc.tile_pool(name="ypool", bufs=4))
        opool = ctx.enter_context(tc.tile_pool(name="opool", bufs=2))
        psN = ctx.enter_context(tc.tile_pool(name="psN", bufs=2, space="PSUM"))
        psA = ctx.enter_context(tc.tile_pool(name="psA", bufs=2, space="PSUM"))
        psG = ctx.enter_context(tc.tile_pool(name="psG", bufs=2, space="PSUM"))
        psO = ctx.enter_context(tc.tile_pool(name="psO", bufs=2, space="PSUM"))

        sb_win = singles.tile([C, 27, 2 * C], BF16)
        nc.sync.dma_start(out=sb_win[:], in_=w_in[:])
        sb_wout = singles.tile([C, 27, C], BF16)
        nc.sync.dma_start(out=sb_wout[:], in_=w_out[:])
        sb_ones = singles.tile([C, 128], BF16)
        nc.sync.dma_start(out=sb_ones[:], in_=onesw[:])
        sb_c = singles.tile([C, 13], F32)
        nc.sync.dma_start(out=sb_c[:], in_=consts[:])
        sb_c2 = singles.tile([128, 2], F32)
        nc.sync.dma_start(out=sb_c2[:], in_=consts2[:])
        c_aux0 = sb_c2[:, 0:1]
        c_aux1 = sb_c2[:, 1:2]
        c_ba = sb_c[:, 0:1]
        c_bg = sb_c[:, 1:2]
        c_lr = sb_c[:, 2:3]
        c_li = sb_c[:, 3:4]
        c_nli = sb_c[:, 4:5]
        c_gcre = sb_c[:, 5:6]
        c_gcim = sb_c[:, 6:7]
        c_bout = sb_c[:, 7:8]
        c_m0 = sb_c[:, 8:9]
        c_1m0 = sb_c[:, 9:10]
        c_m1 = sb_c[:, 10:11]
        c_1m1 = sb_c[:, 11:12]
        c_eps = sb_c[:, 12:13]

        # Warm-up reads: make each compute engine observe the const-DMA
        # semaphores early, so steady-state ops carry at most one sync wait
        # (walrus rejects DVE ops with two wait commands).
        wu_v = singles.tile([C, 13], F32)
        nc.vector.tensor_copy(wu_v[:], sb_c[:])
        wu_s = singles.tile([C, 13], F32)
        nc.scalar.activation(wu_s[:], sb_c[:], AF.Square)

        touchpool = ctx.enter_context(tc.tile_pool(name="touchpool", bufs=2))
        if pack:
            gspool = ctx.enter_context(tc.tile_pool(name="gspool", bufs=2))
            gfpool = ctx.enter_context(tc.tile_pool(name="gfpool", bufs=2))
        if pack2:
            piecepool = ctx.enter_context(tc.tile_pool(name="piecepool", bufs=2))

        def touch(ap, engines="v"):
            """Tiny read of a freshly-DMA'd tile so the engine observes the
            DMA-queue semaphore here; later big consumers then carry only
            engine-sem waits (walrus rejects DVE ops with 2 sync waits)."""
            if "v" in engines:
                tv = touchpool.tile([C, 1], F32, tag="tv")
                nc.vector.tensor_copy(tv[:], ap)
            if "s" in engines:
                ts_ = touchpool.tile([C, 1], F32, tag="ts")
                nc.scalar.activation(ts_[:], ap, AF.Square)

        hn_slabs = [None] * T  # hnorm tiles, data at col offset 1
        y_slabs = [None] * T   # y tiles (bf16), data at col offset 1

        def stage_a(t):
            """x[t] -> hnorm[t] (rms-normed, bf16, [C, 1+NIN+1])."""
            xt = xpool.tile([C, RIN, Wp], F32, tag="xt")
            nc.sync.dma_start(out=xt[:], in_=xh[:, t])
            touch(xt[:, 0, 0:1], engines="vs")
            xf = xt[:].rearrange("p r w -> p (r w)")
            hn = hnpool.tile([C, 1 + NIN + 1], BF16, tag="hn")
            nc.vector.memset(hn[:, 0:1], 0.0)
            nc.vector.memset(hn[:, 1 + NIN:], 0.0)
            for c0, n in col_tiles(NIN):
                sq = sqpool.tile([C, CT], BF16, tag="sq")
                nc.scalar.activation(sq[:, :n], xf[:, c0:c0 + n], AF.Square)
                ps = psN.tile([128, CT], F32, tag="psn")
                nc.tensor.matmul(ps[:, :n], sb_ones[:], sq[:, :n],
                                 start=True, stop=True)
                lg = statpool.tile([C, CT], F32, tag="lg")
                nc.scalar.activation(lg[:, :n], ps[:C, :n], AF.Ln,
                                     scale=1.0 / C, bias=c_eps)
                inv = statpool.tile([C, CT], F32, tag="inv")
                nc.scalar.activation(inv[:, :n], lg[:, :n], AF.Exp, scale=-0.5)
                nc.vector.tensor_mul(hn[:, 1 + c0:1 + c0 + n],
                                     xf[:, c0:c0 + n], inv[:, :n])
            hn_slabs[t] = hn
            return hn

        def gate_epilogue(pa, pg_sb, bt, c0, n):
            """silu(a+ba)*(g+bg) for one coltile; a=pa[0:C] (psum),
            g already realigned to pg_sb [C, n] (sbuf)."""
            if use_silu:
                sa = sapool.tile([C, CT], BF16, tag="sa")
                nc.scalar.activation(sa[:, :n], pa[:C, :n], AF.Silu,
                                     bias=c_ba)
            else:
                sg = sapool.tile([C, CT], BF16, tag="sg")
                nc.scalar.activation(sg[:, :n], pa[:C, :n], AF.Sigmoid,
                                     bias=c_ba)
                av = sapool.tile([C, CT], F32, tag="av")
                nc.vector.scalar_tensor_tensor(av[:, :n], pa[:C, :n], c_ba,
                                               sg[:, :n], ALU.add,
                                               ALU.bypass)
                sa = sapool.tile([C, CT], BF16, tag="sa")
                nc.vector.tensor_mul(sa[:, :n], sg[:, :n], av[:, :n])
            nc.vector.scalar_tensor_tensor(bt[:, c0:c0 + n], pg_sb[:, :n],
                                           c_bg, sa[:, :n],
                                           ALU.add, ALU.mult)

        def conv_in_packed(t):
            """1.5-array-pass conv_in: pass1 M=128 (a0..95,g0..31), pass2
            col-tiled pairs of M=64 (g32..95) for two coltiles at once."""
            slabs = [hn_slabs[min(max(t + kt - 1, 0), T - 1)] for kt in range(3)]
            bt = bpool.tile([C, NU], BF16, tag="bt")
            cts = col_tiles(NU)
            for p0 in range(0, len(cts), 2):
                pair = cts[p0:p0 + 2]
                pas = []
                for c0, n in pair:
                    pa = psA.tile([128, CT], F32, tag="pa")
                    for kt in range(3):
                        rhs_all = slabs[kt]
                        for kh in range(3):
                            for kw in range(3):
                                off = kt * 9 + kh * 3 + kw
                                s = 1 + c0 + kh * Wp + kw - 1
                                nc.tensor.matmul(
                                    pa[:, :n], sb_win[:, off, 0:128],
                                    rhs_all[:, s:s + n],
                                    start=(off == 0), stop=(off == 26))
                    pas.append(pa)
                pg = psG.tile([128, CT], F32, tag="pg")
                for kt in range(3):
                    rhs_all = slabs[kt]
                    for kh in range(3):
                        for kw in range(3):
                            off = kt * 9 + kh * 3 + kw
                            for j, (c0, n) in enumerate(pair):
                                s = 1 + c0 + kh * Wp + kw - 1
                                b = 64 * j
                                nc.tensor.matmul(
                                    pg[b:b + 64, :n],
                                    sb_win[:, off, 128:192],
                                    rhs_all[:, s:s + n],
                                    start=(off == 0), stop=(off == 26),
                                    tile_position=(0, b),
                                    skip_group_check=True)
                for j, (c0, n) in enumerate(pair):
                    b = 64 * j
                    pa = pas[j]
                    gsa = gspool.tile([128, CT], F32, tag="gsa")
                    nc.vector.tensor_copy(gsa[96:128, :n], pa[96:128, :n])
                    gsb = gspool.tile([128, CT], F32, tag="gsb")
                    nc.vector.tensor_copy(gsb[b:b + 64, :n], pg[b:b + 64, :n])
                    gf = gfpool.tile([C, CT], F32, tag="gf")
                    nc.sync.dma_start(out=gf[0:32, :n], in_=gsa[96:128, :n])
                    nc.sync.dma_start(out=gf[32:96, :n], in_=gsb[b:b + 64, :n])
                    gate_epilogue(pa, gf, bt, c0, n)
            return bt

        def conv_in(t):
            """hnorm[t-1..t+1] -> b[t] = silu(a+ba)*(g+bg), bf16 [C, NU]."""
            slabs = [hn_slabs[min(max(t + kt - 1, 0), T - 1)] for kt in range(3)]
            bt = bpool.tile([C, NU], BF16, tag="bt")
            for c0, n in col_tiles(NU):
                pa = psA.tile([C, CT], F32, tag="pa")
                pg = psG.tile([C, CT], F32, tag="pg")
                for kt in range(3):
                    rhs_all = slabs[kt]
                    for kh in range(3):
                        for kw in range(3):
                            off = kt * 9 + kh * 3 + kw
                            s = 1 + c0 + kh * Wp + kw - 1
                            rhs = rhs_all[:, s:s + n]
                            nc.tensor.matmul(pa[:, :n], sb_win[:, off, 0:C],
                                             rhs, start=(off == 0),
                                             stop=(off == 26))
                for kt in range(3):
                    rhs_all = slabs[kt]
                    for kh in range(3):
                        for kw in range(3):
                            off = kt * 9 + kh * 3 + kw
                            s = 1 + c0 + kh * Wp + kw - 1
                            rhs = rhs_all[:, s:s + n]
                            nc.tensor.matmul(pg[:, :n], sb_win[:, off, C:2 * C],
                                             rhs, start=(off == 0),
                                             stop=(off == 26))
                gate_epilogue(pa, pg, bt, c0, n)
            return bt

        scan_state = [None, None]  # hr, hi tiles [C, NU] f32

        def scan_step(t, bt):
            """LRU step + projection -> y[t] (bf16 slab, data at offset 1)."""
            hr_new = hrpool.tile([C, NU], F32, tag="hr")
            hi_new = hipool.tile([C, NU], F32, tag="hi")
            if t == 0:
                nc.vector.tensor_copy(hr_new[:], bt[:])
                nc.vector.memset(hi_new[:], 0.0)
            else:
                hr_old, hi_old = scan_state
                t1 = tmppool.tile([C, NU], F32, tag="tA")
                nc.vector.scalar_tensor_tensor(t1[:], hi_old[:], c_nli, bt[:],
                                               ALU.mult, ALU.add)
                nc.vector.scalar_tensor_tensor(hr_new[:], hr_old[:], c_lr,
                                               t1[:], ALU.mult, ALU.add)
                t2 = tmppool.tile([C, NU], F32, tag="tB")
                nc.vector.scalar_tensor_tensor(t2[:], hi_old[:], c_lr,
                                               hi_old[:], ALU.mult, ALU.bypass)
                nc.vector.scalar_tensor_tensor(hi_new[:], hr_old[:], c_li,
                                               t2[:], ALU.mult, ALU.add)
            scan_state[0], scan_state[1] = hr_new, hi_new
            t3 = tmppool.tile([C, NU], F32, tag="tA")
            nc.vector.scalar_tensor_tensor(t3[:], hr_new[:], c_gcre,
                                           hr_new[:], ALU.mult, ALU.bypass)
            yt = ypool.tile([C, 1 + NU + 1], BF16, tag="yt")
            nc.vector.memset(yt[:, 0:1], 0.0)
            nc.vector.memset(yt[:, 1 + NU:], 0.0)
            nc.vector.scalar_tensor_tensor(yt[:, 1:1 + NU], hi_new[:], c_gcim,
                                           t3[:], ALU.mult, ALU.add)
            # W wrap columns: col 0 <- col W (w=W-1), col W+1 <- col 1 (w=0)
            yv = yt[:, 1:1 + NU].rearrange("p (r w) -> p r w", w=Wp)
            nc.vector.tensor_copy(yv[:, :, 0:1], yv[:, :, W:W + 1])
            nc.vector.tensor_copy(yv[:, :, W + 1:W + 2], yv[:, :, 1:2])
            # H edge replication (active only on global-edge cores, via mask):
            # row0 <- m0*row0 + (1-m0)*row1 ; last <- m1*last + (1-m1)*prev
            e0 = tmppool.tile([C, Wp], F32, tag="tE")
            nc.vector.scalar_tensor_tensor(e0[:], yv[:, 1, :], c_1m0,
                                           yv[:, 1, :], ALU.mult, ALU.bypass)
            nc.vector.scalar_tensor_tensor(yv[:, 0, :], yv[:, 0, :], c_m0,
                                           e0[:], ALU.mult, ALU.add)
            e1 = tmppool.tile([C, Wp], F32, tag="tE")
            nc.vector.scalar_tensor_tensor(e1[:], yv[:, RU - 2, :], c_1m1,
                                           yv[:, RU - 2, :], ALU.mult, ALU.bypass)
            nc.vector.scalar_tensor_tensor(yv[:, RU - 1, :], yv[:, RU - 1, :],
                                           c_m1, e1[:], ALU.mult, ALU.add)
            y_slabs[t] = yt
            return yt

        def conv_out(t):
            """y[t-1..t+1] -> out[t] = x + conv(y) + b_out."""
            slabs = [y_slabs[min(max(t + kt - 1, 0), T - 1)] for kt in range(3)]
            ot = opool.tile([C, HR, Wp], F32, tag="ot")
            # residual input loaded into the output staging tile
            nc.sync.dma_start(out=ot[:], in_=xh[:, t, 2:2 + HR, :])
            touch(ot[:, 0, 0:1], engines="v")
            of = ot[:].rearrange("p r w -> p (r w)")
            for c0, n in col_tiles(NO):
                po = psO.tile([C, CT], F32, tag="po")
                for kt in range(3):
                    rhs_all = slabs[kt]
                    for kh in range(3):
                        for kw in range(3):
                            off = kt * 9 + kh * 3 + kw
                            s = 1 + c0 + kh * Wp + kw - 1
                            rhs = rhs_all[:, s:s + n]
                            nc.tensor.matmul(po[:, :n], sb_wout[:, off, :],
                                             rhs, start=(off == 0),
                                             stop=(off == 26))
                nc.vector.scalar_tensor_tensor(of[:, c0:c0 + n], po[:, :n],
                                               c_bout, of[:, c0:c0 + n],
                                               ALU.add, ALU.add)
            nc.sync.dma_start(out=out[:, t], in_=ot[:, :, 1:1 + W])

        def conv_out_packed(t):
            """conv_out with array packing: pairs (e,o) put e's 96 channels +
            o's first 32 (pos 96) in one pass; the two pairs' leftover 64
            channels share one concurrent col-tiled pass. Misplaced pieces are
            realigned into the staging tile via SBUF->SBUF DMA."""
            slabs = [y_slabs[min(max(t + kt - 1, 0), T - 1)] for kt in range(3)]
            ot = opool.tile([C, HR, Wp], F32, tag="ot")
            nc.sync.dma_start(out=ot[:], in_=xh[:, t, 2:2 + HR, :])
            touch(ot[:, 0, 0:1], engines="v")
            of = ot[:].rearrange("p r w -> p (r w)")
            xflat = xh[:, t, 2:2 + HR, :].rearrange("p r w -> p (r w)")
            cts = col_tiles(NO)

            def mm_group(ps, prange, wslice, c0, n, pos):
                for kt in range(3):
                    rhs_all = slabs[kt]
                    for kh in range(3):
                        for kw in range(3):
                            off = kt * 9 + kh * 3 + kw
                            s = 1 + c0 + kh * Wp + kw - 1
                            nc.tensor.matmul(
                                ps[prange[0]:prange[1], :n],
                                sb_wout[:, off, wslice[0]:wslice[1]],
                                rhs_all[:, s:s + n],
                                start=(off == 0), stop=(off == 26),
                                tile_position=pos, skip_group_check=True)

            def mm_group2(ps, jobs):
                """Interleaved concurrent accumulation groups."""
                for kt in range(3):
                    rhs_all = slabs[kt]
                    for kh in range(3):
                        for kw in range(3):
                            off = kt * 9 + kh * 3 + kw
                            for prange, wslice, c0, n, pos in jobs:
                                s = 1 + c0 + kh * Wp + kw - 1
                                nc.tensor.matmul(
                                    ps[prange[0]:prange[1], :n],
                                    sb_wout[:, off, wslice[0]:wslice[1]],
                                    rhs_all[:, s:s + n],
                                    start=(off == 0), stop=(off == 26),
                                    tile_position=pos, skip_group_check=True)

            def aligned_epi(ps, c0, n):
                nc.vector.scalar_tensor_tensor(of[:, c0:c0 + n], ps[:C, :n],
                                               c_bout, of[:, c0:c0 + n],
                                               ALU.add, ALU.add)

            def piece_epi(ps, pbase, psize, ch0, c0, n):
                """Residual-add for a channel piece at partitions
                [pbase, pbase+psize) holding channels [ch0, ch0+psize);
                realign into ot via DMA."""
                xp = piecepool.tile([128, CT], F32, tag="xp")
                nc.sync.dma_start(out=xp[pbase:pbase + psize, :n],
                                  in_=xflat[ch0:ch0 + psize, c0:c0 + n])
                aux = c_aux0 if pbase == 96 else c_aux1
                pt = piecepool.tile([128, CT], F32, tag="pc")
                nc.vector.scalar_tensor_tensor(
                    pt[pbase:pbase + psize, :n],
                    ps[pbase:pbase + psize, :n],
                    aux[pbase:pbase + psize, :],
                    xp[pbase:pbase + psize, :n], ALU.add, ALU.add)
                nc.sync.dma_start(out=of[ch0:ch0 + psize, c0:c0 + n],
                                  in_=pt[pbase:pbase + psize, :n])

            for e, o in ((0, 1), (2, 3)):
                (ce, ne), (co_, no_) = cts[e], cts[o]
                p1 = psO.tile([128, CT], F32, tag="po")
                mm_group2(p1, [((0, 32), (0, 32), ce, ne, (0, 0)),
                               ((32, 64), (32, 64), ce, ne, (0, 32)),
                               ((64, 96), (64, 96), ce, ne, (0, 64)),
                               ((96, 128), (0, 32), co_, no_, (0, 96))])
                aligned_epi(p1, ce, ne)
                piece_epi(p1, 96, 32, 0, co_, no_)
            p2 = psG.tile([128, CT], F32, tag="pg")
            mm_group2(p2, [((0, 32), (32, 64), cts[1][0], cts[1][1], (0, 0)),
                           ((32, 64), (64, 96), cts[1][0], cts[1][1], (0, 32)),
                           ((64, 96), (32, 64), cts[3][0], cts[3][1], (0, 64)),
                           ((96, 128), (64, 96), cts[3][0], cts[3][1], (0, 96))])
            piece_epi(p2, 0, 64, 32, cts[1][0], cts[1][1])
            piece_epi(p2, 64, 64, 32, cts[3][0], cts[3][1])
            p4 = psO.tile([128, CT], F32, tag="po")
            mm_group(p4, (0, 96), (0, 96), cts[4][0], cts[4][1], (0, 0))
            aligned_epi(p4, cts[4][0], cts[4][1])
            nc.sync.dma_start(out=out[:, t], in_=ot[:, :, 1:1 + W])

        octs = col_tiles(NO)
        use_p2 = pack2 and len(octs) == 5 and all(n == CT for _, n in octs[:4])
        co_fn = conv_out_packed if use_p2 else conv_out

        stage_a(0)
        if T > 1:
            stage_a(1)
        for t in range(T):
            if t + 1 < T:
                stage_a(t + 1)
            bt = conv_in_packed(t) if pack else conv_in(t)
            scan_step(t, bt)
            if t >= 1:
                co_fn(t - 1)
        co_fn(T - 1)

    nc.compile()
    return nc


def build_program_k(C=96, T=16, HR=16, W=128, CT=512):
    """K=128-packed SPMD program.

    Each conv's 27-tap x 96-ch contraction (2592 rows) is regrouped into 21
    matmul streams per output tile instead of 27:
      - 9 A-streams: [slab_prev ch0..95 ; slab_cur ch0..31] (dup rows DMA'd
        into partitions 96..127 of the prev slab tile), one per (kh,kw).
      - 9 B-streams: Q-tile = [slab_cur ch32..95 ; slab_next ch0..63].
      - 3 R-streams: per-kh slabs pack slab_next ch64..95 at the 3 kw
        column shifts (K=96).
    t=0 uses special A-weights (kt0+kt1 folded for ch0..31, K=96) because
    the clamped prev slab's dup rows hold the wrong timestep.

    Pipeline: normalization (stage) runs 3 timesteps ahead, slab-combining
    DMAs (build_in) 1 ahead with a full iteration of slack; x**2 runs on
    DVE and rms uses Sqrt(ACT)+reciprocal(DVE) so the ACT engine only ever
    holds the Sqrt+Silu tables (no table thrash) and the stage matmuls
    never block the PE FIFO.
    """
    Wp = W + 2
    RIN = HR + 4
    RU = HR + 2
    NIN = RIN * Wp
    NU = RU * Wp
    NO = HR * Wp
    C2 = 2 * C

    nc = bacc.Bacc()
    xh = nc.declare_dram_parameter("xh", [C, T, RIN, Wp], F32, isOutput=False)
    wA = nc.declare_dram_parameter("wA", [128, 9, C2], BF16, isOutput=False)
    wB = nc.declare_dram_parameter("wB", [128, 9, C2], BF16, isOutput=False)
    wR = nc.declare_dram_parameter("wR", [96, 3, C2], BF16, isOutput=False)
    wA0 = nc.declare_dram_parameter("wA0", [96, 9, C2], BF16, isOutput=False)
    wK = nc.declare_dram_parameter("wK", [C, 27, C], BF16, isOutput=False)
    onesw = nc.declare_dram_parameter("onesw", [C, 128], BF16, isOutput=False)
    consts = nc.declare_dram_parameter("consts", [C, 13], F32, isOutput=False)
    out = nc.declare_dram_parameter("out", [C, T, HR, W], F32, isOutput=True)

    def col_tiles(total):
        return [(i, min(CT, total - i)) for i in range(0, total, CT)]

    with tile.TileContext(nc) as tc, ExitStack() as ctx:
        singles = ctx.enter_context(tc.tile_pool(name="singles", bufs=1))
        xpool = ctx.enter_context(tc.tile_pool(name="xpool", bufs=2))
        sqpool = ctx.enter_context(tc.tile_pool(name="sqpool", bufs=2))
        statpool = ctx.enter_context(tc.tile_pool(name="statpool", bufs=2))
        hnpool = ctx.enter_context(tc.tile_pool(name="hnpool", bufs=5))
        qinpool = ctx.enter_context(tc.tile_pool(name="qinpool", bufs=2))
        rinpool = ctx.enter_context(tc.tile_pool(name="rinpool", bufs=2))
        sapool = ctx.enter_context(tc.tile_pool(name="sapool", bufs=2))
        bpool = ctx.enter_context(tc.tile_pool(name="bpool", bufs=2))
        tmppool = ctx.enter_context(tc.tile_pool(name="tmppool", bufs=1))
        ypool = ctx.enter_context(tc.tile_pool(name="ypool", bufs=4))
        opool = ctx.enter_context(tc.tile_pool(name="opool", bufs=1))
        gspool = ctx.enter_context(tc.tile_pool(name="gspool", bufs=1))
        gfpool = ctx.enter_context(tc.tile_pool(name="gfpool", bufs=2))
        touchpool = ctx.enter_context(tc.tile_pool(name="touchpool", bufs=2))
        psN = ctx.enter_context(tc.tile_pool(name="psN", bufs=1, space="PSUM"))
        psA = ctx.enter_context(tc.tile_pool(name="psA", bufs=2, space="PSUM"))
        psG = ctx.enter_context(tc.tile_pool(name="psG", bufs=2, space="PSUM"))
        psO = ctx.enter_context(tc.tile_pool(name="psO", bufs=3, space="PSUM"))

        sb_c = singles.tile([C, 13], F32)
        nc.sync.dma_start(out=sb_c[:], in_=consts[:])
        sb_ones = singles.tile([C, 128], BF16)
        nc.sync.dma_start(out=sb_ones[:], in_=onesw[:])
        sb_wA = singles.tile([128, 9, C2], BF16)
        sb_wB = singles.tile([128, 9, C2], BF16)
        sb_wR = singles.tile([96, 3, C2], BF16)
        sb_wA0 = singles.tile([96, 9, C2], BF16)
        sb_wK = singles.tile([C, 27, C], BF16)

        def emit_weight_dmas():
            nc.sync.dma_start(out=sb_wA0[:], in_=wA0[:])
            nc.sync.dma_start(out=sb_wA[:], in_=wA[:])
            nc.sync.dma_start(out=sb_wB[:], in_=wB[:])
            nc.sync.dma_start(out=sb_wR[:], in_=wR[:])
            nc.sync.dma_start(out=sb_wK[:], in_=wK[:])

        c_ba = sb_c[:, 0:1]
        c_bg = sb_c[:, 1:2]
        c_lr = sb_c[:, 2:3]
        c_li = sb_c[:, 3:4]
        c_nli = sb_c[:, 4:5]
        c_gcre = sb_c[:, 5:6]
        c_gcim = sb_c[:, 6:7]
        c_bout = sb_c[:, 7:8]
        c_m0 = sb_c[:, 8:9]
        c_1m0 = sb_c[:, 9:10]
        c_m1 = sb_c[:, 10:11]
        c_1m1 = sb_c[:, 11:12]
        c_eps = sb_c[:, 12:13]

        # Warm-ups: observe const DMA on each engine and preload the only
        # two ACT tables used in steady state (Sqrt, Silu).
        wu_v = singles.tile([C, 13], F32)
        nc.vector.tensor_copy(wu_v[:], sb_c[:])
        wu_s = singles.tile([C, 13], F32)
        nc.scalar.activation(wu_s[:], sb_c[:], AF.Exp)
        wu_s2 = singles.tile([C, 13], BF16)
        nc.scalar.activation(wu_s2[:], sb_c[:], AF.Silu, bias=c_eps)

        def touch(ap, engines="v"):
            if "v" in engines:
                tv = touchpool.tile([C, 1], F32, tag="tv")
                nc.vector.tensor_copy(tv[:], ap)

        hn_slabs = [None] * T   # [128, 1+NIN+1] bf16; rows 96:128 = dup
        sq_tiles = [None] * T
        qin_tiles = [None] * T
        rin_tiles = [None] * T
        y_slabs = [None] * T    # [128, 1+NU+1] bf16

        def stage_sq(t):
            """x[t] load + x**2 on DVE (feeds the rms matmul much later)."""
            xt = xpool.tile([C, RIN, Wp], F32, tag="xt")
            nc.sync.dma_start(out=xt[:], in_=xh[:, t])
            xf = xt[:].rearrange("p r w -> p (r w)")
            sq = sqpool.tile([C, NIN], BF16, tag="sq")
            for c0, n in col_tiles(NIN):
                nc.vector.tensor_mul(sq[:, c0:c0 + n], xf[:, c0:c0 + n],
                                     xf[:, c0:c0 + n])
            sq_tiles[t] = (xt, sq)

        def stage_rest(t):
            """rms reduce (PE) + Sqrt (ACT) + reciprocal (DVE) + hn mul."""
            xt, sq = sq_tiles[t]
            xf = xt[:].rearrange("p r w -> p (r w)")
            hn = hnpool.tile([128, 1 + NIN + 1], BF16, tag="hn")
            nc.vector.memset(hn[0:C, 0:1], 0.0)
            nc.vector.memset(hn[0:C, 1 + NIN:], 0.0)
            for c0, n in col_tiles(NIN):
                ps = psN.tile([128, CT], F32, tag="psn")
                nc.tensor.matmul(ps[:, :n], sb_ones[:], sq[:, c0:c0 + n],
                                 start=True, stop=True)
                nc.scalar.activation(sq[:, c0:c0 + n], ps[:C, :n], AF.Ln,
                                     scale=1.0 / C, bias=c_eps)
                inv = statpool.tile([C, CT], F32, tag="inv")
                nc.scalar.activation(inv[:, :n], sq[:, c0:c0 + n], AF.Exp,
                                     scale=-0.5)
                nc.vector.tensor_mul(hn[0:C, 1 + c0:1 + c0 + n],
                                     xf[:, c0:c0 + n], inv[:, :n])
            hn_slabs[t] = hn
            return hn

        def build_in(t):
            """Slab-combining DMAs for conv_in(t) (+dup used by t+1).
            Needs hn[t] and hn[min(t+1, T-1)] already emitted."""
            cur = hn_slabs[t]
            nxt = hn_slabs[min(t + 1, T - 1)]
            if t + 1 < T:
                nc.gpsimd.dma_start(out=cur[96:128, :], in_=nxt[0:32, :])
            qi = qinpool.tile([128, 1 + NIN + 1], BF16, tag="qi")
            nc.gpsimd.dma_start(out=qi[0:64, :], in_=cur[32:96, :])
            nc.gpsimd.dma_start(out=qi[64:128, :], in_=nxt[0:64, :])
            qin_tiles[t] = qi
            rs = []
            for kh in range(3):
                r_ = rinpool.tile([96, NU], BF16, tag=f"r{kh}")
                for kw in range(3):
                    d = kh * Wp + kw
                    nc.gpsimd.dma_start(out=r_[32 * kw:32 * kw + 32, :],
                                        in_=nxt[64:96, d:d + NU])
                rs.append(r_)
            rin_tiles[t] = rs

        def conv_in_k(t, bt_arg, pair_range):
            a_sl = hn_slabs[max(t - 1, 0)]
            wa_sb = sb_wA0 if t == 0 else sb_wA
            ka = 96 if t == 0 else 128
            q = qin_tiles[t]
            rr = rin_tiles[t]

            streams = []
            for j in range(9):
                kh, kw = divmod(j, 3)
                s = kh * Wp + kw
                streams.append((
                    lambda m0, m1, jj=j: wa_sb[0:ka, jj, m0:m1],
                    lambda c0, n, ss=s: a_sl[0:ka, ss + c0:ss + c0 + n]))
            for j in range(9):
                kh, kw = divmod(j, 3)
                s = kh * Wp + kw
                streams.append((
                    lambda m0, m1, jj=j: sb_wB[:, jj, m0:m1],
                    lambda c0, n, ss=s: q[:, ss + c0:ss + c0 + n]))
            for kh in range(3):
                streams.append((
                    lambda m0, m1, kk=kh: sb_wR[:, kk, m0:m1],
                    lambda c0, n, kk=kh: rr[kk][:, c0:c0 + n]))
            NS = len(streams)

            bt = bt_arg
            cts = col_tiles(NU)
            for p0 in pair_range:
                pair = cts[p0:p0 + 2]
                pas = []
                for c0, n in pair:
                    pa = psA.tile([128, CT], F32, tag="pa")
                    for i, (lw, rh) in enumerate(streams):
                        nc.tensor.matmul(pa[:, :n], lw(0, 128), rh(c0, n),
                                         start=(i == 0), stop=(i == NS - 1))
                    pas.append(pa)
                pg = psG.tile([128, CT], F32, tag="pg")
                for i, (lw, rh) in enumerate(streams):
                    for j, (c0, n) in enumerate(pair):
                        b = 64 * j
                        nc.tensor.matmul(
                            pg[b:b + 64, :n], lw(128, 192), rh(c0, n),
                            start=(i == 0), stop=(i == NS - 1),
                            tile_position=(0, b), skip_group_check=True)
                for j, (c0, n) in enumerate(pair):
                    b = 64 * j
                    pa = pas[j]
                    gsa = gspool.tile([128, CT], BF16, tag="gsa")
                    nc.vector.tensor_copy(gsa[96:128, :n], pa[96:128, :n])
                    gsb = gspool.tile([128, CT], BF16, tag="gsb")
                    nc.vector.tensor_copy(gsb[b:b + 64, :n], pg[b:b + 64, :n])
                    gf = gfpool.tile([C, CT], BF16, tag="gf")
                    nc.sync.dma_start(out=gf[0:32, :n], in_=gsa[96:128, :n])
                    nc.sync.dma_start(out=gf[32:96, :n], in_=gsb[b:b + 64, :n])
                    sa = sapool.tile([C, CT], BF16, tag="sa")
                    nc.scalar.activation(sa[:, :n], pa[:C, :n], AF.Silu,
                                         bias=c_ba)
                    nc.vector.scalar_tensor_tensor(bt[:, c0:c0 + n],
                                                   gf[:, :n], c_bg, sa[:, :n],
                                                   ALU.add, ALU.mult)

        hr = singles.tile([C, NU], F32)
        hi = singles.tile([C, NU], F32)

        NH = (RU // 2) * Wp  # first-half columns (rows 0..RU/2-1)

        def scan_half(t, bt, yt, h0, h1):
            hrh = hr[:, h0:h1]
            hih = hi[:, h0:h1]
            bth = bt[:, h0:h1]
            if t == 0:
                nc.vector.tensor_copy(hrh, bth)
                nc.vector.memset(hih, 0.0)
            else:
                t1 = tmppool.tile([C, NH], F32, tag="tA")
                nc.vector.scalar_tensor_tensor(t1[:, :h1 - h0], hih, c_nli,
                                               bth, ALU.mult, ALU.add)
                nc.vector.scalar_tensor_tensor(hih, hih, c_lr, hih,
                                               ALU.mult, ALU.bypass)
                nc.vector.scalar_tensor_tensor(hih, hrh, c_li, hih,
                                               ALU.mult, ALU.add)
                nc.vector.scalar_tensor_tensor(hrh, hrh, c_lr,
                                               t1[:, :h1 - h0],
                                               ALU.mult, ALU.add)
            t3 = tmppool.tile([C, NH], F32, tag="tA")
            nc.vector.scalar_tensor_tensor(t3[:, :h1 - h0], hrh, c_gcre,
                                           hrh, ALU.mult, ALU.bypass)
            nc.vector.scalar_tensor_tensor(yt[0:C, 1 + h0:1 + h1], hih,
                                           c_gcim, t3[:, :h1 - h0],
                                           ALU.mult, ALU.add)
            yv = yt[0:C, 1 + h0:1 + h1].rearrange("p (r w) -> p r w", w=Wp)
            nr = (h1 - h0) // Wp
            nc.vector.tensor_copy(yv[:, :, 0:1], yv[:, :, W:W + 1])
            nc.vector.tensor_copy(yv[:, :, W + 1:W + 2], yv[:, :, 1:2])
            if h0 == 0:
                e0 = tmppool.tile([C, Wp], F32, tag="tE")
                nc.vector.scalar_tensor_tensor(e0[:], yv[:, 1, :], c_1m0,
                                               yv[:, 1, :], ALU.mult,
                                               ALU.bypass)
                nc.vector.scalar_tensor_tensor(yv[:, 0, :], yv[:, 0, :],
                                               c_m0, e0[:], ALU.mult,
                                               ALU.add)
            else:
                e1 = tmppool.tile([C, Wp], F32, tag="tE")
                nc.vector.scalar_tensor_tensor(e1[:], yv[:, nr - 2, :],
                                               c_1m1, yv[:, nr - 2, :],
                                               ALU.mult, ALU.bypass)
                nc.vector.scalar_tensor_tensor(yv[:, nr - 1, :],
                                               yv[:, nr - 1, :], c_m1,
                                               e1[:], ALU.mult, ALU.add)

        def conv_out_k(t):
            """Direct 27-tap conv_out: kt-ordered so the y[t]-dependent
            taps (kt2) come last in each accumulation group."""
            slabs = [y_slabs[min(max(t + kt - 1, 0), T - 1)] for kt in range(3)]
            ot = opool.tile([C, HR, Wp], F32, tag="ot")
            nc.sync.dma_start(out=ot[:], in_=xh[:, t, 2:2 + HR, :])
            touch(ot[:, 0, 0:1], engines="v")
            of = ot[:].rearrange("p r w -> p (r w)")
            for c0, n in col_tiles(NO):
                po = psO.tile([C, CT], F32, tag="po", name="po")
                for kt in range(3):
                    rhs_all = slabs[kt]
                    for kh in range(3):
                        for kw in range(3):
                            off = kt * 9 + kh * 3 + kw
                            s = 1 + c0 + kh * Wp + kw - 1
                            nc.tensor.matmul(po[:, :n], sb_wK[:, off, :],
                                             rhs_all[0:C, s:s + n],
                                             start=(off == 0),
                                             stop=(off == 26))
                nc.vector.scalar_tensor_tensor(of[:, c0:c0 + n], po[:, :n],
                                               c_bout, of[:, c0:c0 + n],
                                               ALU.add, ALU.add)
            nc.sync.dma_start(out=out[:, t], in_=ot[:, :, 1:1 + W])

        for u in range(min(3, T)):
            stage_sq(u)
            stage_rest(u)
        emit_weight_dmas()
        build_in(0)
        for t in range(T):
            if t + 1 < T:
                build_in(t + 1)
            if t + 3 < T:
                stage_sq(t + 3)
            bt = bpool.tile([C, NU], BF16, tag="bt")
            yt = ypool.tile([128, 1 + NU + 1], BF16, tag="yt")
            nc.vector.memset(yt[0:C, 0:1], 0.0)
            nc.vector.memset(yt[0:C, 1 + NU:], 0.0)
            conv_in_k(t, bt, [0, 2])
            scan_half(t, bt, yt, 0, NH)
            conv_in_k(t, bt, [4])
            scan_half(t, bt, yt, NH, NU)
            y_slabs[t] = yt
            if t >= 1:
                conv_out_k(t - 1)
            if t + 3 < T:
                stage_rest(t + 3)
        conv_out_k(T - 1)

    nc.compile()
    return nc



def prep_core_inputs(x, norm_w, conv_in_w, conv_in_b, nu_log, theta_log,
                     c_re, c_im, conv_out_w, conv_out_b, n_qh):
    """Build per-core input maps. Cores = batch-major, then H quarters."""
    B, C, T, H, W = x.shape
    HR = H // n_qh

    nu = np.exp(np.asarray(nu_log, np.float64))
    theta = np.exp(np.asarray(theta_log, np.float64))
    lam_re = (np.exp(-nu) * np.cos(theta)).astype(np.float32)
    lam_im = (np.exp(-nu) * np.sin(theta)).astype(np.float32)
    gamma = np.sqrt(1.0 - np.exp(-2.0 * nu))
    gcre = (gamma * np.asarray(c_re, np.float64)).astype(np.float32)
    gcim = (gamma * np.asarray(c_im, np.float64)).astype(np.float32)

    w_in_f = np.asarray(conv_in_w, np.float32) * \
        np.asarray(norm_w, np.float32)[None, :, None, None, None]
    w_in_t = np.ascontiguousarray(
        np.transpose(w_in_f, (1, 2, 3, 4, 0)).reshape(C, 27, 2 * C)
    ).astype(ml_dtypes.bfloat16)
    w_out_t = np.ascontiguousarray(
        np.transpose(np.asarray(conv_out_w, np.float32),
                     (1, 2, 3, 4, 0)).reshape(C, 27, C)
    ).astype(ml_dtypes.bfloat16)
    ones = np.ones((C, 128), ml_dtypes.bfloat16)

    xp = np.concatenate([x[..., -1:], x, x[..., :1]], axis=-1)  # W circular

    in_maps = []
    for b in range(B):
        for q in range(n_qh):
            rows = np.clip(np.arange(q * HR - 2, q * HR + HR + 2), 0, H - 1)
            xh = np.ascontiguousarray(xp[b][:, :, rows, :]).astype(np.float32)
            m0 = 0.0 if q == 0 else 1.0
            m1 = 0.0 if q == n_qh - 1 else 1.0
            cvec = np.stack([
                np.asarray(conv_in_b, np.float32)[:C],
                np.asarray(conv_in_b, np.float32)[C:],
                lam_re, lam_im, -lam_im, gcre, gcim,
                np.asarray(conv_out_b, np.float32),
                np.full(C, m0, np.float32), np.full(C, 1.0 - m0, np.float32),
                np.full(C, m1, np.float32), np.full(C, 1.0 - m1, np.float32),
                np.full(C, EPS, np.float32),
            ], axis=1)
            bo = np.asarray(conv_out_b, np.float32)
            aux = np.zeros((128, 2), np.float32)
            aux[96:128, 0] = bo[0:32]
            aux[:, 1] = bo[32 + (np.arange(128) % 64)]
            in_maps.append({
                "xh": xh,
                "w_in": w_in_t,
                "w_out": w_out_t,
                "onesw": ones,
                "consts": np.ascontiguousarray(cvec),
                "consts2": aux,
            })
    return in_maps


def prep_core_inputs_k(x, norm_w, conv_in_w, conv_in_b, nu_log, theta_log,
                       c_re, c_im, conv_out_w, conv_out_b, n_qh):
    """Per-core inputs for the K=128-packed program."""
    B, C, T, H, W = x.shape
    HR = H // n_qh
    C2 = 2 * C

    nu = np.exp(np.asarray(nu_log, np.float64))
    theta = np.exp(np.asarray(theta_log, np.float64))
    lam_re = (np.exp(-nu) * np.cos(theta)).astype(np.float32)
    lam_im = (np.exp(-nu) * np.sin(theta)).astype(np.float32)
    gamma = np.sqrt(1.0 - np.exp(-2.0 * nu))
    gcre = (gamma * np.asarray(c_re, np.float64)).astype(np.float32)
    gcim = (gamma * np.asarray(c_im, np.float64)).astype(np.float32)

    w_in_f = np.asarray(conv_in_w, np.float32) * \
        np.asarray(norm_w, np.float32)[None, :, None, None, None]
    # wt[cin, kt, kh, kw, cout]
    wt = np.transpose(w_in_f, (1, 2, 3, 4, 0))
    wto = np.transpose(np.asarray(conv_out_w, np.float32), (1, 2, 3, 4, 0))

    def pack(w, co):
        """w: [cin, kt, kh, kw, co] -> (wA, wB, wR, wR3, wA0)."""
        wA = np.zeros((128, 9, co), np.float32)
        wB = np.zeros((128, 9, co), np.float32)
        wA0 = np.zeros((96, 9, co), np.float32)
        for j in range(9):
            kh, kw = divmod(j, 3)
            wA[0:96, j] = w[:, 0, kh, kw]
            wA[96:128, j] = w[0:32, 1, kh, kw]
            wB[0:64, j] = w[32:96, 1, kh, kw]
            wB[64:128, j] = w[0:64, 2, kh, kw]
            wA0[0:32, j] = w[0:32, 0, kh, kw] + w[0:32, 1, kh, kw]
            wA0[32:96, j] = w[32:96, 0, kh, kw]
        wR = np.zeros((96, 3, co), np.float32)
        for kh in range(3):
            for kw in range(3):
                wR[32 * kw:32 * (kw + 1), kh] = w[64:96, 2, kh, kw]
        bf = ml_dtypes.bfloat16
        return (np.ascontiguousarray(wA).astype(bf),
                np.ascontiguousarray(wB).astype(bf),
                np.ascontiguousarray(wR).astype(bf),
                np.ascontiguousarray(wA0).astype(bf))

    wA, wB, wR, wA0 = pack(wt, C2)
    wK = np.ascontiguousarray(
        wto.reshape(C, 27, C)).astype(ml_dtypes.bfloat16)
    ones = np.ones((C, 128), ml_dtypes.bfloat16)

    xp = np.concatenate([x[..., -1:], x, x[..., :1]], axis=-1)  # W circular

    in_maps = []
    for b in range(B):
        for q in range(n_qh):
            rows = np.clip(np.arange(q * HR - 2, q * HR + HR + 2), 0, H - 1)
            xh = np.ascontiguousarray(xp[b][:, :, rows, :]).astype(np.float32)
            m0 = 0.0 if q == 0 else 1.0
            m1 = 0.0 if q == n_qh - 1 else 1.0
            cvec = np.stack([
                np.asarray(conv_in_b, np.float32)[:C],
                np.asarray(conv_in_b, np.float32)[C:],
                lam_re, lam_im, -lam_im, gcre, gcim,
                np.asarray(conv_out_b, np.float32),
                np.full(C, m0, np.float32), np.full(C, 1.0 - m0, np.float32),
                np.full(C, m1, np.float32), np.full(C, 1.0 - m1, np.float32),
                np.full(C, EPS, np.float32),
            ], axis=1)
            in_maps.append({
                "xh": xh,
                "wA": wA, "wB": wB, "wR": wR, "wA0": wA0,
                "wK": wK,
                "onesw": ones,
                "consts": np.ascontiguousarray(cvec),
            })
    return in_maps


LAST_RESULT = None  # BassKernelResults of the most recent kernel() call


def kernel(x, norm_w, conv_in_w, conv_in_b, nu_log, theta_log, c_re, c_im,
           conv_out_w, conv_out_b):
    global LAST_RESULT
    from concourse.bass_utils import run_bass_kernel_spmd

    x = np.asarray(x, np.float32)
    B, C, T, H, W = x.shape
    HR = H // QH
    if os.environ.get("KERNEL_KPACK", "1") == "1":
        in_maps = prep_core_inputs_k(x, norm_w, conv_in_w, conv_in_b, nu_log,
                                     theta_log, c_re, c_im, conv_out_w,
                                     conv_out_b, QH)
        nc = build_program_k(C=C, T=T, HR=HR, W=W, CT=512)
    else:
        in_maps = prep_core_inputs(x, norm_w, conv_in_w, conv_in_b, nu_log,
                                   theta_log, c_re, c_im, conv_out_w,
                                   conv_out_b, QH)
        nc = build_program(C=C, T=T, HR=HR, W=W, CT=512,
                           use_silu=os.environ.get("KERNEL_NO_SILU", "") != "1",
                           pack=os.environ.get("KERNEL_PACK", "1") == "1",
                           pack2=os.environ.get("KERNEL_PACK2", "0") == "1")
    trace = os.environ.get("KERNEL_TRACE", "") == "1"
    res = run_bass_kernel_spmd(nc, in_maps, list(range(N_CORES)), trace=trace)
    LAST_RESULT = res
    out = np.empty((B, C, T, H, W), np.float32)
    for core in range(N_CORES):
        b, q = core // QH, core % QH
        out[b, :, :, q * HR:(q + 1) * HR, :] = res.results[core]["out"]
    return out



# revision 20
# speedup vs baseline: 1.0444x; 1.0444x over previous
"""ConvLRUBlock Trainium2 kernel.

Reference computation (per batch b):
    h   = rms_norm(x, norm_w)                  # over channel dim
    uv  = conv3d_3x3x3(h, w_in) + b_in         # pad: replicate T/H, circular W
    u   = silu(a) * g          (a, g = uv split on channels)
    y_t = Re(h_t) c_re + Im(h_t) c_im,  h_t = lam h_{t-1} + gamma u_t  (diag LRU)
    out = x + conv3d_3x3x3(y, w_out) + b_out

Sharding: 8 cores = (batch 2) x (H quarters 4). Each core receives its H
slice plus 2 halo rows each side (edge-replicated) and the W dim circularly
padded to W+2, so no inter-core communication is needed. All conv padding is
resolved by host-side halo materialization + in-kernel index clamping (T) +
in-SBUF wrap-column fixes (W for the second conv).

In-kernel layout: channels (96) on SBUF partitions; spatial (rows x (W+2))
flattened on the free dim. 3x3x3 convs = 27 accumulating matmuls per output
tile; kh/kw become column shifts of the rhs AP, kt picks one of 3 t-slabs.
The LRU scan is 16 sequential complex steps on the vector engine.
"""

import os
from contextlib import ExitStack

import ml_dtypes
import numpy as np

import concourse.bacc as bacc
import concourse.bass as bass  # noqa: F401
import concourse.tile as tile
from concourse import mybir

F32 = mybir.dt.float32
BF16 = mybir.dt.bfloat16
ALU = mybir.AluOpType
AF = mybir.ActivationFunctionType

EPS = 1e-6

# Full-problem constants
B_FULL, C_FULL, T_FULL, H_FULL, W_FULL = 2, 96, 16, 64, 128
QH = 4  # H quarters
N_CORES = 8


def build_program(C=96, T=16, HR=16, W=128, CT=512, use_silu=True,
                  pack=False, pack2=False):
    """Build the single-core SPMD Bass program.

    C: channels; T: time steps; HR: output H rows per core; W: width.
    CT: matmul/psum column tile (<=512). use_silu: Silu on ACT vs
    Sigmoid+mults (the simulator does not implement Silu).
    """
    Wp = W + 2           # circular-padded width
    RIN = HR + 4         # input rows (2 halo each side, for two convs)
    RU = HR + 2          # u/y rows (1 halo each side, for conv_out)
    NIN = RIN * Wp       # flattened input cols per t
    NU = RU * Wp         # flattened u/y cols per t
    NO = HR * Wp         # flattened output cols per t

    nc = bacc.Bacc()
    xh = nc.declare_dram_parameter("xh", [C, T, RIN, Wp], F32, isOutput=False)
    w_in = nc.declare_dram_parameter("w_in", [C, 27, 2 * C], BF16, isOutput=False)
    w_out = nc.declare_dram_parameter("w_out", [C, 27, C], BF16, isOutput=False)
    onesw = nc.declare_dram_parameter("onesw", [C, 128], BF16, isOutput=False)
    consts = nc.declare_dram_parameter("consts", [C, 13], F32, isOutput=False)
    consts2 = nc.declare_dram_parameter("consts2", [128, 2], F32, isOutput=False)
    out = nc.declare_dram_parameter("out", [C, T, HR, W], F32, isOutput=True)

    def col_tiles(total):
        return [(i, min(CT, total - i)) for i in range(0, total, CT)]

    with tile.TileContext(nc) as tc, ExitStack() as ctx:
        singles = ctx.enter_context(tc.tile_pool(name="singles", bufs=1))
        xpool = ctx.enter_context(tc.tile_pool(name="xpool", bufs=2))
        sqpool = ctx.enter_context(tc.tile_pool(name="sqpool", bufs=2))
        statpool = ctx.enter_context(tc.tile_pool(name="statpool", bufs=2))
        hnpool = ctx.enter_context(tc.tile_pool(name="hnpool", bufs=4))
        sapool = ctx.enter_context(tc.tile_pool(name="sapool", bufs=3))
        bpool = ctx.enter_context(tc.tile_pool(name="bpool", bufs=2))
        hrpool = ctx.enter_context(tc.tile_pool(name="hrpool", bufs=2))
        hipool = ctx.enter_context(tc.tile_pool(name="hipool", bufs=2))
        tmppool = ctx.enter_context(tc.tile_pool(name="tmppool", bufs=2))
        ypool = ctx.enter_context(tc.tile_pool(name="ypool", bufs=4))
        opool = ctx.enter_context(tc.tile_pool(name="opool", bufs=2))
        psN = ctx.enter_context(tc.tile_pool(name="psN", bufs=2, space="PSUM"))
        psA = ctx.enter_context(tc.tile_pool(name="psA", bufs=2, space="PSUM"))
        psG = ctx.enter_context(tc.tile_pool(name="psG", bufs=2, space="PSUM"))
        psO = ctx.enter_context(tc.tile_pool(name="psO", bufs=2, space="PSUM"))

        sb_win = singles.tile([C, 27, 2 * C], BF16)
        nc.sync.dma_start(out=sb_win[:], in_=w_in[:])
        sb_wout = singles.tile([C, 27, C], BF16)
        nc.sync.dma_start(out=sb_wout[:], in_=w_out[:])
        sb_ones = singles.tile([C, 128], BF16)
        nc.sync.dma_start(out=sb_ones[:], in_=onesw[:])
        sb_c = singles.tile([C, 13], F32)
        nc.sync.dma_start(out=sb_c[:], in_=consts[:])
        sb_c2 = singles.tile([128, 2], F32)
        nc.sync.dma_start(out=sb_c2[:], in_=consts2[:])
        c_aux0 = sb_c2[:, 0:1]
        c_aux1 = sb_c2[:, 1:2]
        c_ba = sb_c[:, 0:1]
        c_bg = sb_c[:, 1:2]
        c_lr = sb_c[:, 2:3]
        c_li = sb_c[:, 3:4]
        c_nli = sb_c[:, 4:5]
        c_gcre = sb_c[:, 5:6]
        c_gcim = sb_c[:, 6:7]
        c_bout = sb_c[:, 7:8]
        c_m0 = sb_c[:, 8:9]
        c_1m0 = sb_c[:, 9:10]
        c_m1 = sb_c[:, 10:11]
        c_1m1 = sb_c[:, 11:12]
        c_eps = sb_c[:, 12:13]

        # Warm-up reads: make each compute engine observe the const-DMA
        # semaphores early, so steady-state ops carry at most one sync wait
        # (walrus rejects DVE ops with two wait commands).
        wu_v = singles.tile([C, 13], F32)
        nc.vector.tensor_copy(wu_v[:], sb_c[:])
        wu_s = singles.tile([C, 13], F32)
        nc.scalar.activation(wu_s[:], sb_c[:], AF.Square)

        touchpool = ctx.enter_context(tc.tile_pool(name="touchpool", bufs=2))
        if pack:
            gspool = ctx.enter_context(tc.tile_pool(name="gspool", bufs=2))
            gfpool = ctx.enter_context(tc.tile_pool(name="gfpool", bufs=2))
        if pack2:
            piecepool = ctx.enter_context(tc.tile_pool(name="piecepool", bufs=2))

        def touch(ap, engines="v"):
            """Tiny read of a freshly-DMA'd tile so the engine observes the
            DMA-queue semaphore here; later big consumers then carry only
            engine-sem waits (walrus rejects DVE ops with 2 sync waits)."""
            if "v" in engines:
                tv = touchpool.tile([C, 1], F32, tag="tv")
                nc.vector.tensor_copy(tv[:], ap)
            if "s" in engines:
                ts_ = touchpool.tile([C, 1], F32, tag="ts")
                nc.scalar.activation(ts_[:], ap, AF.Square)

        hn_slabs = [None] * T  # hnorm tiles, data at col offset 1
        y_slabs = [None] * T   # y tiles (bf16), data at col offset 1

        def stage_a(t):
            """x[t] -> hnorm[t] (rms-normed, bf16, [C, 1+NIN+1])."""
            xt = xpool.tile([C, RIN, Wp], F32, tag="xt")
            nc.sync.dma_start(out=xt[:], in_=xh[:, t])
            touch(xt[:, 0, 0:1], engines="vs")
            xf = xt[:].rearrange("p r w -> p (r w)")
            hn = hnpool.tile([C, 1 + NIN + 1], BF16, tag="hn")
            nc.vector.memset(hn[:, 0:1], 0.0)
            nc.vector.memset(hn[:, 1 + NIN:], 0.0)
            for c0, n in col_tiles(NIN):
                sq = sqpool.tile([C, CT], BF16, tag="sq")
                nc.scalar.activation(sq[:, :n], xf[:, c0:c0 + n], AF.Square)
                ps = psN.tile([128, CT], F32, tag="psn")
                nc.tensor.matmul(ps[:, :n], sb_ones[:], sq[:, :n],
                                 start=True, stop=True)
                lg = statpool.tile([C, CT], F32, tag="lg")
                nc.scalar.activation(lg[:, :n], ps[:C, :n], AF.Ln,
                                     scale=1.0 / C, bias=c_eps)
                inv = statpool.tile([C, CT], F32, tag="inv")
                nc.scalar.activation(inv[:, :n], lg[:, :n], AF.Exp, scale=-0.5)
                nc.vector.tensor_mul(hn[:, 1 + c0:1 + c0 + n],
                                     xf[:, c0:c0 + n], inv[:, :n])
            hn_slabs[t] = hn
            return hn

        def gate_epilogue(pa, pg_sb, bt, c0, n):
            """silu(a+ba)*(g+bg) for one coltile; a=pa[0:C] (psum),
            g already realigned to pg_sb [C, n] (sbuf)."""
            if use_silu:
                sa = sapool.tile([C, CT], BF16, tag="sa")
                nc.scalar.activation(sa[:, :n], pa[:C, :n], AF.Silu,
                                     bias=c_ba)
            else:
                sg = sapool.tile([C, CT], BF16, tag="sg")
                nc.scalar.activation(sg[:, :n], pa[:C, :n], AF.Sigmoid,
                                     bias=c_ba)
                av = sapool.tile([C, CT], F32, tag="av")
                nc.vector.scalar_tensor_tensor(av[:, :n], pa[:C, :n], c_ba,
                                               sg[:, :n], ALU.add,
                                               ALU.bypass)
                sa = sapool.tile([C, CT], BF16, tag="sa")
                nc.vector.tensor_mul(sa[:, :n], sg[:, :n], av[:, :n])
            nc.vector.scalar_tensor_tensor(bt[:, c0:c0 + n], pg_sb[:, :n],
                                           c_bg, sa[:, :n],
                                           ALU.add, ALU.mult)

        def conv_in_packed(t):
            """1.5-array-pass conv_in: pass1 M=128 (a0..95,g0..31), pass2
            col-tiled pairs of M=64 (g32..95) for two coltiles at once."""
            slabs = [hn_slabs[min(max(t + kt - 1, 0), T - 1)] for kt in range(3)]
            bt = bpool.tile([C, NU], BF16, tag="bt")
            cts = col_tiles(NU)
            for p0 in range(0, len(cts), 2):
                pair = cts[p0:p0 + 2]
                pas = []
                for c0, n in pair:
                    pa = psA.tile([128, CT], F32, tag="pa")
                    for kt in range(3):
                        rhs_all = slabs[kt]
                        for kh in range(3):
                            for kw in range(3):
                                off = kt * 9 + kh * 3 + kw
                                s = 1 + c0 + kh * Wp + kw - 1
                                nc.tensor.matmul(
                                    pa[:, :n], sb_win[:, off, 0:128],
                                    rhs_all[:, s:s + n],
                                    start=(off == 0), stop=(off == 26))
                    pas.append(pa)
                pg = psG.tile([128, CT], F32, tag="pg")
                for kt in range(3):
                    rhs_all = slabs[kt]
                    for kh in range(3):
                        for kw in range(3):
                            off = kt * 9 + kh * 3 + kw
                            for j, (c0, n) in enumerate(pair):
                                s = 1 + c0 + kh * Wp + kw - 1
                                b = 64 * j
                                nc.tensor.matmul(
                                    pg[b:b + 64, :n],
                                    sb_win[:, off, 128:192],
                                    rhs_all[:, s:s + n],
                                    start=(off == 0), stop=(off == 26),
                                    tile_position=(0, b),
                                    skip_group_check=True)
                for j, (c0, n) in enumerate(pair):
                    b = 64 * j
                    pa = pas[j]
                    gsa = gspool.tile([128, CT], F32, tag="gsa")
                    nc.vector.tensor_copy(gsa[96:128, :n], pa[96:128, :n])
                    gsb = gspool.tile([128, CT], F32, tag="gsb")
                    nc.vector.tensor_copy(gsb[b:b + 64, :n], pg[b:b + 64, :n])
                    gf = gfpool.tile([C, CT], F32, tag="gf")
                    nc.sync.dma_start(out=gf[0:32, :n], in_=gsa[96:128, :n])
                    nc.sync.dma_start(out=gf[32:96, :n], in_=gsb[b:b + 64, :n])
                    gate_epilogue(pa, gf, bt, c0, n)
            return bt

        def conv_in(t):
            """hnorm[t-1..t+1] -> b[t] = silu(a+ba)*(g+bg), bf16 [C, NU]."""
            slabs = [hn_slabs[min(max(t + kt - 1, 0), T - 1)] for kt in range(3)]
            bt = bpool.tile([C, NU], BF16, tag="bt")
            for c0, n in col_tiles(NU):
                pa = psA.tile([C, CT], F32, tag="pa")
                pg = psG.tile([C, CT], F32, tag="pg")
                for kt in range(3):
                    rhs_all = slabs[kt]
                    for kh in range(3):
                        for kw in range(3):
                            off = kt * 9 + kh * 3 + kw
                            s = 1 + c0 + kh * Wp + kw - 1
                            rhs = rhs_all[:, s:s + n]
                            nc.tensor.matmul(pa[:, :n], sb_win[:, off, 0:C],
                                             rhs, start=(off == 0),
                                             stop=(off == 26))
                for kt in range(3):
                    rhs_all = slabs[kt]
                    for kh in range(3):
                        for kw in range(3):
                            off = kt * 9 + kh * 3 + kw
                            s = 1 + c0 + kh * Wp + kw - 1
                            rhs = rhs_all[:, s:s + n]
                            nc.tensor.matmul(pg[:, :n], sb_win[:, off, C:2 * C],
                                             rhs, start=(off == 0),
                                             stop=(off == 26))
                gate_epilogue(pa, pg, bt, c0, n)
            return bt

        scan_state = [None, None]  # hr, hi tiles [C, NU] f32

        def scan_step(t, bt):
            """LRU step + projection -> y[t] (bf16 slab, data at offset 1)."""
            hr_new = hrpool.tile([C, NU], F32, tag="hr")
            hi_new = hipool.tile([C, NU], F32, tag="hi")
            if t == 0:
                nc.vector.tensor_copy(hr_new[:], bt[:])
                nc.vector.memset(hi_new[:], 0.0)
            else:
                hr_old, hi_old = scan_state
                t1 = tmppool.tile([C, NU], F32, tag="tA")
                nc.vector.scalar_tensor_tensor(t1[:], hi_old[:], c_nli, bt[:],
                                               ALU.mult, ALU.add)
                nc.vector.scalar_tensor_tensor(hr_new[:], hr_old[:], c_lr,
                                               t1[:], ALU.mult, ALU.add)
                t2 = tmppool.tile([C, NU], F32, tag="tB")
                nc.vector.scalar_tensor_tensor(t2[:], hi_old[:], c_lr,
                                               hi_old[:], ALU.mult, ALU.bypass)
                nc.vector.scalar_tensor_tensor(hi_new[:], hr_old[:], c_li,
                                               t2[:], ALU.mult, ALU.add)
            scan_state[0], scan_state[1] = hr_new, hi_new
            t3 = tmppool.tile([C, NU], F32, tag="tA")
            nc.vector.scalar_tensor_tensor(t3[:], hr_new[:], c_gcre,
                                           hr_new[:], ALU.mult, ALU.bypass)
            yt = ypool.tile([C, 1 + NU + 1], BF16, tag="yt")
            nc.vector.memset(yt[:, 0:1], 0.0)
            nc.vector.memset(yt[:, 1 + NU:], 0.0)
            nc.vector.scalar_tensor_tensor(yt[:, 1:1 + NU], hi_new[:], c_gcim,
                                           t3[:], ALU.mult, ALU.add)
            # W wrap columns: col 0 <- col W (w=W-1), col W+1 <- col 1 (w=0)
            yv = yt[:, 1:1 + NU].rearrange("p (r w) -> p r w", w=Wp)
            nc.vector.tensor_copy(yv[:, :, 0:1], yv[:, :, W:W + 1])
            nc.vector.tensor_copy(yv[:, :, W + 1:W + 2], yv[:, :, 1:2])
            # H edge replication (active only on global-edge cores, via mask):
            # row0 <- m0*row0 + (1-m0)*row1 ; last <- m1*last + (1-m1)*prev
            e0 = tmppool.tile([C, Wp], F32, tag="tE")
            nc.vector.scalar_tensor_tensor(e0[:], yv[:, 1, :], c_1m0,
                                           yv[:, 1, :], ALU.mult, ALU.bypass)
            nc.vector.scalar_tensor_tensor(yv[:, 0, :], yv[:, 0, :], c_m0,
                                           e0[:], ALU.mult, ALU.add)
            e1 = tmppool.tile([C, Wp], F32, tag="tE")
            nc.vector.scalar_tensor_tensor(e1[:], yv[:, RU - 2, :], c_1m1,
                                           yv[:, RU - 2, :], ALU.mult, ALU.bypass)
            nc.vector.scalar_tensor_tensor(yv[:, RU - 1, :], yv[:, RU - 1, :],
                                           c_m1, e1[:], ALU.mult, ALU.add)
            y_slabs[t] = yt
            return yt

        def conv_out(t):
            """y[t-1..t+1] -> out[t] = x + conv(y) + b_out."""
            slabs = [y_slabs[min(max(t + kt - 1, 0), T - 1)] for kt in range(3)]
            ot = opool.tile([C, HR, Wp], F32, tag="ot")
            # residual input loaded into the output staging tile
            nc.sync.dma_start(out=ot[:], in_=xh[:, t, 2:2 + HR, :])
            touch(ot[:, 0, 0:1], engines="v")
            of = ot[:].rearrange("p r w -> p (r w)")
            for c0, n in col_tiles(NO):
                po = psO.tile([C, CT], F32, tag="po")
                for kt in range(3):
                    rhs_all = slabs[kt]
                    for kh in range(3):
                        for kw in range(3):
                            off = kt * 9 + kh * 3 + kw
                            s = 1 + c0 + kh * Wp + kw - 1
                            rhs = rhs_all[:, s:s + n]
                            nc.tensor.matmul(po[:, :n], sb_wout[:, off, :],
                                             rhs, start=(off == 0),
                                             stop=(off == 26))
                nc.vector.scalar_tensor_tensor(of[:, c0:c0 + n], po[:, :n],
                                               c_bout, of[:, c0:c0 + n],
                                               ALU.add, ALU.add)
            nc.sync.dma_start(out=out[:, t], in_=ot[:, :, 1:1 + W])

        def conv_out_packed(t):
            """conv_out with array packing: pairs (e,o) put e's 96 channels +
            o's first 32 (pos 96) in one pass; the two pairs' leftover 64
            channels share one concurrent col-tiled pass. Misplaced pieces are
            realigned into the staging tile via SBUF->SBUF DMA."""
            slabs = [y_slabs[min(max(t + kt - 1, 0), T - 1)] for kt in range(3)]
            ot = opool.tile([C, HR, Wp], F32, tag="ot")
            nc.sync.dma_start(out=ot[:], in_=xh[:, t, 2:2 + HR, :])
            touch(ot[:, 0, 0:1], engines="v")
            of = ot[:].rearrange("p r w -> p (r w)")
            xflat = xh[:, t, 2:2 + HR, :].rearrange("p r w -> p (r w)")
            cts = col_tiles(NO)

            def mm_group(ps, prange, wslice, c0, n, pos):
                for kt in range(3):
                    rhs_all = slabs[kt]
                    for kh in range(3):
                        for kw in range(3):
                            off = kt * 9 + kh * 3 + kw
                            s = 1 + c0 + kh * Wp + kw - 1
                            nc.tensor.matmul(
                                ps[prange[0]:prange[1], :n],
                                sb_wout[:, off, wslice[0]:wslice[1]],
                                rhs_all[:, s:s + n],
                                start=(off == 0), stop=(off == 26),
                                tile_position=pos, skip_group_check=True)

            def mm_group2(ps, jobs):
                """Interleaved concurrent accumulation groups."""
                for kt in range(3):
                    rhs_all = slabs[kt]
                    for kh in range(3):
                        for kw in range(3):
                            off = kt * 9 + kh * 3 + kw
                            for prange, wslice, c0, n, pos in jobs:
                                s = 1 + c0 + kh * Wp + kw - 1
                                nc.tensor.matmul(
                                    ps[prange[0]:prange[1], :n],
                                    sb_wout[:, off, wslice[0]:wslice[1]],
                                    rhs_all[:, s:s + n],
                                    start=(off == 0), stop=(off == 26),
                                    tile_position=pos, skip_group_check=True)

            def aligned_epi(ps, c0, n):
                nc.vector.scalar_tensor_tensor(of[:, c0:c0 + n], ps[:C, :n],
                                               c_bout, of[:, c0:c0 + n],
                                               ALU.add, ALU.add)

            def piece_epi(ps, pbase, psize, ch0, c0, n):
                """Residual-add for a channel piece at partitions
                [pbase, pbase+psize) holding channels [ch0, ch0+psize);
                realign into ot via DMA."""
                xp = piecepool.tile([128, CT], F32, tag="xp")
                nc.sync.dma_start(out=xp[pbase:pbase + psize, :n],
                                  in_=xflat[ch0:ch0 + psize, c0:c0 + n])
                aux = c_aux0 if pbase == 96 else c_aux1
                pt = piecepool.tile([128, CT], F32, tag="pc")
                nc.vector.scalar_tensor_tensor(
                    pt[pbase:pbase + psize, :n],
                    ps[pbase:pbase + psize, :n],
                    aux[pbase:pbase + psize, :],
                    xp[pbase:pbase + psize, :n], ALU.add, ALU.add)
                nc.sync.dma_start(out=of[ch0:ch0 + psize, c0:c0 + n],
                                  in_=pt[pbase:pbase + psize, :n])

            for e, o in ((0, 1), (2, 3)):
                (ce, ne), (co_, no_) = cts[e], cts[o]
                p1 = psO.tile([128, CT], F32, tag="po")
                mm_group2(p1, [((0, 32), (0, 32), ce, ne, (0, 0)),
                               ((32, 64), (32, 64), ce, ne, (0, 32)),
                               ((64, 96), (64, 96), ce, ne, (0, 64)),
                               ((96, 128), (0, 32), co_, no_, (0, 96))])
                aligned_epi(p1, ce, ne)
                piece_epi(p1, 96, 32, 0, co_, no_)
            p2 = psG.tile([128, CT], F32, tag="pg")
            mm_group2(p2, [((0, 32), (32, 64), cts[1][0], cts[1][1], (0, 0)),
                           ((32, 64), (64, 96), cts[1][0], cts[1][1], (0, 32)),
                           ((64, 96), (32, 64), cts[3][0], cts[3][1], (0, 64)),
                           ((96, 128), (64, 96), cts[3][0], cts[3][1], (0, 96))])
            piece_epi(p2, 0, 64, 32, cts[1][0], cts[1][1])
            piece_epi(p2, 64, 64, 32, cts[3][0], cts[3][1])
            p4 = psO.tile([128, CT], F32, tag="po")
            mm_group(p4, (0, 96), (0, 96), cts[4][0], cts[4][1], (0, 0))
            aligned_epi(p4, cts[4][0], cts[4][1])
            nc.sync.dma_start(out=out[:, t], in_=ot[:, :, 1:1 + W])

        octs = col_tiles(NO)
        use_p2 = pack2 and len(octs) == 5 and all(n == CT for _, n in octs[:4])
        co_fn = conv_out_packed if use_p2 else conv_out

        stage_a(0)
        if T > 1:
            stage_a(1)
        for t in range(T):
            if t + 1 < T:
                stage_a(t + 1)
            bt = conv_in_packed(t) if pack else conv_in(t)
            scan_step(t, bt)
            if t >= 1:
                co_fn(t - 1)
        co_fn(T - 1)

    nc.compile()
    return nc


def build_program_k(C=96, T=16, HR=16, W=128, CT=512, cout_kp=False):
    """K=128-packed SPMD program.

    Each conv's 27-tap x 96-ch contraction (2592 rows) is regrouped into 21
    matmul streams per output tile instead of 27:
      - 9 A-streams: [slab_prev ch0..95 ; slab_cur ch0..31] (dup rows DMA'd
        into partitions 96..127 of the prev slab tile), one per (kh,kw).
      - 9 B-streams: Q-tile = [slab_cur ch32..95 ; slab_next ch0..63].
      - 3 R-streams: per-kh slabs pack slab_next ch64..95 at the 3 kw
        column shifts (K=96).
    t=0 uses special A-weights (kt0+kt1 folded for ch0..31, K=96) because
    the clamped prev slab's dup rows hold the wrong timestep.

    Pipeline: normalization (stage) runs 3 timesteps ahead, slab-combining
    DMAs (build_in) 1 ahead with a full iteration of slack; x**2 runs on
    DVE and rms uses Sqrt(ACT)+reciprocal(DVE) so the ACT engine only ever
    holds the Sqrt+Silu tables (no table thrash) and the stage matmuls
    never block the PE FIFO.
    """
    Wp = W + 2
    RIN = HR + 4
    RU = HR + 2
    NIN = RIN * Wp
    NU = RU * Wp
    NO = HR * Wp
    C2 = 2 * C

    nc = bacc.Bacc()
    xh = nc.declare_dram_parameter("xh", [C, T, RIN, Wp], F32, isOutput=False)
    wA = nc.declare_dram_parameter("wA", [128, 9, C2], BF16, isOutput=False)
    wB = nc.declare_dram_parameter("wB", [128, 9, C2], BF16, isOutput=False)
    wR = nc.declare_dram_parameter("wR", [96, 3, C2], BF16, isOutput=False)
    wA0 = nc.declare_dram_parameter("wA0", [96, 9, C2], BF16, isOutput=False)
    if cout_kp:
        wKA = nc.declare_dram_parameter("wKA", [128, 9, C], BF16, isOutput=False)
        wKB = nc.declare_dram_parameter("wKB", [128, 9, C], BF16, isOutput=False)
        wKR = nc.declare_dram_parameter("wKR", [96, 3, C], BF16, isOutput=False)
        wKA0 = nc.declare_dram_parameter("wKA0", [96, 9, C], BF16, isOutput=False)
    else:
        wK = nc.declare_dram_parameter("wK", [C, 27, C], BF16, isOutput=False)
    onesw = nc.declare_dram_parameter("onesw", [C, 128], BF16, isOutput=False)
    consts = nc.declare_dram_parameter("consts", [C, 13], F32, isOutput=False)
    aux = nc.declare_dram_parameter("aux128", [128, 8], F32, isOutput=False)
    out = nc.declare_dram_parameter("out", [C, T, HR, W], F32, isOutput=True)

    def col_tiles(total):
        return [(i, min(CT, total - i)) for i in range(0, total, CT)]

    with tile.TileContext(nc) as tc, ExitStack() as ctx:
        singles = ctx.enter_context(tc.tile_pool(name="singles", bufs=1))
        xpool = ctx.enter_context(tc.tile_pool(name="xpool", bufs=2))
        sqpool = ctx.enter_context(tc.tile_pool(name="sqpool", bufs=2))
        statpool = ctx.enter_context(tc.tile_pool(name="statpool", bufs=2))
        hnpool = ctx.enter_context(tc.tile_pool(name="hnpool", bufs=5))
        qinpool = ctx.enter_context(tc.tile_pool(name="qinpool", bufs=2))
        rinpool = ctx.enter_context(tc.tile_pool(name="rinpool", bufs=2))
        sapool = ctx.enter_context(tc.tile_pool(name="sapool", bufs=2))
        bpool = ctx.enter_context(tc.tile_pool(name="bpool", bufs=2))
        tmppool = ctx.enter_context(tc.tile_pool(name="tmppool", bufs=1))
        ypool = ctx.enter_context(
            tc.tile_pool(name="ypool", bufs=5 if cout_kp else 4))
        opool = ctx.enter_context(tc.tile_pool(name="opool", bufs=1))
        if cout_kp:
            qopool = ctx.enter_context(tc.tile_pool(name="qopool", bufs=2))
            rkpool = ctx.enter_context(tc.tile_pool(name="rkpool", bufs=2))
            xspool = ctx.enter_context(tc.tile_pool(name="xspool", bufs=3))
            ostgpool = ctx.enter_context(tc.tile_pool(name="ostgpool", bufs=3))
        gspool = ctx.enter_context(tc.tile_pool(name="gspool", bufs=1))
        gfpool = ctx.enter_context(tc.tile_pool(name="gfpool", bufs=2))
        touchpool = ctx.enter_context(tc.tile_pool(name="touchpool", bufs=2))
        psN = ctx.enter_context(tc.tile_pool(name="psN", bufs=1, space="PSUM"))
        psA = ctx.enter_context(tc.tile_pool(name="psA", bufs=2, space="PSUM"))
        psG = ctx.enter_context(tc.tile_pool(name="psG", bufs=2, space="PSUM"))
        psO = ctx.enter_context(tc.tile_pool(name="psO", bufs=3, space="PSUM"))

        sb_c = singles.tile([C, 13], F32)
        nc.sync.dma_start(out=sb_c[:], in_=consts[:])
        sb_aux = singles.tile([128, 8], F32)
        nc.sync.dma_start(out=sb_aux[:], in_=aux[:])
        sb_ones = singles.tile([C, 128], BF16)
        nc.sync.dma_start(out=sb_ones[:], in_=onesw[:])
        sb_wA = singles.tile([128, 9, C2], BF16)
        sb_wB = singles.tile([128, 9, C2], BF16)
        sb_wR = singles.tile([96, 3, C2], BF16)
        sb_wA0 = singles.tile([96, 9, C2], BF16)
        if cout_kp:
            sb_wKA = singles.tile([128, 9, C], BF16)
            sb_wKB = singles.tile([128, 9, C], BF16)
            sb_wKR = singles.tile([96, 3, C], BF16)
            sb_wKA0 = singles.tile([96, 9, C], BF16)
        else:
            sb_wK = singles.tile([C, 27, C], BF16)

        def emit_weight_dmas():
            nc.sync.dma_start(out=sb_wA0[:], in_=wA0[:])
            nc.sync.dma_start(out=sb_wA[:], in_=wA[:])
            nc.sync.dma_start(out=sb_wB[:], in_=wB[:])
            nc.sync.dma_start(out=sb_wR[:], in_=wR[:])
            if cout_kp:
                nc.sync.dma_start(out=sb_wKA[:], in_=wKA[:])
                nc.sync.dma_start(out=sb_wKB[:], in_=wKB[:])
                nc.sync.dma_start(out=sb_wKR[:], in_=wKR[:])
                nc.sync.dma_start(out=sb_wKA0[:], in_=wKA0[:])
            else:
                nc.sync.dma_start(out=sb_wK[:], in_=wK[:])

        c_ba = sb_c[:, 0:1]
        c_bg = sb_c[:, 1:2]
        c_lr = sb_c[:, 2:3]
        c_li = sb_c[:, 3:4]
        c_nli = sb_c[:, 4:5]
        c_gcre = sb_c[:, 5:6]
        c_gcim = sb_c[:, 6:7]
        c_bout = sb_c[:, 7:8]
        c_m0 = sb_c[:, 8:9]
        c_1m0 = sb_c[:, 9:10]
        c_m1 = sb_c[:, 10:11]
        c_1m1 = sb_c[:, 11:12]
        c_eps = sb_c[:, 12:13]
        c_ba_hi = sb_aux[:, 0:1]   # [96:128] = ba[0:32]
        c_ba_pg = sb_aux[:, 1:2]   # [0:64] and [64:128] = ba[32:96]

        # Warm-ups: observe const DMA on each engine and preload the only
        # two ACT tables used in steady state (natural_log_exp, Silu).
        wu_v = singles.tile([C, 13], F32)
        nc.vector.tensor_copy(wu_v[:], sb_c[:])
        wu_s = singles.tile([C, 13], F32)
        nc.scalar.activation(wu_s[:], sb_c[:], AF.Exp)
        wu_s2 = singles.tile([128, 8], BF16)
        nc.scalar.activation(wu_s2[:], sb_aux[:], AF.Silu, bias=c_ba_hi)

        def touch(ap, engines="v"):
            if "v" in engines:
                tv = touchpool.tile([C, 1], F32, tag="tv")
                nc.vector.tensor_copy(tv[:], ap)

        ag_swap = os.environ.get("KERNEL_AGSWAP", "1") == "1"

        hn_slabs = [None] * T   # [128, 1+NIN+1] bf16; rows 96:128 = dup
        sq_tiles = [None] * T
        qin_tiles = [None] * T
        rin_tiles = [None] * T
        y_slabs = [None] * T    # [128, 1+NU+1] bf16

        sq_on_act = os.environ.get("KERNEL_SQACT", "1") == "1"

        def stage_sq(t):
            """x[t] load + x**2 on ACT (Square lives in every table set, so
            this costs no table loads and frees the DVE)."""
            xt = xpool.tile([C, RIN, Wp], F32, tag="xt")
            nc.sync.dma_start(out=xt[:], in_=xh[:, t])
            xf = xt[:].rearrange("p r w -> p (r w)")
            sq = sqpool.tile([C, NIN], BF16, tag="sq")
            for c0, n in col_tiles(NIN):
                if sq_on_act:
                    nc.scalar.activation(sq[:, c0:c0 + n], xf[:, c0:c0 + n],
                                         AF.Square)
                else:
                    nc.vector.tensor_mul(sq[:, c0:c0 + n], xf[:, c0:c0 + n],
                                         xf[:, c0:c0 + n])
            sq_tiles[t] = (xt, sq)

        def stage_rest(t):
            """rms reduce (PE) + Sqrt (ACT) + reciprocal (DVE) + hn mul."""
            xt, sq = sq_tiles[t]
            xf = xt[:].rearrange("p r w -> p (r w)")
            hn = hnpool.tile([128, 1 + NIN + 1], BF16, tag="hn")
            nc.vector.memset(hn[0:C, 0:1], 0.0)
            nc.vector.memset(hn[0:C, 1 + NIN:], 0.0)
            for c0, n in col_tiles(NIN):
                ps = psN.tile([128, CT], F32, tag="psn")
                nc.tensor.matmul(ps[:, :n], sb_ones[:], sq[:, c0:c0 + n],
                                 start=True, stop=True)
                nc.scalar.activation(sq[:, c0:c0 + n], ps[:C, :n], AF.Ln,
                                     scale=1.0 / C, bias=c_eps)
                inv = statpool.tile([C, CT], F32, tag="inv")
                nc.scalar.activation(inv[:, :n], sq[:, c0:c0 + n], AF.Exp,
                                     scale=-0.5)
                nc.vector.tensor_mul(hn[0:C, 1 + c0:1 + c0 + n],
                                     xf[:, c0:c0 + n], inv[:, :n])
            hn_slabs[t] = hn
            return hn

        def build_in(t):
            """Slab-combining DMAs for conv_in(t) (+dup used by t+1).
            Needs hn[t] and hn[min(t+1, T-1)] already emitted."""
            cur = hn_slabs[t]
            nxt = hn_slabs[min(t + 1, T - 1)]
            if t + 1 < T:
                nc.gpsimd.dma_start(out=cur[96:128, :], in_=nxt[0:32, :])
            qi = qinpool.tile([128, 1 + NIN + 1], BF16, tag="qi")
            nc.gpsimd.dma_start(out=qi[0:64, :], in_=cur[32:96, :])
            nc.gpsimd.dma_start(out=qi[64:128, :], in_=nxt[0:64, :])
            qin_tiles[t] = qi
            rs = []
            for kh in range(3):
                r_ = rinpool.tile([96, NU], BF16, tag=f"r{kh}")
                for kw in range(3):
                    d = kh * Wp + kw
                    nc.gpsimd.dma_start(out=r_[32 * kw:32 * kw + 32, :],
                                        in_=nxt[64:96, d:d + NU])
                rs.append(r_)
            rin_tiles[t] = rs

        def conv_in_k(t, bt_arg, pair_range):
            a_sl = hn_slabs[max(t - 1, 0)]
            wa_sb = sb_wA0 if t == 0 else sb_wA
            ka = 96 if t == 0 else 128
            q = qin_tiles[t]
            rr = rin_tiles[t]

            streams = []
            for j in range(9):
                kh, kw = divmod(j, 3)
                s = kh * Wp + kw
                streams.append((
                    lambda m0, m1, jj=j: wa_sb[0:ka, jj, m0:m1],
                    lambda c0, n, ss=s: a_sl[0:ka, ss + c0:ss + c0 + n]))
            for j in range(9):
                kh, kw = divmod(j, 3)
                s = kh * Wp + kw
                streams.append((
                    lambda m0, m1, jj=j: sb_wB[:, jj, m0:m1],
                    lambda c0, n, ss=s: q[:, ss + c0:ss + c0 + n]))
            for kh in range(3):
                streams.append((
                    lambda m0, m1, kk=kh: sb_wR[:, kk, m0:m1],
                    lambda c0, n, kk=kh: rr[kk][:, c0:c0 + n]))
            NS = len(streams)

            bt = bt_arg
            cts = col_tiles(NU)
            for p0 in pair_range:
                pair = cts[p0:p0 + 2]
                pas = []
                for c0, n in pair:
                    pa = psA.tile([128, CT], F32, tag="pa")
                    for i, (lw, rh) in enumerate(streams):
                        nc.tensor.matmul(pa[:, :n], lw(0, 128), rh(c0, n),
                                         start=(i == 0), stop=(i == NS - 1))
                    pas.append(pa)
                pg = psG.tile([128, CT], F32, tag="pg")
                for i, (lw, rh) in enumerate(streams):
                    for j, (c0, n) in enumerate(pair):
                        b = 64 * j
                        nc.tensor.matmul(
                            pg[b:b + 64, :n], lw(128, 192), rh(c0, n),
                            start=(i == 0), stop=(i == NS - 1),
                            tile_position=(0, b), skip_group_check=True)
                for j, (c0, n) in enumerate(pair):
                    b = 64 * j
                    pa = pas[j]
                    if ag_swap:
                        # Output channels are permuted so g (96) sits aligned
                        # at psum partitions 0:96 while a rides the spare
                        # slots: a[0:32] at pa[96:128], a[32:96] at pg[b:b+64].
                        # Silu runs on the pieces in place (ACT reads PSUM), a
                        # DMA realigns the bf16 silu outputs, and the final STT
                        # reads g straight from PSUM — no DVE casts.
                        sa = sapool.tile([128, CT], BF16, tag="sa")
                        nc.scalar.activation(sa[96:128, :n], pa[96:128, :n],
                                             AF.Silu, bias=c_ba_hi[96:128])
                        sb_ = sapool.tile([128, CT], BF16, tag="sb")
                        nc.scalar.activation(sb_[b:b + 64, :n],
                                             pg[b:b + 64, :n],
                                             AF.Silu, bias=c_ba_pg[b:b + 64])
                        gf = gfpool.tile([C, CT], BF16, tag="gf")
                        nc.sync.dma_start(out=gf[0:32, :n], in_=sa[96:128, :n])
                        nc.sync.dma_start(out=gf[32:96, :n],
                                          in_=sb_[b:b + 64, :n])
                        nc.vector.scalar_tensor_tensor(bt[:, c0:c0 + n],
                                                       pa[:C, :n], c_bg,
                                                       gf[:, :n],
                                                       ALU.add, ALU.mult)
                    else:
                        gsa = gspool.tile([128, CT], BF16, tag="gsa")
                        nc.vector.tensor_copy(gsa[96:128, :n], pa[96:128, :n])
                        gsb = gspool.tile([128, CT], BF16, tag="gsb")
                        nc.vector.tensor_copy(gsb[b:b + 64, :n],
                                              pg[b:b + 64, :n])
                        gf = gfpool.tile([C, CT], BF16, tag="gf")
                        nc.sync.dma_start(out=gf[0:32, :n], in_=gsa[96:128, :n])
                        nc.sync.dma_start(out=gf[32:96, :n],
                                          in_=gsb[b:b + 64, :n])
                        sa = sapool.tile([C, CT], BF16, tag="sa")
                        nc.scalar.activation(sa[:, :n], pa[:C, :n], AF.Silu,
                                             bias=c_ba)
                        nc.vector.scalar_tensor_tensor(bt[:, c0:c0 + n],
                                                       gf[:, :n], c_bg,
                                                       sa[:, :n],
                                                       ALU.add, ALU.mult)

        hr = singles.tile([C, NU], F32)
        hi = singles.tile([C, NU], F32)

        NH = (RU // 2) * Wp  # first-half columns (rows 0..RU/2-1)

        def scan_half(t, bt, yt, h0, h1):
            hrh = hr[:, h0:h1]
            hih = hi[:, h0:h1]
            bth = bt[:, h0:h1]
            if t == 0:
                nc.vector.tensor_copy(hrh, bth)
                nc.vector.memset(hih, 0.0)
            else:
                t1 = tmppool.tile([C, NH], F32, tag="tA")
                nc.vector.scalar_tensor_tensor(t1[:, :h1 - h0], hih, c_nli,
                                               bth, ALU.mult, ALU.add)
                nc.vector.scalar_tensor_tensor(hih, hih, c_lr, hih,
                                               ALU.mult, ALU.bypass)
                nc.vector.scalar_tensor_tensor(hih, hrh, c_li, hih,
                                               ALU.mult, ALU.add)
                nc.vector.scalar_tensor_tensor(hrh, hrh, c_lr,
                                               t1[:, :h1 - h0],
                                               ALU.mult, ALU.add)
            t3 = tmppool.tile([C, NH], F32, tag="tA")
            nc.vector.scalar_tensor_tensor(t3[:, :h1 - h0], hrh, c_gcre,
                                           hrh, ALU.mult, ALU.bypass)
            nc.vector.scalar_tensor_tensor(yt[0:C, 1 + h0:1 + h1], hih,
                                           c_gcim, t3[:, :h1 - h0],
                                           ALU.mult, ALU.add)
            yv = yt[0:C, 1 + h0:1 + h1].rearrange("p (r w) -> p r w", w=Wp)
            nr = (h1 - h0) // Wp
            nc.vector.tensor_copy(yv[:, :, 0:1], yv[:, :, W:W + 1])
            nc.vector.tensor_copy(yv[:, :, W + 1:W + 2], yv[:, :, 1:2])
            if h0 == 0:
                e0 = tmppool.tile([C, Wp], F32, tag="tE")
                nc.vector.scalar_tensor_tensor(e0[:], yv[:, 1, :], c_1m0,
                                               yv[:, 1, :], ALU.mult,
                                               ALU.bypass)
                nc.vector.scalar_tensor_tensor(yv[:, 0, :], yv[:, 0, :],
                                               c_m0, e0[:], ALU.mult,
                                               ALU.add)
            else:
                e1 = tmppool.tile([C, Wp], F32, tag="tE")
                nc.vector.scalar_tensor_tensor(e1[:], yv[:, nr - 2, :],
                                               c_1m1, yv[:, nr - 2, :],
                                               ALU.mult, ALU.bypass)
                nc.vector.scalar_tensor_tensor(yv[:, nr - 1, :],
                                               yv[:, nr - 1, :], c_m1,
                                               e1[:], ALU.mult, ALU.add)

        def conv_out_k(t):
            """Direct 27-tap conv_out: kt-ordered so the y[t]-dependent
            taps (kt2) come last in each accumulation group."""
            slabs = [y_slabs[min(max(t + kt - 1, 0), T - 1)] for kt in range(3)]
            ot = opool.tile([C, HR, Wp], F32, tag="ot")
            nc.sync.dma_start(out=ot[:], in_=xh[:, t, 2:2 + HR, :])
            touch(ot[:, 0, 0:1], engines="v")
            of = ot[:].rearrange("p r w -> p (r w)")
            for c0, n in col_tiles(NO):
                po = psO.tile([C, CT], F32, tag="po", name="po")
                for kt in range(3):
                    rhs_all = slabs[kt]
                    for kh in range(3):
                        for kw in range(3):
                            off = kt * 9 + kh * 3 + kw
                            s = 1 + c0 + kh * Wp + kw - 1
                            nc.tensor.matmul(po[:, :n], sb_wK[:, off, :],
                                             rhs_all[0:C, s:s + n],
                                             start=(off == 0),
                                             stop=(off == 26))
                nc.vector.scalar_tensor_tensor(of[:, c0:c0 + n], po[:, :n],
                                               c_bout, of[:, c0:c0 + n],
                                               ALU.add, ALU.add)
            nc.sync.dma_start(out=out[:, t], in_=ot[:, :, 1:1 + W])

        # ---- K+M-packed conv_out ------------------------------------------
        # Same 21-stream K-regrouping as conv_in (A: y[s-1]96 + y[s]0:32 via
        # dup rows; Q: y[s]32:96 + y[s+1]0:64 materialized; R: y[s+1]64:96 at
        # 9 kw shifts), and the 96-wide M packed over 4 col tiles in 3 passes
        # (main96 + next tile's 32-rider, then a 64||64 pg pass). Outputs use
        # exact 4-row x 128 col tiles via 3D rhs APs (no wrap-col compute).
        qro = {}          # s -> (qo tile, [r tiles kh=0..2])
        RKW = HR * Wp     # R-tile width (only (HR-1)*Wp + W + ... used)

        def build_out(t):
            """After scan(t): dup rows for y[t-1]; qo/R for conv_out(t-1)."""
            if t == 0:
                return
            s = t - 1
            cur, nxt = y_slabs[s], y_slabs[t]
            nc.gpsimd.dma_start(out=cur[96:128, :], in_=nxt[0:32, :])
            _build_qr(s, cur, nxt)

        def _build_qr(s, cur, nxt):
            qo = qopool.tile([128, 1 + NU + 1], BF16, tag="qo")
            nc.gpsimd.dma_start(out=qo[0:64, :], in_=cur[32:96, :])
            nc.gpsimd.dma_start(out=qo[64:128, :], in_=nxt[0:64, :])
            rs = []
            for kh in range(3):
                r_ = rkpool.tile([96, RKW], BF16, tag=f"rk{kh}")
                for kw in range(3):
                    d = kh * Wp + kw
                    nw = (HR - 1) * Wp + W + 2 - kw
                    nc.sync.dma_start(out=r_[32 * kw:32 * kw + 32, 0:nw],
                                      in_=nxt[64:96, 1 + d:1 + d + nw])
                rs.append(r_)
            qro[s] = (qo, rs)

        def conv_out_kp(s, t):
            """Emit the 3 matmul passes + epilogue for output timestep s,
            writing out[:, s]. All rhs views are 4-row x 128-col 3D APs."""
            prev = y_slabs[max(s - 1, 0)]
            wa_sb = sb_wKA0 if s == 0 else sb_wKA
            ka = 96 if s == 0 else 128
            qo, rs = qro[s]
            pv = prev[:, 1:1 + NU].rearrange("p (r w) -> p r w", w=Wp)
            qv = qo[:, 1:1 + NU].rearrange("p (r w) -> p r w", w=Wp)
            rv = [r_[:].rearrange("p (r w) -> p r w", w=Wp) for r_ in rs]

            streams = []
            for j in range(9):
                kh, kw = divmod(j, 3)
                streams.append((
                    lambda m0, m1, jj=j: wa_sb[0:ka, jj, m0:m1],
                    lambda r0, kh=kh, kw=kw:
                        pv[0:ka, r0 + kh:r0 + kh + 4, kw:kw + 128]))
            for j in range(9):
                kh, kw = divmod(j, 3)
                streams.append((
                    lambda m0, m1, jj=j: sb_wKB[:, jj, m0:m1],
                    lambda r0, kh=kh, kw=kw:
                        qv[:, r0 + kh:r0 + kh + 4, kw:kw + 128]))
            for kh in range(3):
                streams.append((
                    lambda m0, m1, kk=kh: sb_wKR[:, kk, m0:m1],
                    lambda r0, kk=kh: rv[kk][:, r0:r0 + 4, 0:128]))
            NS = len(streams)

            p1 = psO.tile([128, CT], F32, tag="po1")
            p2 = psO.tile([128, CT], F32, tag="po2")
            p3 = psO.tile([128, CT], F32, tag="po3")
            # (prange, wslice, out-row r0, tile_position)
            jobs = [
                (p1, (0, 64), (0, 64), 0, (0, 0)),
                (p1, (64, 96), (64, 96), 0, (0, 64)),
                (p1, (96, 128), (0, 32), 4, (0, 96)),
                (p2, (0, 64), (0, 64), 8, (0, 0)),
                (p2, (64, 96), (64, 96), 8, (0, 64)),
                (p2, (96, 128), (0, 32), 12, (0, 96)),
                (p3, (0, 64), (32, 96), 4, (0, 0)),
                (p3, (64, 128), (32, 96), 12, (0, 64)),
            ]
            for i, (lw, rh) in enumerate(streams):
                for ps, prange, wslice, r0, pos in jobs:
                    nc.tensor.matmul(ps[prange[0]:prange[1], :],
                                     lw(wslice[0], wslice[1]), rh(r0),
                                     start=(i == 0), stop=(i == NS - 1),
                                     tile_position=pos, skip_group_check=True)

            # Epilogue: residual + bias per piece, direct DMA to DRAM rows.
            def xs_load(tile, p0, p1_, ch0, r0):
                nc.sync.dma_start(
                    out=tile[p0:p1_, :],
                    in_=xh[ch0:ch0 + p1_ - p0, s, 2 + r0:2 + r0 + 4, 1:1 + W])
                touch(tile[p0, 0:1] if False else tile[p0:p0 + 1, 0:1])

            def emit_out(stage, p0, p1_, ch0, r0):
                nc.sync.dma_start(out=out[ch0:ch0 + p1_ - p0, s, r0:r0 + 4, :],
                                  in_=stage[p0:p1_, :])

            # aligned tiles r0=0 (p1[0:96]) and r0=8 (p2[0:96])
            for psrc, r0 in ((p1, 0), (p2, 8)):
                xs = xspool.tile([128, CT], F32, tag="xse")
                xs_load(xs, 0, 96, 0, r0)
                og = ostgpool.tile([128, CT], F32, tag="oge")
                nc.vector.scalar_tensor_tensor(og[0:96, :], psrc[0:96, :],
                                               c_bout, xs[0:96, :],
                                               ALU.add, ALU.add)
                emit_out(og, 0, 96, 0, r0)
            # misaligned pieces: (psum, prange, ch0, r0, aux bias col)
            pieces = [
                (p1, 96, 128, 0, 4, 2),
                (p3, 0, 64, 32, 4, 3),
                (p2, 96, 128, 0, 12, 2),
                (p3, 64, 128, 32, 12, 4),
            ]
            for psrc, p0, p1_, ch0, r0, bcol in pieces:
                xs = xspool.tile([128, CT], F32, tag="xsm")
                xs_load(xs, p0, p1_, ch0, r0)
                og = ostgpool.tile([128, CT], F32, tag="ogm")
                nc.vector.scalar_tensor_tensor(og[p0:p1_, :], psrc[p0:p1_, :],
                                               sb_aux[p0:p1_, bcol:bcol + 1],
                                               xs[p0:p1_, :],
                                               ALU.add, ALU.add)
                emit_out(og, p0, p1_, ch0, r0)

        for u in range(min(3, T)):
            stage_sq(u)
            stage_rest(u)
        emit_weight_dmas()
        build_in(0)
        for t in range(T):
            if t + 1 < T:
                build_in(t + 1)
            if t + 3 < T:
                stage_sq(t + 3)
            bt = bpool.tile([C, NU], BF16, tag="bt")
            yt = ypool.tile([128, 1 + NU + 1], BF16, tag="yt")
            nc.vector.memset(yt[0:C, 0:1], 0.0)
            nc.vector.memset(yt[0:C, 1 + NU:], 0.0)
            conv_in_k(t, bt, [0, 2])
            scan_half(t, bt, yt, 0, NH)
            conv_in_k(t, bt, [4])
            scan_half(t, bt, yt, NH, NU)
            y_slabs[t] = yt
            if t >= 1:
                conv_out_k(t - 1)
            if t + 3 < T:
                stage_rest(t + 3)
        conv_out_k(T - 1)

    nc.compile()
    return nc



def prep_core_inputs(x, norm_w, conv_in_w, conv_in_b, nu_log, theta_log,
                     c_re, c_im, conv_out_w, conv_out_b, n_qh):
    """Build per-core input maps. Cores = batch-major, then H quarters."""
    B, C, T, H, W = x.shape
    HR = H // n_qh

    nu = np.exp(np.asarray(nu_log, np.float64))
    theta = np.exp(np.asarray(theta_log, np.float64))
    lam_re = (np.exp(-nu) * np.cos(theta)).astype(np.float32)
    lam_im = (np.exp(-nu) * np.sin(theta)).astype(np.float32)
    gamma = np.sqrt(1.0 - np.exp(-2.0 * nu))
    gcre = (gamma * np.asarray(c_re, np.float64)).astype(np.float32)
    gcim = (gamma * np.asarray(c_im, np.float64)).astype(np.float32)

    w_in_f = np.asarray(conv_in_w, np.float32) * \
        np.asarray(norm_w, np.float32)[None, :, None, None, None]
    w_in_t = np.ascontiguousarray(
        np.transpose(w_in_f, (1, 2, 3, 4, 0)).reshape(C, 27, 2 * C)
    ).astype(ml_dtypes.bfloat16)
    w_out_t = np.ascontiguousarray(
        np.transpose(np.asarray(conv_out_w, np.float32),
                     (1, 2, 3, 4, 0)).reshape(C, 27, C)
    ).astype(ml_dtypes.bfloat16)
    ones = np.ones((C, 128), ml_dtypes.bfloat16)

    xp = np.concatenate([x[..., -1:], x, x[..., :1]], axis=-1)  # W circular

    in_maps = []
    for b in range(B):
        for q in range(n_qh):
            rows = np.clip(np.arange(q * HR - 2, q * HR + HR + 2), 0, H - 1)
            xh = np.ascontiguousarray(xp[b][:, :, rows, :]).astype(np.float32)
            m0 = 0.0 if q == 0 else 1.0
            m1 = 0.0 if q == n_qh - 1 else 1.0
            cvec = np.stack([
                np.asarray(conv_in_b, np.float32)[:C],
                np.asarray(conv_in_b, np.float32)[C:],
                lam_re, lam_im, -lam_im, gcre, gcim,
                np.asarray(conv_out_b, np.float32),
                np.full(C, m0, np.float32), np.full(C, 1.0 - m0, np.float32),
                np.full(C, m1, np.float32), np.full(C, 1.0 - m1, np.float32),
                np.full(C, EPS, np.float32),
            ], axis=1)
            bo = np.asarray(conv_out_b, np.float32)
            aux = np.zeros((128, 2), np.float32)
            aux[96:128, 0] = bo[0:32]
            aux[:, 1] = bo[32 + (np.arange(128) % 64)]
            in_maps.append({
                "xh": xh,
                "w_in": w_in_t,
                "w_out": w_out_t,
                "onesw": ones,
                "consts": np.ascontiguousarray(cvec),
                "consts2": aux,
            })
    return in_maps


def prep_core_inputs_k(x, norm_w, conv_in_w, conv_in_b, nu_log, theta_log,
                       c_re, c_im, conv_out_w, conv_out_b, n_qh):
    """Per-core inputs for the K=128-packed program."""
    B, C, T, H, W = x.shape
    HR = H // n_qh
    C2 = 2 * C

    nu = np.exp(np.asarray(nu_log, np.float64))
    theta = np.exp(np.asarray(theta_log, np.float64))
    lam_re = (np.exp(-nu) * np.cos(theta)).astype(np.float32)
    lam_im = (np.exp(-nu) * np.sin(theta)).astype(np.float32)
    gamma = np.sqrt(1.0 - np.exp(-2.0 * nu))
    gcre = (gamma * np.asarray(c_re, np.float64)).astype(np.float32)
    gcim = (gamma * np.asarray(c_im, np.float64)).astype(np.float32)

    w_in_f = np.asarray(conv_in_w, np.float32) * \
        np.asarray(norm_w, np.float32)[None, :, None, None, None]
    # wt[cin, kt, kh, kw, cout]; cout permuted so g-channels (96:192) land
    # first (psum-aligned) and a-channels ride the spare packing slots.
    wt = np.transpose(w_in_f, (1, 2, 3, 4, 0))
    if os.environ.get("KERNEL_AGSWAP", "1") == "1":
        perm = np.concatenate([np.arange(C, 2 * C), np.arange(0, C)])
        wt = np.ascontiguousarray(wt[..., perm])
    wto = np.transpose(np.asarray(conv_out_w, np.float32), (1, 2, 3, 4, 0))

    def pack(w, co):
        """w: [cin, kt, kh, kw, co] -> (wA, wB, wR, wR3, wA0)."""
        wA = np.zeros((128, 9, co), np.float32)
        wB = np.zeros((128, 9, co), np.float32)
        wA0 = np.zeros((96, 9, co), np.float32)
        for j in range(9):
            kh, kw = divmod(j, 3)
            wA[0:96, j] = w[:, 0, kh, kw]
            wA[96:128, j] = w[0:32, 1, kh, kw]
            wB[0:64, j] = w[32:96, 1, kh, kw]
            wB[64:128, j] = w[0:64, 2, kh, kw]
            wA0[0:32, j] = w[0:32, 0, kh, kw] + w[0:32, 1, kh, kw]
            wA0[32:96, j] = w[32:96, 0, kh, kw]
        wR = np.zeros((96, 3, co), np.float32)
        for kh in range(3):
            for kw in range(3):
                wR[32 * kw:32 * (kw + 1), kh] = w[64:96, 2, kh, kw]
        bf = ml_dtypes.bfloat16
        return (np.ascontiguousarray(wA).astype(bf),
                np.ascontiguousarray(wB).astype(bf),
                np.ascontiguousarray(wR).astype(bf),
                np.ascontiguousarray(wA0).astype(bf))

    wA, wB, wR, wA0 = pack(wt, C2)
    wK = np.ascontiguousarray(
        wto.reshape(C, 27, C)).astype(ml_dtypes.bfloat16)
    ones = np.ones((C, 128), ml_dtypes.bfloat16)

    xp = np.concatenate([x[..., -1:], x, x[..., :1]], axis=-1)  # W circular

    in_maps = []
    for b in range(B):
        for q in range(n_qh):
            rows = np.clip(np.arange(q * HR - 2, q * HR + HR + 2), 0, H - 1)
            xh = np.ascontiguousarray(xp[b][:, :, rows, :]).astype(np.float32)
            m0 = 0.0 if q == 0 else 1.0
            m1 = 0.0 if q == n_qh - 1 else 1.0
            cvec = np.stack([
                np.asarray(conv_in_b, np.float32)[:C],
                np.asarray(conv_in_b, np.float32)[C:],
                lam_re, lam_im, -lam_im, gcre, gcim,
                np.asarray(conv_out_b, np.float32),
                np.full(C, m0, np.float32), np.full(C, 1.0 - m0, np.float32),
                np.full(C, m1, np.float32), np.full(C, 1.0 - m1, np.float32),
                np.full(C, EPS, np.float32),
            ], axis=1)
            ba = np.asarray(conv_in_b, np.float32)[:C]
            bo = np.asarray(conv_out_b, np.float32)
            aux_np = np.zeros((128, 8), np.float32)
            aux_np[96:128, 0] = ba[0:32]
            aux_np[0:64, 1] = ba[32:96]
            aux_np[64:128, 1] = ba[32:96]
            aux_np[96:128, 2] = bo[0:32]
            aux_np[0:64, 3] = bo[32:96]
            aux_np[64:128, 4] = bo[32:96]
            in_maps.append({
                "xh": xh,
                "wA": wA, "wB": wB, "wR": wR, "wA0": wA0,
                "wK": wK,
                "onesw": ones,
                "consts": np.ascontiguousarray(cvec),
                "aux128": aux_np,
            })
    return in_maps


LAST_RESULT = None  # BassKernelResults of the most recent kernel() call


def _fix_act_tables():
    """Make Ln/Exp resolve to the combined natural_log_exp_and_others set.

    The act-table-load placement pass picks each activation's first
    containing set; Ln's home (natural_log) differs from Exp's
    (exp_and_others), so an interleaved Ln/Exp stream reloads tables on
    every op (~2.7us each). Removing ln/exp from all other sets (in the
    cached dict, same keys/order, so set ids stay valid) forces both onto
    the one set that holds them together."""
    from concourse.hw_specs import get_activation_tables
    AFt = mybir.ActivationFunctionType
    for arch in ("gen3",):
        try:
            tables = get_activation_tables(arch)
        except Exception:
            continue
        for name, fns in tables.items():
            if name != "natural_log_exp_and_others":
                fns.discard(AFt.Ln)
                fns.discard(AFt.Exp)


def kernel(x, norm_w, conv_in_w, conv_in_b, nu_log, theta_log, c_re, c_im,
           conv_out_w, conv_out_b):
    global LAST_RESULT
    from concourse.bass_utils import run_bass_kernel_spmd

    if os.environ.get("KERNEL_ACTFIX", "1") == "1":
        _fix_act_tables()

    x = np.asarray(x, np.float32)
    B, C, T, H, W = x.shape
    HR = H // QH
    if os.environ.get("KERNEL_KPACK", "1") == "1":
        in_maps = prep_core_inputs_k(x, norm_w, conv_in_w, conv_in_b, nu_log,
                                     theta_log, c_re, c_im, conv_out_w,
                                     conv_out_b, QH)
        nc = build_program_k(C=C, T=T, HR=HR, W=W, CT=512)
    else:
        in_maps = prep_core_inputs(x, norm_w, conv_in_w, conv_in_b, nu_log,
                                   theta_log, c_re, c_im, conv_out_w,
                                   conv_out_b, QH)
        nc = build_program(C=C, T=T, HR=HR, W=W, CT=512,
                           use_silu=os.environ.get("KERNEL_NO_SILU", "") != "1",
                           pack=os.environ.get("KERNEL_PACK", "1") == "1",
                           pack2=os.environ.get("KERNEL_PACK2", "0") == "1")
    trace = os.environ.get("KERNEL_TRACE", "") == "1"
    res = run_bass_kernel_spmd(nc, in_maps, list(range(N_CORES)), trace=trace)
    LAST_RESULT = res
    out = np.empty((B, C, T, H, W), np.float32)
    for core in range(N_CORES):
        b, q = core // QH, core % QH
        out[b, :, :, q * HR:(q + 1) * HR, :] = res.results[core]["out"]
    return out



# revision 41
# speedup vs baseline: 1.2388x; 1.1861x over previous
"""ConvLRUBlock Trainium2 kernel.

Reference computation (per batch b):
    h   = rms_norm(x, norm_w)                  # over channel dim
    uv  = conv3d_3x3x3(h, w_in) + b_in         # pad: replicate T/H, circular W
    u   = silu(a) * g          (a, g = uv split on channels)
    y_t = Re(h_t) c_re + Im(h_t) c_im,  h_t = lam h_{t-1} + gamma u_t  (diag LRU)
    out = x + conv3d_3x3x3(y, w_out) + b_out

Sharding: 8 cores = (batch 2) x (H quarters 4). Each core receives its H
slice plus 2 halo rows each side (edge-replicated) and the W dim circularly
padded to W+2, so no inter-core communication is needed. All conv padding is
resolved by host-side halo materialization + in-kernel index clamping (T) +
in-SBUF wrap-column fixes (W for the second conv).

In-kernel layout: channels (96) on SBUF partitions; spatial (rows x (W+2))
flattened on the free dim. 3x3x3 convs = 27 accumulating matmuls per output
tile; kh/kw become column shifts of the rhs AP, kt picks one of 3 t-slabs.
The LRU scan is 16 sequential complex steps on the vector engine.
"""

import os
from contextlib import ExitStack

import ml_dtypes
import numpy as np

import concourse.bacc as bacc
import concourse.bass as bass  # noqa: F401
import concourse.tile as tile
from concourse import mybir

F32 = mybir.dt.float32
BF16 = mybir.dt.bfloat16
ALU = mybir.AluOpType
AF = mybir.ActivationFunctionType

EPS = 1e-6

# Full-problem constants
B_FULL, C_FULL, T_FULL, H_FULL, W_FULL = 2, 96, 16, 64, 128
QH = 4  # H quarters
N_CORES = 8


def build_program(C=96, T=16, HR=16, W=128, CT=512, use_silu=True,
                  pack=False, pack2=False):
    """Build the single-core SPMD Bass program.

    C: channels; T: time steps; HR: output H rows per core; W: width.
    CT: matmul/psum column tile (<=512). use_silu: Silu on ACT vs
    Sigmoid+mults (the simulator does not implement Silu).
    """
    Wp = W + 2           # circular-padded width
    RIN = HR + 4         # input rows (2 halo each side, for two convs)
    RU = HR + 2          # u/y rows (1 halo each side, for conv_out)
    NIN = RIN * Wp       # flattened input cols per t
    NU = RU * Wp         # flattened u/y cols per t
    NO = HR * Wp         # flattened output cols per t

    nc = bacc.Bacc()
    xh = nc.declare_dram_parameter("xh", [C, T, RIN, Wp], F32, isOutput=False)
    w_in = nc.declare_dram_parameter("w_in", [C, 27, 2 * C], BF16, isOutput=False)
    w_out = nc.declare_dram_parameter("w_out", [C, 27, C], BF16, isOutput=False)
    onesw = nc.declare_dram_parameter("onesw", [C, 128], BF16, isOutput=False)
    consts = nc.declare_dram_parameter("consts", [C, 13], F32, isOutput=False)
    consts2 = nc.declare_dram_parameter("consts2", [128, 2], F32, isOutput=False)
    out = nc.declare_dram_parameter("out", [C, T, HR, W], F32, isOutput=True)

    def col_tiles(total):
        return [(i, min(CT, total - i)) for i in range(0, total, CT)]

    with tile.TileContext(nc) as tc, ExitStack() as ctx:
        singles = ctx.enter_context(tc.tile_pool(name="singles", bufs=1))
        xpool = ctx.enter_context(tc.tile_pool(name="xpool", bufs=2))
        sqpool = ctx.enter_context(tc.tile_pool(name="sqpool", bufs=2))
        statpool = ctx.enter_context(tc.tile_pool(name="statpool", bufs=2))
        hnpool = ctx.enter_context(tc.tile_pool(name="hnpool", bufs=4))
        sapool = ctx.enter_context(tc.tile_pool(name="sapool", bufs=3))
        bpool = ctx.enter_context(tc.tile_pool(name="bpool", bufs=2))
        hrpool = ctx.enter_context(tc.tile_pool(name="hrpool", bufs=2))
        hipool = ctx.enter_context(tc.tile_pool(name="hipool", bufs=2))
        tmppool = ctx.enter_context(tc.tile_pool(name="tmppool", bufs=2))
        ypool = ctx.enter_context(tc.tile_pool(name="ypool", bufs=4))
        opool = ctx.enter_context(tc.tile_pool(name="opool", bufs=2))
        psN = ctx.enter_context(tc.tile_pool(name="psN", bufs=2, space="PSUM"))
        psA = ctx.enter_context(tc.tile_pool(name="psA", bufs=2, space="PSUM"))
        psG = ctx.enter_context(tc.tile_pool(name="psG", bufs=2, space="PSUM"))
        psO = ctx.enter_context(tc.tile_pool(name="psO", bufs=2, space="PSUM"))

        sb_win = singles.tile([C, 27, 2 * C], BF16)
        nc.sync.dma_start(out=sb_win[:], in_=w_in[:])
        sb_wout = singles.tile([C, 27, C], BF16)
        nc.sync.dma_start(out=sb_wout[:], in_=w_out[:])
        sb_ones = singles.tile([C, 128], BF16)
        nc.sync.dma_start(out=sb_ones[:], in_=onesw[:])
        sb_c = singles.tile([C, 13], F32)
        nc.sync.dma_start(out=sb_c[:], in_=consts[:])
        sb_c2 = singles.tile([128, 2], F32)
        nc.sync.dma_start(out=sb_c2[:], in_=consts2[:])
        c_aux0 = sb_c2[:, 0:1]
        c_aux1 = sb_c2[:, 1:2]
        c_ba = sb_c[:, 0:1]
        c_bg = sb_c[:, 1:2]
        c_lr = sb_c[:, 2:3]
        c_li = sb_c[:, 3:4]
        c_nli = sb_c[:, 4:5]
        c_gcre = sb_c[:, 5:6]
        c_gcim = sb_c[:, 6:7]
        c_bout = sb_c[:, 7:8]
        c_m0 = sb_c[:, 8:9]
        c_1m0 = sb_c[:, 9:10]
        c_m1 = sb_c[:, 10:11]
        c_1m1 = sb_c[:, 11:12]
        c_eps = sb_c[:, 12:13]

        # Warm-up reads: make each compute engine observe the const-DMA
        # semaphores early, so steady-state ops carry at most one sync wait
        # (walrus rejects DVE ops with two wait commands).
        wu_v = singles.tile([C, 13], F32)
        nc.vector.tensor_copy(wu_v[:], sb_c[:])
        wu_s = singles.tile([C, 13], F32)
        nc.scalar.activation(wu_s[:], sb_c[:], AF.Square)

        touchpool = ctx.enter_context(tc.tile_pool(name="touchpool", bufs=2))
        if pack:
            gspool = ctx.enter_context(tc.tile_pool(name="gspool", bufs=2))
            gfpool = ctx.enter_context(tc.tile_pool(name="gfpool", bufs=2))
        if pack2:
            piecepool = ctx.enter_context(tc.tile_pool(name="piecepool", bufs=2))

        def touch(ap, engines="v"):
            """Tiny read of a freshly-DMA'd tile so the engine observes the
            DMA-queue semaphore here; later big consumers then carry only
            engine-sem waits (walrus rejects DVE ops with 2 sync waits)."""
            if "v" in engines:
                tv = touchpool.tile([C, 1], F32, tag="tv")
                nc.vector.tensor_copy(tv[:], ap)
            if "s" in engines:
                ts_ = touchpool.tile([C, 1], F32, tag="ts")
                nc.scalar.activation(ts_[:], ap, AF.Square)

        hn_slabs = [None] * T  # hnorm tiles, data at col offset 1
        y_slabs = [None] * T   # y tiles (bf16), data at col offset 1

        def stage_a(t):
            """x[t] -> hnorm[t] (rms-normed, bf16, [C, 1+NIN+1])."""
            xt = xpool.tile([C, RIN, Wp], F32, tag="xt")
            nc.sync.dma_start(out=xt[:], in_=xh[:, t])
            touch(xt[:, 0, 0:1], engines="vs")
            xf = xt[:].rearrange("p r w -> p (r w)")
            hn = hnpool.tile([C, 1 + NIN + 1], BF16, tag="hn")
            nc.vector.memset(hn[:, 0:1], 0.0)
            nc.vector.memset(hn[:, 1 + NIN:], 0.0)
            for c0, n in col_tiles(NIN):
                sq = sqpool.tile([C, CT], BF16, tag="sq")
                nc.scalar.activation(sq[:, :n], xf[:, c0:c0 + n], AF.Square)
                ps = psN.tile([128, CT], F32, tag="psn")
                nc.tensor.matmul(ps[:, :n], sb_ones[:], sq[:, :n],
                                 start=True, stop=True)
                lg = statpool.tile([C, CT], F32, tag="lg")
                nc.scalar.activation(lg[:, :n], ps[:C, :n], AF.Ln,
                                     scale=1.0 / C, bias=c_eps)
                inv = statpool.tile([C, CT], F32, tag="inv")
                nc.scalar.activation(inv[:, :n], lg[:, :n], AF.Exp, scale=-0.5)
                nc.vector.tensor_mul(hn[:, 1 + c0:1 + c0 + n],
                                     xf[:, c0:c0 + n], inv[:, :n])
            hn_slabs[t] = hn
            return hn

        def gate_epilogue(pa, pg_sb, bt, c0, n):
            """silu(a+ba)*(g+bg) for one coltile; a=pa[0:C] (psum),
            g already realigned to pg_sb [C, n] (sbuf)."""
            if use_silu:
                sa = sapool.tile([C, CT], BF16, tag="sa")
                nc.scalar.activation(sa[:, :n], pa[:C, :n], AF.Silu,
                                     bias=c_ba)
            else:
                sg = sapool.tile([C, CT], BF16, tag="sg")
                nc.scalar.activation(sg[:, :n], pa[:C, :n], AF.Sigmoid,
                                     bias=c_ba)
                av = sapool.tile([C, CT], F32, tag="av")
                nc.vector.scalar_tensor_tensor(av[:, :n], pa[:C, :n], c_ba,
                                               sg[:, :n], ALU.add,
                                               ALU.bypass)
                sa = sapool.tile([C, CT], BF16, tag="sa")
                nc.vector.tensor_mul(sa[:, :n], sg[:, :n], av[:, :n])
            nc.vector.scalar_tensor_tensor(bt[:, c0:c0 + n], pg_sb[:, :n],
                                           c_bg, sa[:, :n],
                                           ALU.add, ALU.mult)

        def conv_in_packed(t):
            """1.5-array-pass conv_in: pass1 M=128 (a0..95,g0..31), pass2
            col-tiled pairs of M=64 (g32..95) for two coltiles at once."""
            slabs = [hn_slabs[min(max(t + kt - 1, 0), T - 1)] for kt in range(3)]
            bt = bpool.tile([C, NU], BF16, tag="bt")
            cts = col_tiles(NU)
            for p0 in range(0, len(cts), 2):
                pair = cts[p0:p0 + 2]
                pas = []
                for c0, n in pair:
                    pa = psA.tile([128, CT], F32, tag="pa")
                    for kt in range(3):
                        rhs_all = slabs[kt]
                        for kh in range(3):
                            for kw in range(3):
                                off = kt * 9 + kh * 3 + kw
                                s = 1 + c0 + kh * Wp + kw - 1
                                nc.tensor.matmul(
                                    pa[:, :n], sb_win[:, off, 0:128],
                                    rhs_all[:, s:s + n],
                                    start=(off == 0), stop=(off == 26))
                    pas.append(pa)
                pg = psG.tile([128, CT], F32, tag="pg")
                for kt in range(3):
                    rhs_all = slabs[kt]
                    for kh in range(3):
                        for kw in range(3):
                            off = kt * 9 + kh * 3 + kw
                            for j, (c0, n) in enumerate(pair):
                                s = 1 + c0 + kh * Wp + kw - 1
                                b = 64 * j
                                nc.tensor.matmul(
                                    pg[b:b + 64, :n],
                                    sb_win[:, off, 128:192],
                                    rhs_all[:, s:s + n],
                                    start=(off == 0), stop=(off == 26),
                                    tile_position=(0, b),
                                    skip_group_check=True)
                for j, (c0, n) in enumerate(pair):
                    b = 64 * j
                    pa = pas[j]
                    gsa = gspool.tile([128, CT], F32, tag="gsa")
                    nc.vector.tensor_copy(gsa[96:128, :n], pa[96:128, :n])
                    gsb = gspool.tile([128, CT], F32, tag="gsb")
                    nc.vector.tensor_copy(gsb[b:b + 64, :n], pg[b:b + 64, :n])
                    gf = gfpool.tile([C, CT], F32, tag="gf")
                    nc.sync.dma_start(out=gf[0:32, :n], in_=gsa[96:128, :n])
                    nc.sync.dma_start(out=gf[32:96, :n], in_=gsb[b:b + 64, :n])
                    gate_epilogue(pa, gf, bt, c0, n)
            return bt

        def conv_in(t):
            """hnorm[t-1..t+1] -> b[t] = silu(a+ba)*(g+bg), bf16 [C, NU]."""
            slabs = [hn_slabs[min(max(t + kt - 1, 0), T - 1)] for kt in range(3)]
            bt = bpool.tile([C, NU], BF16, tag="bt")
            for c0, n in col_tiles(NU):
                pa = psA.tile([C, CT], F32, tag="pa")
                pg = psG.tile([C, CT], F32, tag="pg")
                for kt in range(3):
                    rhs_all = slabs[kt]
                    for kh in range(3):
                        for kw in range(3):
                            off = kt * 9 + kh * 3 + kw
                            s = 1 + c0 + kh * Wp + kw - 1
                            rhs = rhs_all[:, s:s + n]
                            nc.tensor.matmul(pa[:, :n], sb_win[:, off, 0:C],
                                             rhs, start=(off == 0),
                                             stop=(off == 26))
                for kt in range(3):
                    rhs_all = slabs[kt]
                    for kh in range(3):
                        for kw in range(3):
                            off = kt * 9 + kh * 3 + kw
                            s = 1 + c0 + kh * Wp + kw - 1
                            rhs = rhs_all[:, s:s + n]
                            nc.tensor.matmul(pg[:, :n], sb_win[:, off, C:2 * C],
                                             rhs, start=(off == 0),
                                             stop=(off == 26))
                gate_epilogue(pa, pg, bt, c0, n)
            return bt

        scan_state = [None, None]  # hr, hi tiles [C, NU] f32

        def scan_step(t, bt):
            """LRU step + projection -> y[t] (bf16 slab, data at offset 1)."""
            hr_new = hrpool.tile([C, NU], F32, tag="hr")
            hi_new = hipool.tile([C, NU], F32, tag="hi")
            if t == 0:
                nc.vector.tensor_copy(hr_new[:], bt[:])
                nc.vector.memset(hi_new[:], 0.0)
            else:
                hr_old, hi_old = scan_state
                t1 = tmppool.tile([C, NU], F32, tag="tA")
                nc.vector.scalar_tensor_tensor(t1[:], hi_old[:], c_nli, bt[:],
                                               ALU.mult, ALU.add)
                nc.vector.scalar_tensor_tensor(hr_new[:], hr_old[:], c_lr,
                                               t1[:], ALU.mult, ALU.add)
                t2 = tmppool.tile([C, NU], F32, tag="tB")
                nc.vector.scalar_tensor_tensor(t2[:], hi_old[:], c_lr,
                                               hi_old[:], ALU.mult, ALU.bypass)
                nc.vector.scalar_tensor_tensor(hi_new[:], hr_old[:], c_li,
                                               t2[:], ALU.mult, ALU.add)
            scan_state[0], scan_state[1] = hr_new, hi_new
            t3 = tmppool.tile([C, NU], F32, tag="tA")
            nc.vector.scalar_tensor_tensor(t3[:], hr_new[:], c_gcre,
                                           hr_new[:], ALU.mult, ALU.bypass)
            yt = ypool.tile([C, 1 + NU + 1], BF16, tag="yt")
            nc.vector.memset(yt[:, 0:1], 0.0)
            nc.vector.memset(yt[:, 1 + NU:], 0.0)
            nc.vector.scalar_tensor_tensor(yt[:, 1:1 + NU], hi_new[:], c_gcim,
                                           t3[:], ALU.mult, ALU.add)
            # W wrap columns: col 0 <- col W (w=W-1), col W+1 <- col 1 (w=0)
            yv = yt[:, 1:1 + NU].rearrange("p (r w) -> p r w", w=Wp)
            nc.vector.tensor_copy(yv[:, :, 0:1], yv[:, :, W:W + 1])
            nc.vector.tensor_copy(yv[:, :, W + 1:W + 2], yv[:, :, 1:2])
            # H edge replication (active only on global-edge cores, via mask):
            # row0 <- m0*row0 + (1-m0)*row1 ; last <- m1*last + (1-m1)*prev
            e0 = tmppool.tile([C, Wp], F32, tag="tE")
            nc.vector.scalar_tensor_tensor(e0[:], yv[:, 1, :], c_1m0,
                                           yv[:, 1, :], ALU.mult, ALU.bypass)
            nc.vector.scalar_tensor_tensor(yv[:, 0, :], yv[:, 0, :], c_m0,
                                           e0[:], ALU.mult, ALU.add)
            e1 = tmppool.tile([C, Wp], F32, tag="tE")
            nc.vector.scalar_tensor_tensor(e1[:], yv[:, RU - 2, :], c_1m1,
                                           yv[:, RU - 2, :], ALU.mult, ALU.bypass)
            nc.vector.scalar_tensor_tensor(yv[:, RU - 1, :], yv[:, RU - 1, :],
                                           c_m1, e1[:], ALU.mult, ALU.add)
            y_slabs[t] = yt
            return yt

        def conv_out(t):
            """y[t-1..t+1] -> out[t] = x + conv(y) + b_out."""
            slabs = [y_slabs[min(max(t + kt - 1, 0), T - 1)] for kt in range(3)]
            ot = opool.tile([C, HR, Wp], F32, tag="ot")
            # residual input loaded into the output staging tile
            nc.sync.dma_start(out=ot[:], in_=xh[:, t, 2:2 + HR, :])
            touch(ot[:, 0, 0:1], engines="v")
            of = ot[:].rearrange("p r w -> p (r w)")
            for c0, n in col_tiles(NO):
                po = psO.tile([C, CT], F32, tag="po")
                for kt in range(3):
                    rhs_all = slabs[kt]
                    for kh in range(3):
                        for kw in range(3):
                            off = kt * 9 + kh * 3 + kw
                            s = 1 + c0 + kh * Wp + kw - 1
                            rhs = rhs_all[:, s:s + n]
                            nc.tensor.matmul(po[:, :n], sb_wout[:, off, :],
                                             rhs, start=(off == 0),
                                             stop=(off == 26))
                nc.vector.scalar_tensor_tensor(of[:, c0:c0 + n], po[:, :n],
                                               c_bout, of[:, c0:c0 + n],
                                               ALU.add, ALU.add)
            nc.sync.dma_start(out=out[:, t], in_=ot[:, :, 1:1 + W])

        def conv_out_packed(t):
            """conv_out with array packing: pairs (e,o) put e's 96 channels +
            o's first 32 (pos 96) in one pass; the two pairs' leftover 64
            channels share one concurrent col-tiled pass. Misplaced pieces are
            realigned into the staging tile via SBUF->SBUF DMA."""
            slabs = [y_slabs[min(max(t + kt - 1, 0), T - 1)] for kt in range(3)]
            ot = opool.tile([C, HR, Wp], F32, tag="ot")
            nc.sync.dma_start(out=ot[:], in_=xh[:, t, 2:2 + HR, :])
            touch(ot[:, 0, 0:1], engines="v")
            of = ot[:].rearrange("p r w -> p (r w)")
            xflat = xh[:, t, 2:2 + HR, :].rearrange("p r w -> p (r w)")
            cts = col_tiles(NO)

            def mm_group(ps, prange, wslice, c0, n, pos):
                for kt in range(3):
                    rhs_all = slabs[kt]
                    for kh in range(3):
                        for kw in range(3):
                            off = kt * 9 + kh * 3 + kw
                            s = 1 + c0 + kh * Wp + kw - 1
                            nc.tensor.matmul(
                                ps[prange[0]:prange[1], :n],
                                sb_wout[:, off, wslice[0]:wslice[1]],
                                rhs_all[:, s:s + n],
                                start=(off == 0), stop=(off == 26),
                                tile_position=pos, skip_group_check=True)

            def mm_group2(ps, jobs):
                """Interleaved concurrent accumulation groups."""
                for kt in range(3):
                    rhs_all = slabs[kt]
                    for kh in range(3):
                        for kw in range(3):
                            off = kt * 9 + kh * 3 + kw
                            for prange, wslice, c0, n, pos in jobs:
                                s = 1 + c0 + kh * Wp + kw - 1
                                nc.tensor.matmul(
                                    ps[prange[0]:prange[1], :n],
                                    sb_wout[:, off, wslice[0]:wslice[1]],
                                    rhs_all[:, s:s + n],
                                    start=(off == 0), stop=(off == 26),
                                    tile_position=pos, skip_group_check=True)

            def aligned_epi(ps, c0, n):
                nc.vector.scalar_tensor_tensor(of[:, c0:c0 + n], ps[:C, :n],
                                               c_bout, of[:, c0:c0 + n],
                                               ALU.add, ALU.add)

            def piece_epi(ps, pbase, psize, ch0, c0, n):
                """Residual-add for a channel piece at partitions
                [pbase, pbase+psize) holding channels [ch0, ch0+psize);
                realign into ot via DMA."""
                xp = piecepool.tile([128, CT], F32, tag="xp")
                nc.sync.dma_start(out=xp[pbase:pbase + psize, :n],
                                  in_=xflat[ch0:ch0 + psize, c0:c0 + n])
                aux = c_aux0 if pbase == 96 else c_aux1
                pt = piecepool.tile([128, CT], F32, tag="pc")
                nc.vector.scalar_tensor_tensor(
                    pt[pbase:pbase + psize, :n],
                    ps[pbase:pbase + psize, :n],
                    aux[pbase:pbase + psize, :],
                    xp[pbase:pbase + psize, :n], ALU.add, ALU.add)
                nc.sync.dma_start(out=of[ch0:ch0 + psize, c0:c0 + n],
                                  in_=pt[pbase:pbase + psize, :n])

            for e, o in ((0, 1), (2, 3)):
                (ce, ne), (co_, no_) = cts[e], cts[o]
                p1 = psO.tile([128, CT], F32, tag="po")
                mm_group2(p1, [((0, 32), (0, 32), ce, ne, (0, 0)),
                               ((32, 64), (32, 64), ce, ne, (0, 32)),
                               ((64, 96), (64, 96), ce, ne, (0, 64)),
                               ((96, 128), (0, 32), co_, no_, (0, 96))])
                aligned_epi(p1, ce, ne)
                piece_epi(p1, 96, 32, 0, co_, no_)
            p2 = psG.tile([128, CT], F32, tag="pg")
            mm_group2(p2, [((0, 32), (32, 64), cts[1][0], cts[1][1], (0, 0)),
                           ((32, 64), (64, 96), cts[1][0], cts[1][1], (0, 32)),
                           ((64, 96), (32, 64), cts[3][0], cts[3][1], (0, 64)),
                           ((96, 128), (64, 96), cts[3][0], cts[3][1], (0, 96))])
            piece_epi(p2, 0, 64, 32, cts[1][0], cts[1][1])
            piece_epi(p2, 64, 64, 32, cts[3][0], cts[3][1])
            p4 = psO.tile([128, CT], F32, tag="po")
            mm_group(p4, (0, 96), (0, 96), cts[4][0], cts[4][1], (0, 0))
            aligned_epi(p4, cts[4][0], cts[4][1])
            nc.sync.dma_start(out=out[:, t], in_=ot[:, :, 1:1 + W])

        octs = col_tiles(NO)
        use_p2 = pack2 and len(octs) == 5 and all(n == CT for _, n in octs[:4])
        co_fn = conv_out_packed if use_p2 else conv_out

        stage_a(0)
        if T > 1:
            stage_a(1)
        for t in range(T):
            if t + 1 < T:
                stage_a(t + 1)
            bt = conv_in_packed(t) if pack else conv_in(t)
            scan_step(t, bt)
            if t >= 1:
                co_fn(t - 1)
        co_fn(T - 1)

    nc.compile()
    return nc


def build_program_k(C=96, T=16, HR=16, W=128, CT=512, cout_kp=False):
    """K=128-packed SPMD program.

    Each conv's 27-tap x 96-ch contraction (2592 rows) is regrouped into 21
    matmul streams per output tile instead of 27:
      - 9 A-streams: [slab_prev ch0..95 ; slab_cur ch0..31] (dup rows DMA'd
        into partitions 96..127 of the prev slab tile), one per (kh,kw).
      - 9 B-streams: Q-tile = [slab_cur ch32..95 ; slab_next ch0..63].
      - 3 R-streams: per-kh slabs pack slab_next ch64..95 at the 3 kw
        column shifts (K=96).
    t=0 uses special A-weights (kt0+kt1 folded for ch0..31, K=96) because
    the clamped prev slab's dup rows hold the wrong timestep.

    Pipeline: normalization (stage) runs 3 timesteps ahead, slab-combining
    DMAs (build_in) 1 ahead with a full iteration of slack; x**2 runs on
    DVE and rms uses Sqrt(ACT)+reciprocal(DVE) so the ACT engine only ever
    holds the Sqrt+Silu tables (no table thrash) and the stage matmuls
    never block the PE FIFO.
    """
    Wp = W + 2
    RIN = HR + 4
    RU = HR + 2
    NIN = RIN * Wp
    NU = RU * Wp
    NO = HR * Wp
    C2 = 2 * C

    nc = bacc.Bacc()
    xh = nc.declare_dram_parameter("xh", [C, T, RIN, Wp], F32, isOutput=False)
    wA = nc.declare_dram_parameter("wA", [128, 9, C2], BF16, isOutput=False)
    wB = nc.declare_dram_parameter("wB", [128, 9, C2], BF16, isOutput=False)
    wR = nc.declare_dram_parameter("wR", [96, 3, C2], BF16, isOutput=False)
    wA0 = nc.declare_dram_parameter("wA0", [96, 9, C2], BF16, isOutput=False)
    if cout_kp:
        wKA = nc.declare_dram_parameter("wKA", [128, 9, C], BF16, isOutput=False)
        wKB = nc.declare_dram_parameter("wKB", [128, 9, C], BF16, isOutput=False)
        wKR = nc.declare_dram_parameter("wKR", [96, 3, C], BF16, isOutput=False)
        wKA0 = nc.declare_dram_parameter("wKA0", [96, 9, C], BF16, isOutput=False)
    else:
        wK = nc.declare_dram_parameter("wK", [C, 27, C], BF16, isOutput=False)
    onesw = nc.declare_dram_parameter("onesw", [C, 128], BF16, isOutput=False)
    consts = nc.declare_dram_parameter("consts", [C, 13], F32, isOutput=False)
    aux = nc.declare_dram_parameter("aux128", [128, 8], F32, isOutput=False)
    out = nc.declare_dram_parameter("out", [C, T, HR, W], F32, isOutput=True)

    def col_tiles(total):
        return [(i, min(CT, total - i)) for i in range(0, total, CT)]

    with tile.TileContext(nc) as tc, ExitStack() as ctx:
        singles = ctx.enter_context(tc.tile_pool(name="singles", bufs=1))
        xpool = ctx.enter_context(tc.tile_pool(name="xpool", bufs=2))
        sqpool = ctx.enter_context(tc.tile_pool(name="sqpool", bufs=1))
        statpool = ctx.enter_context(tc.tile_pool(name="statpool", bufs=1))
        hnpool = ctx.enter_context(tc.tile_pool(name="hnpool", bufs=5))
        qinpool = ctx.enter_context(tc.tile_pool(name="qinpool", bufs=2))
        rinpool = ctx.enter_context(tc.tile_pool(name="rinpool", bufs=2))
        sapool = ctx.enter_context(tc.tile_pool(name="sapool", bufs=2))
        bpool = ctx.enter_context(tc.tile_pool(name="bpool", bufs=2))
        tmppool = ctx.enter_context(tc.tile_pool(name="tmppool", bufs=1))
        ypool = ctx.enter_context(tc.tile_pool(name="ypool", bufs=4))
        opool = ctx.enter_context(tc.tile_pool(name="opool", bufs=1))
        if cout_kp:
            qopool = ctx.enter_context(tc.tile_pool(name="qopool", bufs=2))
            rkpool = ctx.enter_context(tc.tile_pool(name="rkpool", bufs=2))
            xspool = ctx.enter_context(tc.tile_pool(name="xspool", bufs=3))
        gspool = ctx.enter_context(tc.tile_pool(name="gspool", bufs=1))
        gfpool = ctx.enter_context(tc.tile_pool(name="gfpool", bufs=2))
        touchpool = ctx.enter_context(tc.tile_pool(name="touchpool", bufs=2))
        psN = ctx.enter_context(tc.tile_pool(name="psN", bufs=1, space="PSUM"))
        psA = ctx.enter_context(tc.tile_pool(
            name="psA", bufs=3 if cout_kp else 2, space="PSUM"))
        psG = ctx.enter_context(tc.tile_pool(name="psG", bufs=2, space="PSUM"))
        psO = ctx.enter_context(tc.tile_pool(
            name="psO", bufs=2 if cout_kp else 3, space="PSUM"))

        sb_c = singles.tile([C, 13], F32)
        nc.sync.dma_start(out=sb_c[:], in_=consts[:])
        sb_aux = singles.tile([128, 8], F32)
        nc.sync.dma_start(out=sb_aux[:], in_=aux[:])
        sb_ones = singles.tile([C, 128], BF16)
        nc.sync.dma_start(out=sb_ones[:], in_=onesw[:])
        sb_wA = singles.tile([128, 9, C2], BF16)
        sb_wB = singles.tile([128, 9, C2], BF16)
        sb_wR = singles.tile([96, 3, C2], BF16)
        sb_wA0 = singles.tile([96, 9, C2], BF16)
        if cout_kp:
            sb_wKA = singles.tile([128, 9, C], BF16)
            sb_wKB = singles.tile([128, 9, C], BF16)
            sb_wKR = singles.tile([96, 3, C], BF16)
            sb_wKA0 = singles.tile([96, 9, C], BF16)
        else:
            sb_wK = singles.tile([C, 27, C], BF16)

        def emit_weight_dmas():
            nc.sync.dma_start(out=sb_wA0[:], in_=wA0[:])
            nc.sync.dma_start(out=sb_wA[:], in_=wA[:])
            nc.sync.dma_start(out=sb_wB[:], in_=wB[:])
            nc.sync.dma_start(out=sb_wR[:], in_=wR[:])
            if cout_kp:
                nc.sync.dma_start(out=sb_wKA[:], in_=wKA[:])
                nc.sync.dma_start(out=sb_wKB[:], in_=wKB[:])
                nc.sync.dma_start(out=sb_wKR[:], in_=wKR[:])
                nc.sync.dma_start(out=sb_wKA0[:], in_=wKA0[:])
            else:
                nc.sync.dma_start(out=sb_wK[:], in_=wK[:])

        c_ba = sb_c[:, 0:1]
        c_bg = sb_c[:, 1:2]
        c_lr = sb_c[:, 2:3]
        c_li = sb_c[:, 3:4]
        c_nli = sb_c[:, 4:5]
        c_gcre = sb_c[:, 5:6]
        c_gcim = sb_c[:, 6:7]
        c_bout = sb_c[:, 7:8]
        c_m0 = sb_c[:, 8:9]
        c_1m0 = sb_c[:, 9:10]
        c_m1 = sb_c[:, 10:11]
        c_1m1 = sb_c[:, 11:12]
        c_eps = sb_c[:, 12:13]
        c_ba_hi = sb_aux[:, 0:1]   # [96:128] = ba[0:32]
        c_ba_pg = sb_aux[:, 1:2]   # [0:64] and [64:128] = ba[32:96]

        # Warm-ups: observe const DMA on each engine and preload the only
        # two ACT tables used in steady state (natural_log_exp, Silu).
        wu_v = singles.tile([C, 13], F32)
        nc.vector.tensor_copy(wu_v[:], sb_c[:])
        wu_v2 = singles.tile([128, 8], F32)
        nc.vector.tensor_copy(wu_v2[:], sb_aux[:])
        wu_s = singles.tile([C, 13], F32)
        nc.scalar.activation(wu_s[:], sb_c[:], AF.Exp)
        wu_s2 = singles.tile([128, 8], BF16)
        nc.scalar.activation(wu_s2[:], sb_aux[:], AF.Silu, bias=c_ba_hi)

        def touch(ap, engines="v"):
            if "v" in engines:
                tv = touchpool.tile([C, 1], F32, tag="tv")
                nc.vector.tensor_copy(tv[:], ap)

        ag_swap = os.environ.get("KERNEL_AGSWAP", "1") == "1"

        hn_slabs = [None] * T   # [128, 1+NIN+1] bf16; rows 96:128 = dup
        sq_tiles = [None] * T
        qin_tiles = [None] * T
        rin_tiles = [None] * T
        y_slabs = [None] * T    # [128, 1+NU+1] bf16

        def stage_sq(t):
            """x[t] load + x**2 on ACT in one op (Square lives in every table
            set: no table loads, and one op gives the scheduler no seams)."""
            xt = xpool.tile([C, RIN, Wp], F32, tag="xt")
            nc.sync.dma_start(out=xt[:], in_=xh[:, t])
            xf = xt[:].rearrange("p r w -> p (r w)")
            sq = sqpool.tile([C, NIN], BF16, tag="sq")
            nc.scalar.activation(sq[:, :], xf[:, :], AF.Square)
            sq_tiles[t] = (xt, sq)

        def stage_rest(t):
            """rms reduce (PE) + table-cheap Copy drains + ONE Ln + ONE Exp.

            Ln and Exp live in different activation-table home sets; the tile
            scheduler also splices gate Silus between ACT ops, so any
            multi-op Ln/Exp sequence thrashes table loads (~2.7us each).
            Draining psum via Copy (present in every set) and doing a single
            full-width Ln then Exp caps the damage at ~3 loads per call."""
            xt, sq = sq_tiles[t]
            xf = xt[:].rearrange("p r w -> p (r w)")
            hn = hnpool.tile([128, 1 + NIN + 1], BF16, tag="hn")
            nc.vector.memset(hn[0:C, 0:1], 0.0)
            nc.vector.memset(hn[0:C, 1 + NIN:], 0.0)
            for c0, n in col_tiles(NIN):
                ps = psN.tile([128, CT], F32, tag="psn")
                nc.tensor.matmul(ps[:, :n], sb_ones[:], sq[:, c0:c0 + n],
                                 start=True, stop=True)
                nc.scalar.copy(sq[:, c0:c0 + n], ps[:C, :n])
            inv = statpool.tile([C, NIN], BF16, tag="inv")
            nc.scalar.activation(inv[:, :], sq[:, :], AF.Ln,
                                 scale=1.0 / C, bias=c_eps)
            nc.scalar.activation(sq[:, :], inv[:, :], AF.Exp, scale=-0.5)
            for c0, n in col_tiles(NIN):
                nc.vector.tensor_mul(hn[0:C, 1 + c0:1 + c0 + n],
                                     xf[:, c0:c0 + n], sq[:, c0:c0 + n])
            hn_slabs[t] = hn
            return hn

        def build_in(t):
            """Slab-combining DMAs for conv_in(t) (+dup used by t+1).
            Needs hn[t] and hn[min(t+1, T-1)] already emitted."""
            cur = hn_slabs[t]
            nxt = hn_slabs[min(t + 1, T - 1)]
            if t + 1 < T:
                nc.gpsimd.dma_start(out=cur[96:128, :], in_=nxt[0:32, :])
            qi = qinpool.tile([128, 1 + NIN + 1], BF16, tag="qi")
            nc.gpsimd.dma_start(out=qi[0:64, :], in_=cur[32:96, :])
            nc.gpsimd.dma_start(out=qi[64:128, :], in_=nxt[0:64, :])
            qin_tiles[t] = qi
            rs = []
            for kh in range(3):
                r_ = rinpool.tile([96, NU], BF16, tag=f"r{kh}")
                for kw in range(3):
                    d = kh * Wp + kw
                    nc.gpsimd.dma_start(out=r_[32 * kw:32 * kw + 32, :],
                                        in_=nxt[64:96, d:d + NU])
                rs.append(r_)
            rin_tiles[t] = rs

        def conv_in_k(t, bt_arg, pair_range):
            a_sl = hn_slabs[max(t - 1, 0)]
            wa_sb = sb_wA0 if t == 0 else sb_wA
            ka = 96 if t == 0 else 128
            q = qin_tiles[t]
            rr = rin_tiles[t]

            streams = []
            for j in range(9):
                kh, kw = divmod(j, 3)
                s = kh * Wp + kw
                streams.append((
                    lambda m0, m1, jj=j: wa_sb[0:ka, jj, m0:m1],
                    lambda c0, n, ss=s: a_sl[0:ka, ss + c0:ss + c0 + n]))
            for j in range(9):
                kh, kw = divmod(j, 3)
                s = kh * Wp + kw
                streams.append((
                    lambda m0, m1, jj=j: sb_wB[:, jj, m0:m1],
                    lambda c0, n, ss=s: q[:, ss + c0:ss + c0 + n]))
            for kh in range(3):
                streams.append((
                    lambda m0, m1, kk=kh: sb_wR[:, kk, m0:m1],
                    lambda c0, n, kk=kh: rr[kk][:, c0:c0 + n]))
            NS = len(streams)

            bt = bt_arg
            cts = col_tiles(NU)
            for p0 in pair_range:
                pair = cts[p0:p0 + 2]
                pas = []
                for c0, n in pair:
                    pa = psA.tile([128, CT], F32, tag="pa")
                    for i, (lw, rh) in enumerate(streams):
                        nc.tensor.matmul(pa[:, :n], lw(0, 128), rh(c0, n),
                                         start=(i == 0), stop=(i == NS - 1))
                    pas.append(pa)
                pg = psG.tile([128, CT], F32, tag="pg")
                for i, (lw, rh) in enumerate(streams):
                    for j, (c0, n) in enumerate(pair):
                        b = 64 * j
                        nc.tensor.matmul(
                            pg[b:b + 64, :n], lw(128, 192), rh(c0, n),
                            start=(i == 0), stop=(i == NS - 1),
                            tile_position=(0, b), skip_group_check=True)
                for j, (c0, n) in enumerate(pair):
                    b = 64 * j
                    pa = pas[j]
                    if ag_swap:
                        # Output channels are permuted so g (96) sits aligned
                        # at psum partitions 0:96 while a rides the spare
                        # slots: a[0:32] at pa[96:128], a[32:96] at pg[b:b+64].
                        # Silu runs on the pieces in place (ACT reads PSUM), a
                        # DMA realigns the bf16 silu outputs, and the final STT
                        # reads g straight from PSUM — no DVE casts.
                        sa = sapool.tile([128, CT], BF16, tag="sa")
                        nc.scalar.activation(sa[96:128, :n], pa[96:128, :n],
                                             AF.Silu, bias=c_ba_hi[96:128])
                        sb_ = sapool.tile([128, CT], BF16, tag="sb")
                        nc.scalar.activation(sb_[b:b + 64, :n],
                                             pg[b:b + 64, :n],
                                             AF.Silu, bias=c_ba_pg[b:b + 64])
                        gf = gfpool.tile([C, CT], BF16, tag="gf")
                        nc.sync.dma_start(out=gf[0:32, :n], in_=sa[96:128, :n])
                        nc.sync.dma_start(out=gf[32:96, :n],
                                          in_=sb_[b:b + 64, :n])
                        nc.vector.scalar_tensor_tensor(bt[:, c0:c0 + n],
                                                       pa[:C, :n], c_bg,
                                                       gf[:, :n],
                                                       ALU.add, ALU.mult)
                    else:
                        gsa = gspool.tile([128, CT], BF16, tag="gsa")
                        nc.vector.tensor_copy(gsa[96:128, :n], pa[96:128, :n])
                        gsb = gspool.tile([128, CT], BF16, tag="gsb")
                        nc.vector.tensor_copy(gsb[b:b + 64, :n],
                                              pg[b:b + 64, :n])
                        gf = gfpool.tile([C, CT], BF16, tag="gf")
                        nc.sync.dma_start(out=gf[0:32, :n], in_=gsa[96:128, :n])
                        nc.sync.dma_start(out=gf[32:96, :n],
                                          in_=gsb[b:b + 64, :n])
                        sa = sapool.tile([C, CT], BF16, tag="sa")
                        nc.scalar.activation(sa[:, :n], pa[:C, :n], AF.Silu,
                                             bias=c_ba)
                        nc.vector.scalar_tensor_tensor(bt[:, c0:c0 + n],
                                                       gf[:, :n], c_bg,
                                                       sa[:, :n],
                                                       ALU.add, ALU.mult)

        hr = singles.tile([C, NU], F32)
        hi = singles.tile([C, NU], F32)

        NH = (RU // 2) * Wp  # first-half columns (rows 0..RU/2-1)

        def scan_half(t, bt, yt, h0, h1):
            hrh = hr[:, h0:h1]
            hih = hi[:, h0:h1]
            bth = bt[:, h0:h1]
            if t == 0:
                nc.vector.tensor_copy(hrh, bth)
                nc.vector.memset(hih, 0.0)
            else:
                t1 = tmppool.tile([C, NH], F32, tag="tA")
                nc.vector.scalar_tensor_tensor(t1[:, :h1 - h0], hih, c_nli,
                                               bth, ALU.mult, ALU.add)
                nc.vector.scalar_tensor_tensor(hih, hih, c_lr, hih,
                                               ALU.mult, ALU.bypass)
                nc.vector.scalar_tensor_tensor(hih, hrh, c_li, hih,
                                               ALU.mult, ALU.add)
                nc.vector.scalar_tensor_tensor(hrh, hrh, c_lr,
                                               t1[:, :h1 - h0],
                                               ALU.mult, ALU.add)
            t3 = tmppool.tile([C, NH], F32, tag="tA")
            nc.vector.scalar_tensor_tensor(t3[:, :h1 - h0], hrh, c_gcre,
                                           hrh, ALU.mult, ALU.bypass)
            nc.vector.scalar_tensor_tensor(yt[0:C, 1 + h0:1 + h1], hih,
                                           c_gcim, t3[:, :h1 - h0],
                                           ALU.mult, ALU.add)
            yv = yt[0:C, 1 + h0:1 + h1].rearrange("p (r w) -> p r w", w=Wp)
            nr = (h1 - h0) // Wp
            nc.vector.tensor_copy(yv[:, :, 0:1], yv[:, :, W:W + 1])
            nc.vector.tensor_copy(yv[:, :, W + 1:W + 2], yv[:, :, 1:2])
            if h0 == 0:
                e0 = tmppool.tile([C, Wp], F32, tag="tE")
                nc.vector.scalar_tensor_tensor(e0[:], yv[:, 1, :], c_1m0,
                                               yv[:, 1, :], ALU.mult,
                                               ALU.bypass)
                nc.vector.scalar_tensor_tensor(yv[:, 0, :], yv[:, 0, :],
                                               c_m0, e0[:], ALU.mult,
                                               ALU.add)
            else:
                e1 = tmppool.tile([C, Wp], F32, tag="tE")
                nc.vector.scalar_tensor_tensor(e1[:], yv[:, nr - 2, :],
                                               c_1m1, yv[:, nr - 2, :],
                                               ALU.mult, ALU.bypass)
                nc.vector.scalar_tensor_tensor(yv[:, nr - 1, :],
                                               yv[:, nr - 1, :], c_m1,
                                               e1[:], ALU.mult, ALU.add)

        def conv_out_k(t):
            """Direct 27-tap conv_out: kt-ordered so the y[t]-dependent
            taps (kt2) come last in each accumulation group."""
            slabs = [y_slabs[min(max(t + kt - 1, 0), T - 1)] for kt in range(3)]
            ot = opool.tile([C, HR, Wp], F32, tag="ot")
            nc.sync.dma_start(out=ot[:], in_=xh[:, t, 2:2 + HR, :])
            touch(ot[:, 0, 0:1], engines="v")
            of = ot[:].rearrange("p r w -> p (r w)")
            for c0, n in col_tiles(NO):
                po = psO.tile([C, CT], F32, tag="po", name="po")
                for kt in range(3):
                    rhs_all = slabs[kt]
                    for kh in range(3):
                        for kw in range(3):
                            off = kt * 9 + kh * 3 + kw
                            s = 1 + c0 + kh * Wp + kw - 1
                            nc.tensor.matmul(po[:, :n], sb_wK[:, off, :],
                                             rhs_all[0:C, s:s + n],
                                             start=(off == 0),
                                             stop=(off == 26))
                nc.vector.scalar_tensor_tensor(of[:, c0:c0 + n], po[:, :n],
                                               c_bout, of[:, c0:c0 + n],
                                               ALU.add, ALU.add)
            nc.sync.dma_start(out=out[:, t], in_=ot[:, :, 1:1 + W])

        # ---- K-packed conv_out --------------------------------------------
        # Same 21-stream K-regrouping as conv_in (A: y[s-1]96 + y[s]0:32 via
        # dup rows; Q: y[s]32:96 + y[s+1]0:64 materialized; R: y[s+1]64:96 at
        # 9 kw shifts). M stays 96-wide and aligned (measured: col-tiled
        # M-packing costs ~330ns/group-step vs 260 for a plain pass, so it
        # saves nothing and complicates the epilogue). Outputs use exact
        # 4-row x 128-col tiles via 3D rhs APs (no wrap-col compute).
        # Queue split: combines on gpsimd, epilogue x/out DMAs on scalar,
        # conv_in's gate realigns keep sync — so none of them FIFO-couple.
        qro = {}          # s -> (qo tile, [r tiles kh=0..2])
        RKW = HR * Wp     # R-tile width (only (HR-1)*Wp + W + ... used)

        def build_out(t):
            """After scan(t): dup rows for y[t-1]; qo/R for conv_out(t-1)."""
            if t == 0:
                return
            s = t - 1
            cur, nxt = y_slabs[s], y_slabs[t]
            nc.gpsimd.dma_start(out=cur[96:128, :], in_=nxt[0:32, :])
            _build_qr(s, cur, nxt)

        def _build_qr(s, cur, nxt):
            qo = qopool.tile([128, 1 + NU + 1], BF16, tag="qo")
            nc.gpsimd.dma_start(out=qo[0:64, :], in_=cur[32:96, :])
            nc.gpsimd.dma_start(out=qo[64:128, :], in_=nxt[0:64, :])
            rs = []
            for kh in range(3):
                r_ = rkpool.tile([96, RKW], BF16, tag=f"rk{kh}")
                for kw in range(3):
                    d = kh * Wp + kw
                    nw = (HR - 1) * Wp + W + 2 - kw
                    nc.gpsimd.dma_start(out=r_[32 * kw:32 * kw + 32, 0:nw],
                                        in_=nxt[64:96, 1 + d:1 + d + nw])
                rs.append(r_)
            qro[s] = (qo, rs)

        def conv_out_kd(s):
            """21 K-streams x 4 col tiles, M=96 aligned; writes out[:, s]."""
            prev = y_slabs[max(s - 1, 0)]
            wa_sb = sb_wKA0 if s == 0 else sb_wKA
            ka = 96 if s == 0 else 128
            qo, rs = qro[s]
            pv = prev[:, 1:1 + NU].rearrange("p (r w) -> p r w", w=Wp)
            qv = qo[:, 1:1 + NU].rearrange("p (r w) -> p r w", w=Wp)
            rv = [r_[:].rearrange("p (r w) -> p r w", w=Wp) for r_ in rs]

            streams = []
            for j in range(9):
                kh, kw = divmod(j, 3)
                streams.append((
                    lambda jj=j: wa_sb[0:ka, jj, :],
                    lambda r0, kh=kh, kw=kw:
                        pv[0:ka, r0 + kh:r0 + kh + 4, kw:kw + 128]))
            for j in range(9):
                kh, kw = divmod(j, 3)
                streams.append((
                    lambda jj=j: sb_wKB[:, jj, :],
                    lambda r0, kh=kh, kw=kw:
                        qv[:, r0 + kh:r0 + kh + 4, kw:kw + 128]))
            for kh in range(3):
                streams.append((
                    lambda kk=kh: sb_wKR[:, kk, :],
                    lambda r0, kk=kh: rv[kk][:, r0:r0 + 4, 0:128]))
            NS = len(streams)

            for r0 in (0, 4, 8, 12):
                xs = xspool.tile([128, CT], F32, tag="xs")
                nc.scalar.dma_start(
                    out=xs[0:96, :],
                    in_=xh[:, s, 2 + r0:2 + r0 + 4, 1:1 + W])
                tv = touchpool.tile([128, 1], F32, tag="tvp")
                nc.vector.tensor_copy(tv[0:96, :], xs[0:96, 0:1])
                po = psO.tile([128, CT], F32, tag="po")
                for i, (lw, rh) in enumerate(streams):
                    nc.tensor.matmul(po[0:96, :], lw(), rh(r0),
                                     start=(i == 0), stop=(i == NS - 1))
                nc.vector.scalar_tensor_tensor(xs[0:96, :], po[0:96, :],
                                               c_bout, xs[0:96, :],
                                               ALU.add, ALU.add)
                nc.scalar.dma_start(out=out[:, s, r0:r0 + 4, :],
                                    in_=xs[0:96, :])

        for u in range(min(3, T)):
            stage_sq(u)
            stage_rest(u)
        emit_weight_dmas()
        build_in(0)
        for t in range(T):
            if t + 1 < T:
                build_in(t + 1)
            if t + 3 < T:
                stage_sq(t + 3)
            bt = bpool.tile([C, NU], BF16, tag="bt")
            yt = ypool.tile([128, 1 + NU + 1], BF16, tag="yt")
            nc.vector.memset(yt[0:C, 0:1], 0.0)
            nc.vector.memset(yt[0:C, 1 + NU:], 0.0)
            conv_in_k(t, bt, [0, 2])
            scan_half(t, bt, yt, 0, NH)
            conv_in_k(t, bt, [4])
            scan_half(t, bt, yt, NH, NU)
            y_slabs[t] = yt
            if cout_kp:
                build_out(t)
                if t >= 2:
                    conv_out_kd(t - 2)
            elif t >= 1:
                conv_out_k(t - 1)
            if t + 3 < T:
                stage_rest(t + 3)
        if cout_kp:
            _build_qr(T - 1, y_slabs[T - 1], y_slabs[T - 1])
            conv_out_kd(T - 2)
            conv_out_kd(T - 1)
        else:
            conv_out_k(T - 1)

    nc.compile()
    return nc



def prep_core_inputs(x, norm_w, conv_in_w, conv_in_b, nu_log, theta_log,
                     c_re, c_im, conv_out_w, conv_out_b, n_qh):
    """Build per-core input maps. Cores = batch-major, then H quarters."""
    B, C, T, H, W = x.shape
    HR = H // n_qh

    nu = np.exp(np.asarray(nu_log, np.float64))
    theta = np.exp(np.asarray(theta_log, np.float64))
    lam_re = (np.exp(-nu) * np.cos(theta)).astype(np.float32)
    lam_im = (np.exp(-nu) * np.sin(theta)).astype(np.float32)
    gamma = np.sqrt(1.0 - np.exp(-2.0 * nu))
    gcre = (gamma * np.asarray(c_re, np.float64)).astype(np.float32)
    gcim = (gamma * np.asarray(c_im, np.float64)).astype(np.float32)

    w_in_f = np.asarray(conv_in_w, np.float32) * \
        np.asarray(norm_w, np.float32)[None, :, None, None, None]
    w_in_t = np.ascontiguousarray(
        np.transpose(w_in_f, (1, 2, 3, 4, 0)).reshape(C, 27, 2 * C)
    ).astype(ml_dtypes.bfloat16)
    w_out_t = np.ascontiguousarray(
        np.transpose(np.asarray(conv_out_w, np.float32),
                     (1, 2, 3, 4, 0)).reshape(C, 27, C)
    ).astype(ml_dtypes.bfloat16)
    ones = np.ones((C, 128), ml_dtypes.bfloat16)

    xp = np.concatenate([x[..., -1:], x, x[..., :1]], axis=-1)  # W circular

    in_maps = []
    for b in range(B):
        for q in range(n_qh):
            rows = np.clip(np.arange(q * HR - 2, q * HR + HR + 2), 0, H - 1)
            xh = np.ascontiguousarray(xp[b][:, :, rows, :]).astype(np.float32)
            m0 = 0.0 if q == 0 else 1.0
            m1 = 0.0 if q == n_qh - 1 else 1.0
            cvec = np.stack([
                np.asarray(conv_in_b, np.float32)[:C],
                np.asarray(conv_in_b, np.float32)[C:],
                lam_re, lam_im, -lam_im, gcre, gcim,
                np.asarray(conv_out_b, np.float32),
                np.full(C, m0, np.float32), np.full(C, 1.0 - m0, np.float32),
                np.full(C, m1, np.float32), np.full(C, 1.0 - m1, np.float32),
                np.full(C, EPS, np.float32),
            ], axis=1)
            bo = np.asarray(conv_out_b, np.float32)
            aux = np.zeros((128, 2), np.float32)
            aux[96:128, 0] = bo[0:32]
            aux[:, 1] = bo[32 + (np.arange(128) % 64)]
            in_maps.append({
                "xh": xh,
                "w_in": w_in_t,
                "w_out": w_out_t,
                "onesw": ones,
                "consts": np.ascontiguousarray(cvec),
                "consts2": aux,
            })
    return in_maps


def prep_core_inputs_k(x, norm_w, conv_in_w, conv_in_b, nu_log, theta_log,
                       c_re, c_im, conv_out_w, conv_out_b, n_qh):
    """Per-core inputs for the K=128-packed program."""
    B, C, T, H, W = x.shape
    HR = H // n_qh
    C2 = 2 * C

    nu = np.exp(np.asarray(nu_log, np.float64))
    theta = np.exp(np.asarray(theta_log, np.float64))
    lam_re = (np.exp(-nu) * np.cos(theta)).astype(np.float32)
    lam_im = (np.exp(-nu) * np.sin(theta)).astype(np.float32)
    gamma = np.sqrt(1.0 - np.exp(-2.0 * nu))
    gcre = (gamma * np.asarray(c_re, np.float64)).astype(np.float32)
    gcim = (gamma * np.asarray(c_im, np.float64)).astype(np.float32)

    w_in_f = np.asarray(conv_in_w, np.float32) * \
        np.asarray(norm_w, np.float32)[None, :, None, None, None]
    # wt[cin, kt, kh, kw, cout]; cout permuted so g-channels (96:192) land
    # first (psum-aligned) and a-channels ride the spare packing slots.
    wt = np.transpose(w_in_f, (1, 2, 3, 4, 0))
    if os.environ.get("KERNEL_AGSWAP", "1") == "1":
        perm = np.concatenate([np.arange(C, 2 * C), np.arange(0, C)])
        wt = np.ascontiguousarray(wt[..., perm])
    wto = np.transpose(np.asarray(conv_out_w, np.float32), (1, 2, 3, 4, 0))

    def pack(w, co):
        """w: [cin, kt, kh, kw, co] -> (wA, wB, wR, wR3, wA0)."""
        wA = np.zeros((128, 9, co), np.float32)
        wB = np.zeros((128, 9, co), np.float32)
        wA0 = np.zeros((96, 9, co), np.float32)
        for j in range(9):
            kh, kw = divmod(j, 3)
            wA[0:96, j] = w[:, 0, kh, kw]
            wA[96:128, j] = w[0:32, 1, kh, kw]
            wB[0:64, j] = w[32:96, 1, kh, kw]
            wB[64:128, j] = w[0:64, 2, kh, kw]
            wA0[0:32, j] = w[0:32, 0, kh, kw] + w[0:32, 1, kh, kw]
            wA0[32:96, j] = w[32:96, 0, kh, kw]
        wR = np.zeros((96, 3, co), np.float32)
        for kh in range(3):
            for kw in range(3):
                wR[32 * kw:32 * (kw + 1), kh] = w[64:96, 2, kh, kw]
        bf = ml_dtypes.bfloat16
        return (np.ascontiguousarray(wA).astype(bf),
                np.ascontiguousarray(wB).astype(bf),
                np.ascontiguousarray(wR).astype(bf),
                np.ascontiguousarray(wA0).astype(bf))

    wA, wB, wR, wA0 = pack(wt, C2)
    if os.environ.get("KERNEL_COUTKP", "1") == "1":
        wKA, wKB, wKR, wKA0 = pack(wto, C)
        wext = {"wKA": wKA, "wKB": wKB, "wKR": wKR, "wKA0": wKA0}
    else:
        wext = {"wK": np.ascontiguousarray(
            wto.reshape(C, 27, C)).astype(ml_dtypes.bfloat16)}
    ones = np.ones((C, 128), ml_dtypes.bfloat16)

    xp = np.concatenate([x[..., -1:], x, x[..., :1]], axis=-1)  # W circular

    in_maps = []
    for b in range(B):
        for q in range(n_qh):
            rows = np.clip(np.arange(q * HR - 2, q * HR + HR + 2), 0, H - 1)
            xh = np.ascontiguousarray(xp[b][:, :, rows, :]).astype(np.float32)
            m0 = 0.0 if q == 0 else 1.0
            m1 = 0.0 if q == n_qh - 1 else 1.0
            cvec = np.stack([
                np.asarray(conv_in_b, np.float32)[:C],
                np.asarray(conv_in_b, np.float32)[C:],
                lam_re, lam_im, -lam_im, gcre, gcim,
                np.asarray(conv_out_b, np.float32),
                np.full(C, m0, np.float32), np.full(C, 1.0 - m0, np.float32),
                np.full(C, m1, np.float32), np.full(C, 1.0 - m1, np.float32),
                np.full(C, EPS, np.float32),
            ], axis=1)
            ba = np.asarray(conv_in_b, np.float32)[:C]
            bo = np.asarray(conv_out_b, np.float32)
            aux_np = np.zeros((128, 8), np.float32)
            aux_np[96:128, 0] = ba[0:32]
            aux_np[0:64, 1] = ba[32:96]
            aux_np[64:128, 1] = ba[32:96]
            aux_np[96:128, 2] = bo[0:32]
            aux_np[0:64, 3] = bo[32:96]
            aux_np[64:128, 4] = bo[32:96]
            in_maps.append({
                "xh": xh,
                "wA": wA, "wB": wB, "wR": wR, "wA0": wA0,
                "onesw": ones,
                "consts": np.ascontiguousarray(cvec),
                "aux128": aux_np,
                **wext,
            })
    return in_maps


LAST_RESULT = None  # BassKernelResults of the most recent kernel() call


def _fix_act_tables():
    """Make Ln/Exp resolve to the combined natural_log_exp_and_others set.

    The act-table-load placement pass picks each activation's first
    containing set; Ln's home (natural_log) differs from Exp's
    (exp_and_others), so an interleaved Ln/Exp stream reloads tables on
    every op (~2.7us each). Removing ln/exp from all other sets (in the
    cached dict, same keys/order, so set ids stay valid) forces both onto
    the one set that holds them together."""
    from concourse.hw_specs import get_activation_tables
    AFt = mybir.ActivationFunctionType
    for arch in ("gen3",):
        try:
            tables = get_activation_tables(arch)
        except Exception:
            continue
        for name, fns in tables.items():
            if name != "natural_log_exp_and_others":
                fns.discard(AFt.Ln)
                fns.discard(AFt.Exp)


def kernel(x, norm_w, conv_in_w, conv_in_b, nu_log, theta_log, c_re, c_im,
           conv_out_w, conv_out_b):
    global LAST_RESULT
    from concourse.bass_utils import run_bass_kernel_spmd

    # KERNEL_ACTFIX=1 crashes the device (walrus/NRT act.json id mismatch);
    # stage_rest's Ln/Exp bursting achieves the same goal safely.
    if os.environ.get("KERNEL_ACTFIX", "0") == "1":
        _fix_act_tables()

    x = np.asarray(x, np.float32)
    B, C, T, H, W = x.shape
    HR = H // QH
    if os.environ.get("KERNEL_KPACK", "1") == "1":
        in_maps = prep_core_inputs_k(x, norm_w, conv_in_w, conv_in_b, nu_log,
                                     theta_log, c_re, c_im, conv_out_w,
                                     conv_out_b, QH)
        nc = build_program_k(
            C=C, T=T, HR=HR, W=W, CT=512,
            cout_kp=os.environ.get("KERNEL_COUTKP", "1") == "1")
    else:
        in_maps = prep_core_inputs(x, norm_w, conv_in_w, conv_in_b, nu_log,
                                   theta_log, c_re, c_im, conv_out_w,
                                   conv_out_b, QH)
        nc = build_program(C=C, T=T, HR=HR, W=W, CT=512,
                           use_silu=os.environ.get("KERNEL_NO_SILU", "") != "1",
                           pack=os.environ.get("KERNEL_PACK", "1") == "1",
                           pack2=os.environ.get("KERNEL_PACK2", "0") == "1")
    trace = os.environ.get("KERNEL_TRACE", "") == "1"
    res = run_bass_kernel_spmd(nc, in_maps, list(range(N_CORES)), trace=trace)
    LAST_RESULT = res
    out = np.empty((B, C, T, H, W), np.float32)
    for core in range(N_CORES):
        b, q = core // QH, core % QH
        out[b, :, :, q * HR:(q + 1) * HR, :] = res.results[core]["out"]
    return out



# revision 42
# speedup vs baseline: 1.3188x; 1.0646x over previous
"""ConvLRUBlock Trainium2 kernel.

Reference computation (per batch b):
    h   = rms_norm(x, norm_w)                  # over channel dim
    uv  = conv3d_3x3x3(h, w_in) + b_in         # pad: replicate T/H, circular W
    u   = silu(a) * g          (a, g = uv split on channels)
    y_t = Re(h_t) c_re + Im(h_t) c_im,  h_t = lam h_{t-1} + gamma u_t  (diag LRU)
    out = x + conv3d_3x3x3(y, w_out) + b_out

Sharding: 8 cores = (batch 2) x (H quarters 4). Each core receives its H
slice plus 2 halo rows each side (edge-replicated) and the W dim circularly
padded to W+2, so no inter-core communication is needed. All conv padding is
resolved by host-side halo materialization + in-kernel index clamping (T) +
in-SBUF wrap-column fixes (W for the second conv).

In-kernel layout: channels (96) on SBUF partitions; spatial (rows x (W+2))
flattened on the free dim. 3x3x3 convs = 27 accumulating matmuls per output
tile; kh/kw become column shifts of the rhs AP, kt picks one of 3 t-slabs.
The LRU scan is 16 sequential complex steps on the vector engine.
"""

import os
from contextlib import ExitStack

import ml_dtypes
import numpy as np

import concourse.bacc as bacc
import concourse.bass as bass  # noqa: F401
import concourse.tile as tile
from concourse import mybir

F32 = mybir.dt.float32
BF16 = mybir.dt.bfloat16
ALU = mybir.AluOpType
AF = mybir.ActivationFunctionType

EPS = 1e-6

# Full-problem constants
B_FULL, C_FULL, T_FULL, H_FULL, W_FULL = 2, 96, 16, 64, 128
QH = 4  # H quarters
N_CORES = 8


def build_program(C=96, T=16, HR=16, W=128, CT=512, use_silu=True,
                  pack=False, pack2=False):
    """Build the single-core SPMD Bass program.

    C: channels; T: time steps; HR: output H rows per core; W: width.
    CT: matmul/psum column tile (<=512). use_silu: Silu on ACT vs
    Sigmoid+mults (the simulator does not implement Silu).
    """
    Wp = W + 2           # circular-padded width
    RIN = HR + 4         # input rows (2 halo each side, for two convs)
    RU = HR + 2          # u/y rows (1 halo each side, for conv_out)
    NIN = RIN * Wp       # flattened input cols per t
    NU = RU * Wp         # flattened u/y cols per t
    NO = HR * Wp         # flattened output cols per t

    nc = bacc.Bacc()
    xh = nc.declare_dram_parameter("xh", [C, T, RIN, Wp], F32, isOutput=False)
    w_in = nc.declare_dram_parameter("w_in", [C, 27, 2 * C], BF16, isOutput=False)
    w_out = nc.declare_dram_parameter("w_out", [C, 27, C], BF16, isOutput=False)
    onesw = nc.declare_dram_parameter("onesw", [C, 128], BF16, isOutput=False)
    consts = nc.declare_dram_parameter("consts", [C, 13], F32, isOutput=False)
    consts2 = nc.declare_dram_parameter("consts2", [128, 2], F32, isOutput=False)
    out = nc.declare_dram_parameter("out", [C, T, HR, W], F32, isOutput=True)

    def col_tiles(total):
        return [(i, min(CT, total - i)) for i in range(0, total, CT)]

    with tile.TileContext(nc) as tc, ExitStack() as ctx:
        singles = ctx.enter_context(tc.tile_pool(name="singles", bufs=1))
        xpool = ctx.enter_context(tc.tile_pool(name="xpool", bufs=2))
        sqpool = ctx.enter_context(tc.tile_pool(name="sqpool", bufs=2))
        statpool = ctx.enter_context(tc.tile_pool(name="statpool", bufs=2))
        hnpool = ctx.enter_context(tc.tile_pool(name="hnpool", bufs=4))
        sapool = ctx.enter_context(tc.tile_pool(name="sapool", bufs=3))
        bpool = ctx.enter_context(tc.tile_pool(name="bpool", bufs=2))
        hrpool = ctx.enter_context(tc.tile_pool(name="hrpool", bufs=2))
        hipool = ctx.enter_context(tc.tile_pool(name="hipool", bufs=2))
        tmppool = ctx.enter_context(tc.tile_pool(name="tmppool", bufs=2))
        ypool = ctx.enter_context(tc.tile_pool(name="ypool", bufs=4))
        opool = ctx.enter_context(tc.tile_pool(name="opool", bufs=2))
        psN = ctx.enter_context(tc.tile_pool(name="psN", bufs=2, space="PSUM"))
        psA = ctx.enter_context(tc.tile_pool(name="psA", bufs=2, space="PSUM"))
        psG = ctx.enter_context(tc.tile_pool(name="psG", bufs=2, space="PSUM"))
        psO = ctx.enter_context(tc.tile_pool(name="psO", bufs=2, space="PSUM"))

        sb_win = singles.tile([C, 27, 2 * C], BF16)
        nc.sync.dma_start(out=sb_win[:], in_=w_in[:])
        sb_wout = singles.tile([C, 27, C], BF16)
        nc.sync.dma_start(out=sb_wout[:], in_=w_out[:])
        sb_ones = singles.tile([C, 128], BF16)
        nc.sync.dma_start(out=sb_ones[:], in_=onesw[:])
        sb_c = singles.tile([C, 13], F32)
        nc.sync.dma_start(out=sb_c[:], in_=consts[:])
        sb_c2 = singles.tile([128, 2], F32)
        nc.sync.dma_start(out=sb_c2[:], in_=consts2[:])
        c_aux0 = sb_c2[:, 0:1]
        c_aux1 = sb_c2[:, 1:2]
        c_ba = sb_c[:, 0:1]
        c_bg = sb_c[:, 1:2]
        c_lr = sb_c[:, 2:3]
        c_li = sb_c[:, 3:4]
        c_nli = sb_c[:, 4:5]
        c_gcre = sb_c[:, 5:6]
        c_gcim = sb_c[:, 6:7]
        c_bout = sb_c[:, 7:8]
        c_m0 = sb_c[:, 8:9]
        c_1m0 = sb_c[:, 9:10]
        c_m1 = sb_c[:, 10:11]
        c_1m1 = sb_c[:, 11:12]
        c_eps = sb_c[:, 12:13]

        # Warm-up reads: make each compute engine observe the const-DMA
        # semaphores early, so steady-state ops carry at most one sync wait
        # (walrus rejects DVE ops with two wait commands).
        wu_v = singles.tile([C, 13], F32)
        nc.vector.tensor_copy(wu_v[:], sb_c[:])
        wu_s = singles.tile([C, 13], F32)
        nc.scalar.activation(wu_s[:], sb_c[:], AF.Square)

        touchpool = ctx.enter_context(tc.tile_pool(name="touchpool", bufs=2))
        if pack:
            gspool = ctx.enter_context(tc.tile_pool(name="gspool", bufs=2))
            gfpool = ctx.enter_context(tc.tile_pool(name="gfpool", bufs=2))
        if pack2:
            piecepool = ctx.enter_context(tc.tile_pool(name="piecepool", bufs=2))

        def touch(ap, engines="v"):
            """Tiny read of a freshly-DMA'd tile so the engine observes the
            DMA-queue semaphore here; later big consumers then carry only
            engine-sem waits (walrus rejects DVE ops with 2 sync waits)."""
            if "v" in engines:
                tv = touchpool.tile([C, 1], F32, tag="tv")
                nc.vector.tensor_copy(tv[:], ap)
            if "s" in engines:
                ts_ = touchpool.tile([C, 1], F32, tag="ts")
                nc.scalar.activation(ts_[:], ap, AF.Square)

        hn_slabs = [None] * T  # hnorm tiles, data at col offset 1
        y_slabs = [None] * T   # y tiles (bf16), data at col offset 1

        def stage_a(t):
            """x[t] -> hnorm[t] (rms-normed, bf16, [C, 1+NIN+1])."""
            xt = xpool.tile([C, RIN, Wp], F32, tag="xt")
            nc.sync.dma_start(out=xt[:], in_=xh[:, t])
            touch(xt[:, 0, 0:1], engines="vs")
            xf = xt[:].rearrange("p r w -> p (r w)")
            hn = hnpool.tile([C, 1 + NIN + 1], BF16, tag="hn")
            nc.vector.memset(hn[:, 0:1], 0.0)
            nc.vector.memset(hn[:, 1 + NIN:], 0.0)
            for c0, n in col_tiles(NIN):
                sq = sqpool.tile([C, CT], BF16, tag="sq")
                nc.scalar.activation(sq[:, :n], xf[:, c0:c0 + n], AF.Square)
                ps = psN.tile([128, CT], F32, tag="psn")
                nc.tensor.matmul(ps[:, :n], sb_ones[:], sq[:, :n],
                                 start=True, stop=True)
                lg = statpool.tile([C, CT], F32, tag="lg")
                nc.scalar.activation(lg[:, :n], ps[:C, :n], AF.Ln,
                                     scale=1.0 / C, bias=c_eps)
                inv = statpool.tile([C, CT], F32, tag="inv")
                nc.scalar.activation(inv[:, :n], lg[:, :n], AF.Exp, scale=-0.5)
                nc.vector.tensor_mul(hn[:, 1 + c0:1 + c0 + n],
                                     xf[:, c0:c0 + n], inv[:, :n])
            hn_slabs[t] = hn
            return hn

        def gate_epilogue(pa, pg_sb, bt, c0, n):
            """silu(a+ba)*(g+bg) for one coltile; a=pa[0:C] (psum),
            g already realigned to pg_sb [C, n] (sbuf)."""
            if use_silu:
                sa = sapool.tile([C, CT], BF16, tag="sa")
                nc.scalar.activation(sa[:, :n], pa[:C, :n], AF.Silu,
                                     bias=c_ba)
            else:
                sg = sapool.tile([C, CT], BF16, tag="sg")
                nc.scalar.activation(sg[:, :n], pa[:C, :n], AF.Sigmoid,
                                     bias=c_ba)
                av = sapool.tile([C, CT], F32, tag="av")
                nc.vector.scalar_tensor_tensor(av[:, :n], pa[:C, :n], c_ba,
                                               sg[:, :n], ALU.add,
                                               ALU.bypass)
                sa = sapool.tile([C, CT], BF16, tag="sa")
                nc.vector.tensor_mul(sa[:, :n], sg[:, :n], av[:, :n])
            nc.vector.scalar_tensor_tensor(bt[:, c0:c0 + n], pg_sb[:, :n],
                                           c_bg, sa[:, :n],
                                           ALU.add, ALU.mult)

        def conv_in_packed(t):
            """1.5-array-pass conv_in: pass1 M=128 (a0..95,g0..31), pass2
            col-tiled pairs of M=64 (g32..95) for two coltiles at once."""
            slabs = [hn_slabs[min(max(t + kt - 1, 0), T - 1)] for kt in range(3)]
            bt = bpool.tile([C, NU], BF16, tag="bt")
            cts = col_tiles(NU)
            for p0 in range(0, len(cts), 2):
                pair = cts[p0:p0 + 2]
                pas = []
                for c0, n in pair:
                    pa = psA.tile([128, CT], F32, tag="pa")
                    for kt in range(3):
                        rhs_all = slabs[kt]
                        for kh in range(3):
                            for kw in range(3):
                                off = kt * 9 + kh * 3 + kw
                                s = 1 + c0 + kh * Wp + kw - 1
                                nc.tensor.matmul(
                                    pa[:, :n], sb_win[:, off, 0:128],
                                    rhs_all[:, s:s + n],
                                    start=(off == 0), stop=(off == 26))
                    pas.append(pa)
                pg = psG.tile([128, CT], F32, tag="pg")
                for kt in range(3):
                    rhs_all = slabs[kt]
                    for kh in range(3):
                        for kw in range(3):
                            off = kt * 9 + kh * 3 + kw
                            for j, (c0, n) in enumerate(pair):
                                s = 1 + c0 + kh * Wp + kw - 1
                                b = 64 * j
                                nc.tensor.matmul(
                                    pg[b:b + 64, :n],
                                    sb_win[:, off, 128:192],
                                    rhs_all[:, s:s + n],
                                    start=(off == 0), stop=(off == 26),
                                    tile_position=(0, b),
                                    skip_group_check=True)
                for j, (c0, n) in enumerate(pair):
                    b = 64 * j
                    pa = pas[j]
                    gsa = gspool.tile([128, CT], F32, tag="gsa")
                    nc.vector.tensor_copy(gsa[96:128, :n], pa[96:128, :n])
                    gsb = gspool.tile([128, CT], F32, tag="gsb")
                    nc.vector.tensor_copy(gsb[b:b + 64, :n], pg[b:b + 64, :n])
                    gf = gfpool.tile([C, CT], F32, tag="gf")
                    nc.sync.dma_start(out=gf[0:32, :n], in_=gsa[96:128, :n])
                    nc.sync.dma_start(out=gf[32:96, :n], in_=gsb[b:b + 64, :n])
                    gate_epilogue(pa, gf, bt, c0, n)
            return bt

        def conv_in(t):
            """hnorm[t-1..t+1] -> b[t] = silu(a+ba)*(g+bg), bf16 [C, NU]."""
            slabs = [hn_slabs[min(max(t + kt - 1, 0), T - 1)] for kt in range(3)]
            bt = bpool.tile([C, NU], BF16, tag="bt")
            for c0, n in col_tiles(NU):
                pa = psA.tile([C, CT], F32, tag="pa")
                pg = psG.tile([C, CT], F32, tag="pg")
                for kt in range(3):
                    rhs_all = slabs[kt]
                    for kh in range(3):
                        for kw in range(3):
                            off = kt * 9 + kh * 3 + kw
                            s = 1 + c0 + kh * Wp + kw - 1
                            rhs = rhs_all[:, s:s + n]
                            nc.tensor.matmul(pa[:, :n], sb_win[:, off, 0:C],
                                             rhs, start=(off == 0),
                                             stop=(off == 26))
                for kt in range(3):
                    rhs_all = slabs[kt]
                    for kh in range(3):
                        for kw in range(3):
                            off = kt * 9 + kh * 3 + kw
                            s = 1 + c0 + kh * Wp + kw - 1
                            rhs = rhs_all[:, s:s + n]
                            nc.tensor.matmul(pg[:, :n], sb_win[:, off, C:2 * C],
                                             rhs, start=(off == 0),
                                             stop=(off == 26))
                gate_epilogue(pa, pg, bt, c0, n)
            return bt

        scan_state = [None, None]  # hr, hi tiles [C, NU] f32

        def scan_step(t, bt):
            """LRU step + projection -> y[t] (bf16 slab, data at offset 1)."""
            hr_new = hrpool.tile([C, NU], F32, tag="hr")
            hi_new = hipool.tile([C, NU], F32, tag="hi")
            if t == 0:
                nc.vector.tensor_copy(hr_new[:], bt[:])
                nc.vector.memset(hi_new[:], 0.0)
            else:
                hr_old, hi_old = scan_state
                t1 = tmppool.tile([C, NU], F32, tag="tA")
                nc.vector.scalar_tensor_tensor(t1[:], hi_old[:], c_nli, bt[:],
                                               ALU.mult, ALU.add)
                nc.vector.scalar_tensor_tensor(hr_new[:], hr_old[:], c_lr,
                                               t1[:], ALU.mult, ALU.add)
                t2 = tmppool.tile([C, NU], F32, tag="tB")
                nc.vector.scalar_tensor_tensor(t2[:], hi_old[:], c_lr,
                                               hi_old[:], ALU.mult, ALU.bypass)
                nc.vector.scalar_tensor_tensor(hi_new[:], hr_old[:], c_li,
                                               t2[:], ALU.mult, ALU.add)
            scan_state[0], scan_state[1] = hr_new, hi_new
            t3 = tmppool.tile([C, NU], F32, tag="tA")
            nc.vector.scalar_tensor_tensor(t3[:], hr_new[:], c_gcre,
                                           hr_new[:], ALU.mult, ALU.bypass)
            yt = ypool.tile([C, 1 + NU + 1], BF16, tag="yt")
            nc.vector.memset(yt[:, 0:1], 0.0)
            nc.vector.memset(yt[:, 1 + NU:], 0.0)
            nc.vector.scalar_tensor_tensor(yt[:, 1:1 + NU], hi_new[:], c_gcim,
                                           t3[:], ALU.mult, ALU.add)
            # W wrap columns: col 0 <- col W (w=W-1), col W+1 <- col 1 (w=0)
            yv = yt[:, 1:1 + NU].rearrange("p (r w) -> p r w", w=Wp)
            nc.vector.tensor_copy(yv[:, :, 0:1], yv[:, :, W:W + 1])
            nc.vector.tensor_copy(yv[:, :, W + 1:W + 2], yv[:, :, 1:2])
            # H edge replication (active only on global-edge cores, via mask):
            # row0 <- m0*row0 + (1-m0)*row1 ; last <- m1*last + (1-m1)*prev
            e0 = tmppool.tile([C, Wp], F32, tag="tE")
            nc.vector.scalar_tensor_tensor(e0[:], yv[:, 1, :], c_1m0,
                                           yv[:, 1, :], ALU.mult, ALU.bypass)
            nc.vector.scalar_tensor_tensor(yv[:, 0, :], yv[:, 0, :], c_m0,
                                           e0[:], ALU.mult, ALU.add)
            e1 = tmppool.tile([C, Wp], F32, tag="tE")
            nc.vector.scalar_tensor_tensor(e1[:], yv[:, RU - 2, :], c_1m1,
                                           yv[:, RU - 2, :], ALU.mult, ALU.bypass)
            nc.vector.scalar_tensor_tensor(yv[:, RU - 1, :], yv[:, RU - 1, :],
                                           c_m1, e1[:], ALU.mult, ALU.add)
            y_slabs[t] = yt
            return yt

        def conv_out(t):
            """y[t-1..t+1] -> out[t] = x + conv(y) + b_out."""
            slabs = [y_slabs[min(max(t + kt - 1, 0), T - 1)] for kt in range(3)]
            ot = opool.tile([C, HR, Wp], F32, tag="ot")
            # residual input loaded into the output staging tile
            nc.sync.dma_start(out=ot[:], in_=xh[:, t, 2:2 + HR, :])
            touch(ot[:, 0, 0:1], engines="v")
            of = ot[:].rearrange("p r w -> p (r w)")
            for c0, n in col_tiles(NO):
                po = psO.tile([C, CT], F32, tag="po")
                for kt in range(3):
                    rhs_all = slabs[kt]
                    for kh in range(3):
                        for kw in range(3):
                            off = kt * 9 + kh * 3 + kw
                            s = 1 + c0 + kh * Wp + kw - 1
                            rhs = rhs_all[:, s:s + n]
                            nc.tensor.matmul(po[:, :n], sb_wout[:, off, :],
                                             rhs, start=(off == 0),
                                             stop=(off == 26))
                nc.vector.scalar_tensor_tensor(of[:, c0:c0 + n], po[:, :n],
                                               c_bout, of[:, c0:c0 + n],
                                               ALU.add, ALU.add)
            nc.sync.dma_start(out=out[:, t], in_=ot[:, :, 1:1 + W])

        def conv_out_packed(t):
            """conv_out with array packing: pairs (e,o) put e's 96 channels +
            o's first 32 (pos 96) in one pass; the two pairs' leftover 64
            channels share one concurrent col-tiled pass. Misplaced pieces are
            realigned into the staging tile via SBUF->SBUF DMA."""
            slabs = [y_slabs[min(max(t + kt - 1, 0), T - 1)] for kt in range(3)]
            ot = opool.tile([C, HR, Wp], F32, tag="ot")
            nc.sync.dma_start(out=ot[:], in_=xh[:, t, 2:2 + HR, :])
            touch(ot[:, 0, 0:1], engines="v")
            of = ot[:].rearrange("p r w -> p (r w)")
            xflat = xh[:, t, 2:2 + HR, :].rearrange("p r w -> p (r w)")
            cts = col_tiles(NO)

            def mm_group(ps, prange, wslice, c0, n, pos):
                for kt in range(3):
                    rhs_all = slabs[kt]
                    for kh in range(3):
                        for kw in range(3):
                            off = kt * 9 + kh * 3 + kw
                            s = 1 + c0 + kh * Wp + kw - 1
                            nc.tensor.matmul(
                                ps[prange[0]:prange[1], :n],
                                sb_wout[:, off, wslice[0]:wslice[1]],
                                rhs_all[:, s:s + n],
                                start=(off == 0), stop=(off == 26),
                                tile_position=pos, skip_group_check=True)

            def mm_group2(ps, jobs):
                """Interleaved concurrent accumulation groups."""
                for kt in range(3):
                    rhs_all = slabs[kt]
                    for kh in range(3):
                        for kw in range(3):
                            off = kt * 9 + kh * 3 + kw
                            for prange, wslice, c0, n, pos in jobs:
                                s = 1 + c0 + kh * Wp + kw - 1
                                nc.tensor.matmul(
                                    ps[prange[0]:prange[1], :n],
                                    sb_wout[:, off, wslice[0]:wslice[1]],
                                    rhs_all[:, s:s + n],
                                    start=(off == 0), stop=(off == 26),
                                    tile_position=pos, skip_group_check=True)

            def aligned_epi(ps, c0, n):
                nc.vector.scalar_tensor_tensor(of[:, c0:c0 + n], ps[:C, :n],
                                               c_bout, of[:, c0:c0 + n],
                                               ALU.add, ALU.add)

            def piece_epi(ps, pbase, psize, ch0, c0, n):
                """Residual-add for a channel piece at partitions
                [pbase, pbase+psize) holding channels [ch0, ch0+psize);
                realign into ot via DMA."""
                xp = piecepool.tile([128, CT], F32, tag="xp")
                nc.sync.dma_start(out=xp[pbase:pbase + psize, :n],
                                  in_=xflat[ch0:ch0 + psize, c0:c0 + n])
                aux = c_aux0 if pbase == 96 else c_aux1
                pt = piecepool.tile([128, CT], F32, tag="pc")
                nc.vector.scalar_tensor_tensor(
                    pt[pbase:pbase + psize, :n],
                    ps[pbase:pbase + psize, :n],
                    aux[pbase:pbase + psize, :],
                    xp[pbase:pbase + psize, :n], ALU.add, ALU.add)
                nc.sync.dma_start(out=of[ch0:ch0 + psize, c0:c0 + n],
                                  in_=pt[pbase:pbase + psize, :n])

            for e, o in ((0, 1), (2, 3)):
                (ce, ne), (co_, no_) = cts[e], cts[o]
                p1 = psO.tile([128, CT], F32, tag="po")
                mm_group2(p1, [((0, 32), (0, 32), ce, ne, (0, 0)),
                               ((32, 64), (32, 64), ce, ne, (0, 32)),
                               ((64, 96), (64, 96), ce, ne, (0, 64)),
                               ((96, 128), (0, 32), co_, no_, (0, 96))])
                aligned_epi(p1, ce, ne)
                piece_epi(p1, 96, 32, 0, co_, no_)
            p2 = psG.tile([128, CT], F32, tag="pg")
            mm_group2(p2, [((0, 32), (32, 64), cts[1][0], cts[1][1], (0, 0)),
                           ((32, 64), (64, 96), cts[1][0], cts[1][1], (0, 32)),
                           ((64, 96), (32, 64), cts[3][0], cts[3][1], (0, 64)),
                           ((96, 128), (64, 96), cts[3][0], cts[3][1], (0, 96))])
            piece_epi(p2, 0, 64, 32, cts[1][0], cts[1][1])
            piece_epi(p2, 64, 64, 32, cts[3][0], cts[3][1])
            p4 = psO.tile([128, CT], F32, tag="po")
            mm_group(p4, (0, 96), (0, 96), cts[4][0], cts[4][1], (0, 0))
            aligned_epi(p4, cts[4][0], cts[4][1])
            nc.sync.dma_start(out=out[:, t], in_=ot[:, :, 1:1 + W])

        octs = col_tiles(NO)
        use_p2 = pack2 and len(octs) == 5 and all(n == CT for _, n in octs[:4])
        co_fn = conv_out_packed if use_p2 else conv_out

        stage_a(0)
        if T > 1:
            stage_a(1)
        for t in range(T):
            if t + 1 < T:
                stage_a(t + 1)
            bt = conv_in_packed(t) if pack else conv_in(t)
            scan_step(t, bt)
            if t >= 1:
                co_fn(t - 1)
        co_fn(T - 1)

    nc.compile()
    return nc


def build_program_k(C=96, T=16, HR=16, W=128, CT=512, cout_kp=False):
    """K=128-packed SPMD program.

    Each conv's 27-tap x 96-ch contraction (2592 rows) is regrouped into 21
    matmul streams per output tile instead of 27:
      - 9 A-streams: [slab_prev ch0..95 ; slab_cur ch0..31] (dup rows DMA'd
        into partitions 96..127 of the prev slab tile), one per (kh,kw).
      - 9 B-streams: Q-tile = [slab_cur ch32..95 ; slab_next ch0..63].
      - 3 R-streams: per-kh slabs pack slab_next ch64..95 at the 3 kw
        column shifts (K=96).
    t=0 uses special A-weights (kt0+kt1 folded for ch0..31, K=96) because
    the clamped prev slab's dup rows hold the wrong timestep.

    Pipeline: normalization (stage) runs 3 timesteps ahead, slab-combining
    DMAs (build_in) 1 ahead with a full iteration of slack; x**2 runs on
    DVE and rms uses Sqrt(ACT)+reciprocal(DVE) so the ACT engine only ever
    holds the Sqrt+Silu tables (no table thrash) and the stage matmuls
    never block the PE FIFO.
    """
    Wp = W + 2
    RIN = HR + 4
    RU = HR + 2
    NIN = RIN * Wp
    NU = RU * Wp
    NO = HR * Wp
    C2 = 2 * C

    nc = bacc.Bacc()
    xh = nc.declare_dram_parameter("xh", [C, T, RIN, Wp], F32, isOutput=False)
    wA = nc.declare_dram_parameter("wA", [128, 9, C2], BF16, isOutput=False)
    wB = nc.declare_dram_parameter("wB", [128, 9, C2], BF16, isOutput=False)
    wR = nc.declare_dram_parameter("wR", [96, 3, C2], BF16, isOutput=False)
    wA0 = nc.declare_dram_parameter("wA0", [96, 9, C2], BF16, isOutput=False)
    if cout_kp:
        wKA = nc.declare_dram_parameter("wKA", [128, 9, C], BF16, isOutput=False)
        wKB = nc.declare_dram_parameter("wKB", [128, 9, C], BF16, isOutput=False)
        wKR = nc.declare_dram_parameter("wKR", [96, 3, C], BF16, isOutput=False)
        wKA0 = nc.declare_dram_parameter("wKA0", [96, 9, C], BF16, isOutput=False)
    else:
        wK = nc.declare_dram_parameter("wK", [C, 27, C], BF16, isOutput=False)
    onesw = nc.declare_dram_parameter("onesw", [C, 128], BF16, isOutput=False)
    consts = nc.declare_dram_parameter("consts", [C, 13], F32, isOutput=False)
    aux = nc.declare_dram_parameter("aux128", [128, 8], F32, isOutput=False)
    out = nc.declare_dram_parameter("out", [C, T, HR, W], F32, isOutput=True)

    def col_tiles(total):
        return [(i, min(CT, total - i)) for i in range(0, total, CT)]

    with tile.TileContext(nc) as tc, ExitStack() as ctx:
        singles = ctx.enter_context(tc.tile_pool(name="singles", bufs=1))
        xpool = ctx.enter_context(tc.tile_pool(name="xpool", bufs=2))
        sqpool = ctx.enter_context(tc.tile_pool(name="sqpool", bufs=1))
        statpool = ctx.enter_context(tc.tile_pool(name="statpool", bufs=1))
        hnpool = ctx.enter_context(tc.tile_pool(name="hnpool", bufs=5))
        qinpool = ctx.enter_context(tc.tile_pool(name="qinpool", bufs=2))
        rinpool = ctx.enter_context(tc.tile_pool(name="rinpool", bufs=2))
        sapool = ctx.enter_context(tc.tile_pool(name="sapool", bufs=2))
        bpool = ctx.enter_context(tc.tile_pool(name="bpool", bufs=2))
        tmppool = ctx.enter_context(tc.tile_pool(name="tmppool", bufs=1))
        ypool = ctx.enter_context(tc.tile_pool(name="ypool", bufs=4))
        opool = ctx.enter_context(tc.tile_pool(name="opool", bufs=1))
        if cout_kp:
            qopool = ctx.enter_context(tc.tile_pool(name="qopool", bufs=2))
            rkpool = ctx.enter_context(tc.tile_pool(name="rkpool", bufs=2))
            xspool = ctx.enter_context(tc.tile_pool(name="xspool", bufs=3))
        gspool = ctx.enter_context(tc.tile_pool(name="gspool", bufs=1))
        gfpool = ctx.enter_context(tc.tile_pool(name="gfpool", bufs=2))
        touchpool = ctx.enter_context(tc.tile_pool(name="touchpool", bufs=2))
        psN = ctx.enter_context(tc.tile_pool(name="psN", bufs=1, space="PSUM"))
        psA = ctx.enter_context(tc.tile_pool(
            name="psA", bufs=3 if cout_kp else 2, space="PSUM"))
        psG = ctx.enter_context(tc.tile_pool(name="psG", bufs=2, space="PSUM"))
        psO = ctx.enter_context(tc.tile_pool(
            name="psO", bufs=2 if cout_kp else 3, space="PSUM"))

        sb_c = singles.tile([C, 13], F32)
        nc.sync.dma_start(out=sb_c[:], in_=consts[:])
        sb_aux = singles.tile([128, 8], F32)
        nc.sync.dma_start(out=sb_aux[:], in_=aux[:])
        sb_ones = singles.tile([C, 128], BF16)
        nc.sync.dma_start(out=sb_ones[:], in_=onesw[:])
        sb_wA = singles.tile([128, 9, C2], BF16)
        sb_wB = singles.tile([128, 9, C2], BF16)
        sb_wR = singles.tile([96, 3, C2], BF16)
        sb_wA0 = singles.tile([96, 9, C2], BF16)
        if cout_kp:
            sb_wKA = singles.tile([128, 9, C], BF16)
            sb_wKB = singles.tile([128, 9, C], BF16)
            sb_wKR = singles.tile([96, 3, C], BF16)
            sb_wKA0 = singles.tile([96, 9, C], BF16)
        else:
            sb_wK = singles.tile([C, 27, C], BF16)

        def emit_weight_dmas():
            nc.sync.dma_start(out=sb_wA0[:], in_=wA0[:])
            nc.sync.dma_start(out=sb_wA[:], in_=wA[:])
            nc.sync.dma_start(out=sb_wB[:], in_=wB[:])
            nc.sync.dma_start(out=sb_wR[:], in_=wR[:])
            if cout_kp:
                nc.sync.dma_start(out=sb_wKA[:], in_=wKA[:])
                nc.sync.dma_start(out=sb_wKB[:], in_=wKB[:])
                nc.sync.dma_start(out=sb_wKR[:], in_=wKR[:])
                nc.sync.dma_start(out=sb_wKA0[:], in_=wKA0[:])
            else:
                nc.sync.dma_start(out=sb_wK[:], in_=wK[:])

        c_ba = sb_c[:, 0:1]
        c_bg = sb_c[:, 1:2]
        c_lr = sb_c[:, 2:3]
        c_li = sb_c[:, 3:4]
        c_nli = sb_c[:, 4:5]
        c_gcre = sb_c[:, 5:6]
        c_gcim = sb_c[:, 6:7]
        c_bout = sb_c[:, 7:8]
        c_m0 = sb_c[:, 8:9]
        c_1m0 = sb_c[:, 9:10]
        c_m1 = sb_c[:, 10:11]
        c_1m1 = sb_c[:, 11:12]
        c_eps = sb_c[:, 12:13]
        c_ba_hi = sb_aux[:, 0:1]   # [96:128] = ba[0:32]
        c_ba_pg = sb_aux[:, 1:2]   # [0:64] and [64:128] = ba[32:96]

        # Warm-ups: observe const DMA on each engine and preload the only
        # two ACT tables used in steady state (natural_log_exp, Silu).
        wu_v = singles.tile([C, 13], F32)
        nc.vector.tensor_copy(wu_v[:], sb_c[:])
        wu_v2 = singles.tile([128, 8], F32)
        nc.vector.tensor_copy(wu_v2[:], sb_aux[:])
        wu_s = singles.tile([C, 13], F32)
        nc.scalar.activation(wu_s[:], sb_c[:], AF.Exp)
        wu_s2 = singles.tile([128, 8], BF16)
        nc.scalar.activation(wu_s2[:], sb_aux[:], AF.Silu, bias=c_ba_hi)

        def touch(ap, engines="v"):
            if "v" in engines:
                tv = touchpool.tile([C, 1], F32, tag="tv")
                nc.vector.tensor_copy(tv[:], ap)

        ag_swap = os.environ.get("KERNEL_AGSWAP", "1") == "1"

        hn_slabs = [None] * T   # [128, 1+NIN+1] bf16; rows 96:128 = dup
        sq_tiles = [None] * T
        qin_tiles = [None] * T
        rin_tiles = [None] * T
        y_slabs = [None] * T    # [128, 1+NU+1] bf16

        def stage_sq(t):
            """x[t] load + x**2 on DVE, tile-by-tile (short ops so the strict
            ACT/DVE FIFOs never head-of-line-block the latency-critical gate
            chain)."""
            xt = xpool.tile([C, RIN, Wp], F32, tag="xt")
            nc.sync.dma_start(out=xt[:], in_=xh[:, t])
            xf = xt[:].rearrange("p r w -> p (r w)")
            sq = sqpool.tile([C, NIN], BF16, tag="sq")
            for c0, n in col_tiles(NIN):
                nc.vector.tensor_mul(sq[:, c0:c0 + n], xf[:, c0:c0 + n],
                                     xf[:, c0:c0 + n])
            sq_tiles[t] = (xt, sq)

        def stage_rest(t):
            """rms reduce (PE) + table-cheap Copy drains + ONE Ln + ONE Exp.

            Ln and Exp live in different activation-table home sets; the tile
            scheduler also splices gate Silus between ACT ops, so any
            multi-op Ln/Exp sequence thrashes table loads (~2.7us each).
            Draining psum via Copy (present in every set) and doing a single
            full-width Ln then Exp caps the damage at ~3 loads per call."""
            xt, sq = sq_tiles[t]
            xf = xt[:].rearrange("p r w -> p (r w)")
            hn = hnpool.tile([128, 1 + NIN + 1], BF16, tag="hn")
            nc.vector.memset(hn[0:C, 0:1], 0.0)
            nc.vector.memset(hn[0:C, 1 + NIN:], 0.0)
            for c0, n in col_tiles(NIN):
                ps = psN.tile([128, CT], F32, tag="psn")
                nc.tensor.matmul(ps[:, :n], sb_ones[:], sq[:, c0:c0 + n],
                                 start=True, stop=True)
                nc.scalar.copy(sq[:, c0:c0 + n], ps[:C, :n])
            inv = statpool.tile([C, NIN], BF16, tag="inv")
            nc.scalar.activation(inv[:, :], sq[:, :], AF.Ln,
                                 scale=1.0 / C, bias=c_eps)
            nc.scalar.activation(sq[:, :], inv[:, :], AF.Exp, scale=-0.5)
            for c0, n in col_tiles(NIN):
                nc.vector.tensor_mul(hn[0:C, 1 + c0:1 + c0 + n],
                                     xf[:, c0:c0 + n], sq[:, c0:c0 + n])
            hn_slabs[t] = hn
            return hn

        def build_in(t):
            """Slab-combining DMAs for conv_in(t) (+dup used by t+1).
            Needs hn[t] and hn[min(t+1, T-1)] already emitted."""
            cur = hn_slabs[t]
            nxt = hn_slabs[min(t + 1, T - 1)]
            if t + 1 < T:
                nc.gpsimd.dma_start(out=cur[96:128, :], in_=nxt[0:32, :])
            qi = qinpool.tile([128, 1 + NIN + 1], BF16, tag="qi")
            nc.gpsimd.dma_start(out=qi[0:64, :], in_=cur[32:96, :])
            nc.gpsimd.dma_start(out=qi[64:128, :], in_=nxt[0:64, :])
            qin_tiles[t] = qi
            rs = []
            for kh in range(3):
                r_ = rinpool.tile([96, NU], BF16, tag=f"r{kh}")
                for kw in range(3):
                    d = kh * Wp + kw
                    nc.gpsimd.dma_start(out=r_[32 * kw:32 * kw + 32, :],
                                        in_=nxt[64:96, d:d + NU])
                rs.append(r_)
            rin_tiles[t] = rs

        def conv_in_k(t, bt_arg, pair_range):
            a_sl = hn_slabs[max(t - 1, 0)]
            wa_sb = sb_wA0 if t == 0 else sb_wA
            ka = 96 if t == 0 else 128
            q = qin_tiles[t]
            rr = rin_tiles[t]

            streams = []
            for j in range(9):
                kh, kw = divmod(j, 3)
                s = kh * Wp + kw
                streams.append((
                    lambda m0, m1, jj=j: wa_sb[0:ka, jj, m0:m1],
                    lambda c0, n, ss=s: a_sl[0:ka, ss + c0:ss + c0 + n]))
            for j in range(9):
                kh, kw = divmod(j, 3)
                s = kh * Wp + kw
                streams.append((
                    lambda m0, m1, jj=j: sb_wB[:, jj, m0:m1],
                    lambda c0, n, ss=s: q[:, ss + c0:ss + c0 + n]))
            for kh in range(3):
                streams.append((
                    lambda m0, m1, kk=kh: sb_wR[:, kk, m0:m1],
                    lambda c0, n, kk=kh: rr[kk][:, c0:c0 + n]))
            NS = len(streams)

            bt = bt_arg
            cts = col_tiles(NU)
            for p0 in pair_range:
                pair = cts[p0:p0 + 2]
                pas = []
                for c0, n in pair:
                    pa = psA.tile([128, CT], F32, tag="pa")
                    for i, (lw, rh) in enumerate(streams):
                        nc.tensor.matmul(pa[:, :n], lw(0, 128), rh(c0, n),
                                         start=(i == 0), stop=(i == NS - 1))
                    pas.append(pa)
                pg = psG.tile([128, CT], F32, tag="pg")
                for i, (lw, rh) in enumerate(streams):
                    for j, (c0, n) in enumerate(pair):
                        b = 64 * j
                        nc.tensor.matmul(
                            pg[b:b + 64, :n], lw(128, 192), rh(c0, n),
                            start=(i == 0), stop=(i == NS - 1),
                            tile_position=(0, b), skip_group_check=True)
                for j, (c0, n) in enumerate(pair):
                    b = 64 * j
                    pa = pas[j]
                    if ag_swap:
                        # Output channels are permuted so g (96) sits aligned
                        # at psum partitions 0:96 while a rides the spare
                        # slots: a[0:32] at pa[96:128], a[32:96] at pg[b:b+64].
                        # Silu runs on the pieces in place (ACT reads PSUM), a
                        # DMA realigns the bf16 silu outputs, and the final STT
                        # reads g straight from PSUM — no DVE casts.
                        sa = sapool.tile([128, CT], BF16, tag="sa")
                        nc.scalar.activation(sa[96:128, :n], pa[96:128, :n],
                                             AF.Silu, bias=c_ba_hi[96:128])
                        sb_ = sapool.tile([128, CT], BF16, tag="sb")
                        nc.scalar.activation(sb_[b:b + 64, :n],
                                             pg[b:b + 64, :n],
                                             AF.Silu, bias=c_ba_pg[b:b + 64])
                        gf = gfpool.tile([C, CT], BF16, tag="gf")
                        nc.sync.dma_start(out=gf[0:32, :n], in_=sa[96:128, :n])
                        nc.sync.dma_start(out=gf[32:96, :n],
                                          in_=sb_[b:b + 64, :n])
                        nc.vector.scalar_tensor_tensor(bt[:, c0:c0 + n],
                                                       pa[:C, :n], c_bg,
                                                       gf[:, :n],
                                                       ALU.add, ALU.mult)
                    else:
                        gsa = gspool.tile([128, CT], BF16, tag="gsa")
                        nc.vector.tensor_copy(gsa[96:128, :n], pa[96:128, :n])
                        gsb = gspool.tile([128, CT], BF16, tag="gsb")
                        nc.vector.tensor_copy(gsb[b:b + 64, :n],
                                              pg[b:b + 64, :n])
                        gf = gfpool.tile([C, CT], BF16, tag="gf")
                        nc.sync.dma_start(out=gf[0:32, :n], in_=gsa[96:128, :n])
                        nc.sync.dma_start(out=gf[32:96, :n],
                                          in_=gsb[b:b + 64, :n])
                        sa = sapool.tile([C, CT], BF16, tag="sa")
                        nc.scalar.activation(sa[:, :n], pa[:C, :n], AF.Silu,
                                             bias=c_ba)
                        nc.vector.scalar_tensor_tensor(bt[:, c0:c0 + n],
                                                       gf[:, :n], c_bg,
                                                       sa[:, :n],
                                                       ALU.add, ALU.mult)

        hr = singles.tile([C, NU], F32)
        hi = singles.tile([C, NU], F32)

        NH = (RU // 2) * Wp  # first-half columns (rows 0..RU/2-1)

        def scan_half(t, bt, yt, h0, h1):
            hrh = hr[:, h0:h1]
            hih = hi[:, h0:h1]
            bth = bt[:, h0:h1]
            if t == 0:
                nc.vector.tensor_copy(hrh, bth)
                nc.vector.memset(hih, 0.0)
            else:
                t1 = tmppool.tile([C, NH], F32, tag="tA")
                nc.vector.scalar_tensor_tensor(t1[:, :h1 - h0], hih, c_nli,
                                               bth, ALU.mult, ALU.add)
                nc.vector.scalar_tensor_tensor(hih, hih, c_lr, hih,
                                               ALU.mult, ALU.bypass)
                nc.vector.scalar_tensor_tensor(hih, hrh, c_li, hih,
                                               ALU.mult, ALU.add)
                nc.vector.scalar_tensor_tensor(hrh, hrh, c_lr,
                                               t1[:, :h1 - h0],
                                               ALU.mult, ALU.add)
            t3 = tmppool.tile([C, NH], F32, tag="tA")
            nc.vector.scalar_tensor_tensor(t3[:, :h1 - h0], hrh, c_gcre,
                                           hrh, ALU.mult, ALU.bypass)
            nc.vector.scalar_tensor_tensor(yt[0:C, 1 + h0:1 + h1], hih,
                                           c_gcim, t3[:, :h1 - h0],
                                           ALU.mult, ALU.add)
            yv = yt[0:C, 1 + h0:1 + h1].rearrange("p (r w) -> p r w", w=Wp)
            nr = (h1 - h0) // Wp
            nc.vector.tensor_copy(yv[:, :, 0:1], yv[:, :, W:W + 1])
            nc.vector.tensor_copy(yv[:, :, W + 1:W + 2], yv[:, :, 1:2])
            if h0 == 0:
                e0 = tmppool.tile([C, Wp], F32, tag="tE")
                nc.vector.scalar_tensor_tensor(e0[:], yv[:, 1, :], c_1m0,
                                               yv[:, 1, :], ALU.mult,
                                               ALU.bypass)
                nc.vector.scalar_tensor_tensor(yv[:, 0, :], yv[:, 0, :],
                                               c_m0, e0[:], ALU.mult,
                                               ALU.add)
            else:
                e1 = tmppool.tile([C, Wp], F32, tag="tE")
                nc.vector.scalar_tensor_tensor(e1[:], yv[:, nr - 2, :],
                                               c_1m1, yv[:, nr - 2, :],
                                               ALU.mult, ALU.bypass)
                nc.vector.scalar_tensor_tensor(yv[:, nr - 1, :],
                                               yv[:, nr - 1, :], c_m1,
                                               e1[:], ALU.mult, ALU.add)

        def conv_out_k(t):
            """Direct 27-tap conv_out: kt-ordered so the y[t]-dependent
            taps (kt2) come last in each accumulation group."""
            slabs = [y_slabs[min(max(t + kt - 1, 0), T - 1)] for kt in range(3)]
            ot = opool.tile([C, HR, Wp], F32, tag="ot")
            nc.sync.dma_start(out=ot[:], in_=xh[:, t, 2:2 + HR, :])
            touch(ot[:, 0, 0:1], engines="v")
            of = ot[:].rearrange("p r w -> p (r w)")
            for c0, n in col_tiles(NO):
                po = psO.tile([C, CT], F32, tag="po", name="po")
                for kt in range(3):
                    rhs_all = slabs[kt]
                    for kh in range(3):
                        for kw in range(3):
                            off = kt * 9 + kh * 3 + kw
                            s = 1 + c0 + kh * Wp + kw - 1
                            nc.tensor.matmul(po[:, :n], sb_wK[:, off, :],
                                             rhs_all[0:C, s:s + n],
                                             start=(off == 0),
                                             stop=(off == 26))
                nc.vector.scalar_tensor_tensor(of[:, c0:c0 + n], po[:, :n],
                                               c_bout, of[:, c0:c0 + n],
                                               ALU.add, ALU.add)
            nc.sync.dma_start(out=out[:, t], in_=ot[:, :, 1:1 + W])

        # ---- K-packed conv_out --------------------------------------------
        # Same 21-stream K-regrouping as conv_in (A: y[s-1]96 + y[s]0:32 via
        # dup rows; Q: y[s]32:96 + y[s+1]0:64 materialized; R: y[s+1]64:96 at
        # 9 kw shifts). M stays 96-wide and aligned (measured: col-tiled
        # M-packing costs ~330ns/group-step vs 260 for a plain pass, so it
        # saves nothing and complicates the epilogue). Outputs use exact
        # 4-row x 128-col tiles via 3D rhs APs (no wrap-col compute).
        # Queue split: combines on gpsimd, epilogue x/out DMAs on scalar,
        # conv_in's gate realigns keep sync — so none of them FIFO-couple.
        qro = {}          # s -> (qo tile, [r tiles kh=0..2])
        RKW = HR * Wp     # R-tile width (only (HR-1)*Wp + W + ... used)

        def build_out(t):
            """After scan(t): dup rows for y[t-1]; qo/R for conv_out(t-1)."""
            if t == 0:
                return
            s = t - 1
            cur, nxt = y_slabs[s], y_slabs[t]
            nc.gpsimd.dma_start(out=cur[96:128, :], in_=nxt[0:32, :])
            _build_qr(s, cur, nxt)

        def _build_qr(s, cur, nxt):
            qo = qopool.tile([128, 1 + NU + 1], BF16, tag="qo")
            nc.gpsimd.dma_start(out=qo[0:64, :], in_=cur[32:96, :])
            nc.gpsimd.dma_start(out=qo[64:128, :], in_=nxt[0:64, :])
            rs = []
            for kh in range(3):
                r_ = rkpool.tile([96, RKW], BF16, tag=f"rk{kh}")
                for kw in range(3):
                    d = kh * Wp + kw
                    nw = (HR - 1) * Wp + W + 2 - kw
                    nc.gpsimd.dma_start(out=r_[32 * kw:32 * kw + 32, 0:nw],
                                        in_=nxt[64:96, 1 + d:1 + d + nw])
                rs.append(r_)
            qro[s] = (qo, rs)

        def conv_out_kd(s):
            """21 K-streams x 4 col tiles, M=96 aligned; writes out[:, s]."""
            prev = y_slabs[max(s - 1, 0)]
            wa_sb = sb_wKA0 if s == 0 else sb_wKA
            ka = 96 if s == 0 else 128
            qo, rs = qro[s]
            pv = prev[:, 1:1 + NU].rearrange("p (r w) -> p r w", w=Wp)
            qv = qo[:, 1:1 + NU].rearrange("p (r w) -> p r w", w=Wp)
            rv = [r_[:].rearrange("p (r w) -> p r w", w=Wp) for r_ in rs]

            streams = []
            for j in range(9):
                kh, kw = divmod(j, 3)
                streams.append((
                    lambda jj=j: wa_sb[0:ka, jj, :],
                    lambda r0, kh=kh, kw=kw:
                        pv[0:ka, r0 + kh:r0 + kh + 4, kw:kw + 128]))
            for j in range(9):
                kh, kw = divmod(j, 3)
                streams.append((
                    lambda jj=j: sb_wKB[:, jj, :],
                    lambda r0, kh=kh, kw=kw:
                        qv[:, r0 + kh:r0 + kh + 4, kw:kw + 128]))
            for kh in range(3):
                streams.append((
                    lambda kk=kh: sb_wKR[:, kk, :],
                    lambda r0, kk=kh: rv[kk][:, r0:r0 + 4, 0:128]))
            NS = len(streams)

            for r0 in (0, 4, 8, 12):
                xs = xspool.tile([128, CT], F32, tag="xs")
                nc.scalar.dma_start(
                    out=xs[0:96, :],
                    in_=xh[:, s, 2 + r0:2 + r0 + 4, 1:1 + W])
                tv = touchpool.tile([128, 1], F32, tag="tvp")
                nc.vector.tensor_copy(tv[0:96, :], xs[0:96, 0:1])
                po = psO.tile([128, CT], F32, tag="po")
                for i, (lw, rh) in enumerate(streams):
                    nc.tensor.matmul(po[0:96, :], lw(), rh(r0),
                                     start=(i == 0), stop=(i == NS - 1))
                nc.vector.scalar_tensor_tensor(xs[0:96, :], po[0:96, :],
                                               c_bout, xs[0:96, :],
                                               ALU.add, ALU.add)
                nc.scalar.dma_start(out=out[:, s, r0:r0 + 4, :],
                                    in_=xs[0:96, :])

        for u in range(min(3, T)):
            stage_sq(u)
            stage_rest(u)
        emit_weight_dmas()
        build_in(0)
        for t in range(T):
            if t + 1 < T:
                build_in(t + 1)
            if t + 3 < T:
                stage_sq(t + 3)
            bt = bpool.tile([C, NU], BF16, tag="bt")
            yt = ypool.tile([128, 1 + NU + 1], BF16, tag="yt")
            nc.vector.memset(yt[0:C, 0:1], 0.0)
            nc.vector.memset(yt[0:C, 1 + NU:], 0.0)
            conv_in_k(t, bt, [0, 2])
            scan_half(t, bt, yt, 0, NH)
            conv_in_k(t, bt, [4])
            scan_half(t, bt, yt, NH, NU)
            y_slabs[t] = yt
            if cout_kp:
                build_out(t)
                if t >= 2:
                    conv_out_kd(t - 2)
            elif t >= 1:
                conv_out_k(t - 1)
            if t + 3 < T:
                stage_rest(t + 3)
        if cout_kp:
            _build_qr(T - 1, y_slabs[T - 1], y_slabs[T - 1])
            conv_out_kd(T - 2)
            conv_out_kd(T - 1)
        else:
            conv_out_k(T - 1)

    nc.compile()
    return nc



def prep_core_inputs(x, norm_w, conv_in_w, conv_in_b, nu_log, theta_log,
                     c_re, c_im, conv_out_w, conv_out_b, n_qh):
    """Build per-core input maps. Cores = batch-major, then H quarters."""
    B, C, T, H, W = x.shape
    HR = H // n_qh

    nu = np.exp(np.asarray(nu_log, np.float64))
    theta = np.exp(np.asarray(theta_log, np.float64))
    lam_re = (np.exp(-nu) * np.cos(theta)).astype(np.float32)
    lam_im = (np.exp(-nu) * np.sin(theta)).astype(np.float32)
    gamma = np.sqrt(1.0 - np.exp(-2.0 * nu))
    gcre = (gamma * np.asarray(c_re, np.float64)).astype(np.float32)
    gcim = (gamma * np.asarray(c_im, np.float64)).astype(np.float32)

    w_in_f = np.asarray(conv_in_w, np.float32) * \
        np.asarray(norm_w, np.float32)[None, :, None, None, None]
    w_in_t = np.ascontiguousarray(
        np.transpose(w_in_f, (1, 2, 3, 4, 0)).reshape(C, 27, 2 * C)
    ).astype(ml_dtypes.bfloat16)
    w_out_t = np.ascontiguousarray(
        np.transpose(np.asarray(conv_out_w, np.float32),
                     (1, 2, 3, 4, 0)).reshape(C, 27, C)
    ).astype(ml_dtypes.bfloat16)
    ones = np.ones((C, 128), ml_dtypes.bfloat16)

    xp = np.concatenate([x[..., -1:], x, x[..., :1]], axis=-1)  # W circular

    in_maps = []
    for b in range(B):
        for q in range(n_qh):
            rows = np.clip(np.arange(q * HR - 2, q * HR + HR + 2), 0, H - 1)
            xh = np.ascontiguousarray(xp[b][:, :, rows, :]).astype(np.float32)
            m0 = 0.0 if q == 0 else 1.0
            m1 = 0.0 if q == n_qh - 1 else 1.0
            cvec = np.stack([
                np.asarray(conv_in_b, np.float32)[:C],
                np.asarray(conv_in_b, np.float32)[C:],
                lam_re, lam_im, -lam_im, gcre, gcim,
                np.asarray(conv_out_b, np.float32),
                np.full(C, m0, np.float32), np.full(C, 1.0 - m0, np.float32),
                np.full(C, m1, np.float32), np.full(C, 1.0 - m1, np.float32),
                np.full(C, EPS, np.float32),
            ], axis=1)
            bo = np.asarray(conv_out_b, np.float32)
            aux = np.zeros((128, 2), np.float32)
            aux[96:128, 0] = bo[0:32]
            aux[:, 1] = bo[32 + (np.arange(128) % 64)]
            in_maps.append({
                "xh": xh,
                "w_in": w_in_t,
                "w_out": w_out_t,
                "onesw": ones,
                "consts": np.ascontiguousarray(cvec),
                "consts2": aux,
            })
    return in_maps


def prep_core_inputs_k(x, norm_w, conv_in_w, conv_in_b, nu_log, theta_log,
                       c_re, c_im, conv_out_w, conv_out_b, n_qh):
    """Per-core inputs for the K=128-packed program."""
    B, C, T, H, W = x.shape
    HR = H // n_qh
    C2 = 2 * C

    nu = np.exp(np.asarray(nu_log, np.float64))
    theta = np.exp(np.asarray(theta_log, np.float64))
    lam_re = (np.exp(-nu) * np.cos(theta)).astype(np.float32)
    lam_im = (np.exp(-nu) * np.sin(theta)).astype(np.float32)
    gamma = np.sqrt(1.0 - np.exp(-2.0 * nu))
    gcre = (gamma * np.asarray(c_re, np.float64)).astype(np.float32)
    gcim = (gamma * np.asarray(c_im, np.float64)).astype(np.float32)

    w_in_f = np.asarray(conv_in_w, np.float32) * \
        np.asarray(norm_w, np.float32)[None, :, None, None, None]
    # wt[cin, kt, kh, kw, cout]; cout permuted so g-channels (96:192) land
    # first (psum-aligned) and a-channels ride the spare packing slots.
    wt = np.transpose(w_in_f, (1, 2, 3, 4, 0))
    if os.environ.get("KERNEL_AGSWAP", "1") == "1":
        perm = np.concatenate([np.arange(C, 2 * C), np.arange(0, C)])
        wt = np.ascontiguousarray(wt[..., perm])
    wto = np.transpose(np.asarray(conv_out_w, np.float32), (1, 2, 3, 4, 0))

    def pack(w, co):
        """w: [cin, kt, kh, kw, co] -> (wA, wB, wR, wR3, wA0)."""
        wA = np.zeros((128, 9, co), np.float32)
        wB = np.zeros((128, 9, co), np.float32)
        wA0 = np.zeros((96, 9, co), np.float32)
        for j in range(9):
            kh, kw = divmod(j, 3)
            wA[0:96, j] = w[:, 0, kh, kw]
            wA[96:128, j] = w[0:32, 1, kh, kw]
            wB[0:64, j] = w[32:96, 1, kh, kw]
            wB[64:128, j] = w[0:64, 2, kh, kw]
            wA0[0:32, j] = w[0:32, 0, kh, kw] + w[0:32, 1, kh, kw]
            wA0[32:96, j] = w[32:96, 0, kh, kw]
        wR = np.zeros((96, 3, co), np.float32)
        for kh in range(3):
            for kw in range(3):
                wR[32 * kw:32 * (kw + 1), kh] = w[64:96, 2, kh, kw]
        bf = ml_dtypes.bfloat16
        return (np.ascontiguousarray(wA).astype(bf),
                np.ascontiguousarray(wB).astype(bf),
                np.ascontiguousarray(wR).astype(bf),
                np.ascontiguousarray(wA0).astype(bf))

    wA, wB, wR, wA0 = pack(wt, C2)
    if os.environ.get("KERNEL_COUTKP", "1") == "1":
        wKA, wKB, wKR, wKA0 = pack(wto, C)
        wext = {"wKA": wKA, "wKB": wKB, "wKR": wKR, "wKA0": wKA0}
    else:
        wext = {"wK": np.ascontiguousarray(
            wto.reshape(C, 27, C)).astype(ml_dtypes.bfloat16)}
    ones = np.ones((C, 128), ml_dtypes.bfloat16)

    xp = np.concatenate([x[..., -1:], x, x[..., :1]], axis=-1)  # W circular

    in_maps = []
    for b in range(B):
        for q in range(n_qh):
            rows = np.clip(np.arange(q * HR - 2, q * HR + HR + 2), 0, H - 1)
            xh = np.ascontiguousarray(xp[b][:, :, rows, :]).astype(np.float32)
            m0 = 0.0 if q == 0 else 1.0
            m1 = 0.0 if q == n_qh - 1 else 1.0
            cvec = np.stack([
                np.asarray(conv_in_b, np.float32)[:C],
                np.asarray(conv_in_b, np.float32)[C:],
                lam_re, lam_im, -lam_im, gcre, gcim,
                np.asarray(conv_out_b, np.float32),
                np.full(C, m0, np.float32), np.full(C, 1.0 - m0, np.float32),
                np.full(C, m1, np.float32), np.full(C, 1.0 - m1, np.float32),
                np.full(C, EPS, np.float32),
            ], axis=1)
            ba = np.asarray(conv_in_b, np.float32)[:C]
            bo = np.asarray(conv_out_b, np.float32)
            aux_np = np.zeros((128, 8), np.float32)
            aux_np[96:128, 0] = ba[0:32]
            aux_np[0:64, 1] = ba[32:96]
            aux_np[64:128, 1] = ba[32:96]
            aux_np[96:128, 2] = bo[0:32]
            aux_np[0:64, 3] = bo[32:96]
            aux_np[64:128, 4] = bo[32:96]
            in_maps.append({
                "xh": xh,
                "wA": wA, "wB": wB, "wR": wR, "wA0": wA0,
                "onesw": ones,
                "consts": np.ascontiguousarray(cvec),
                "aux128": aux_np,
                **wext,
            })
    return in_maps


LAST_RESULT = None  # BassKernelResults of the most recent kernel() call


def _fix_act_tables():
    """Make Ln/Exp resolve to the combined natural_log_exp_and_others set.

    The act-table-load placement pass picks each activation's first
    containing set; Ln's home (natural_log) differs from Exp's
    (exp_and_others), so an interleaved Ln/Exp stream reloads tables on
    every op (~2.7us each). Removing ln/exp from all other sets (in the
    cached dict, same keys/order, so set ids stay valid) forces both onto
    the one set that holds them together."""
    from concourse.hw_specs import get_activation_tables
    AFt = mybir.ActivationFunctionType
    for arch in ("gen3",):
        try:
            tables = get_activation_tables(arch)
        except Exception:
            continue
        for name, fns in tables.items():
            if name != "natural_log_exp_and_others":
                fns.discard(AFt.Ln)
                fns.discard(AFt.Exp)


def kernel(x, norm_w, conv_in_w, conv_in_b, nu_log, theta_log, c_re, c_im,
           conv_out_w, conv_out_b):
    global LAST_RESULT
    from concourse.bass_utils import run_bass_kernel_spmd

    # KERNEL_ACTFIX=1 crashes the device (walrus/NRT act.json id mismatch);
    # stage_rest's Ln/Exp bursting achieves the same goal safely.
    if os.environ.get("KERNEL_ACTFIX", "0") == "1":
        _fix_act_tables()

    x = np.asarray(x, np.float32)
    B, C, T, H, W = x.shape
    HR = H // QH
    if os.environ.get("KERNEL_KPACK", "1") == "1":
        in_maps = prep_core_inputs_k(x, norm_w, conv_in_w, conv_in_b, nu_log,
                                     theta_log, c_re, c_im, conv_out_w,
                                     conv_out_b, QH)
        nc = build_program_k(
            C=C, T=T, HR=HR, W=W, CT=512,
            cout_kp=os.environ.get("KERNEL_COUTKP", "1") == "1")
    else:
        in_maps = prep_core_inputs(x, norm_w, conv_in_w, conv_in_b, nu_log,
                                   theta_log, c_re, c_im, conv_out_w,
                                   conv_out_b, QH)
        nc = build_program(C=C, T=T, HR=HR, W=W, CT=512,
                           use_silu=os.environ.get("KERNEL_NO_SILU", "") != "1",
                           pack=os.environ.get("KERNEL_PACK", "1") == "1",
                           pack2=os.environ.get("KERNEL_PACK2", "0") == "1")
    trace = os.environ.get("KERNEL_TRACE", "") == "1"
    res = run_bass_kernel_spmd(nc, in_maps, list(range(N_CORES)), trace=trace)
    LAST_RESULT = res
    out = np.empty((B, C, T, H, W), np.float32)
    for core in range(N_CORES):
        b, q = core // QH, core % QH
        out[b, :, :, q * HR:(q + 1) * HR, :] = res.results[core]["out"]
    return out

